# revision 1
# baseline (speedup 1.0000x reference)
"""FEDformer FourierCrossAttention kernel for 8 TRN2 NeuronCores.

Sharding: one head per core (H=8 == n_cores). Each core computes, for its head:
  Q = rfft(q)[:64 modes], K = rfft(k)[:64]      (DFT-as-matmul, hi/lo fp16 3-pass)
  X^T = K^T Q (complex, contract E)             (fp32 matmuls, sign-trick accumulate)
  T = tanh(X) (complex, tau/sin/cos form)       (ACT tanh+sin, DVE cody-waite RR)
  Y = sum_y T[x,y] K[e,y]                       (fp16 matmuls)
  Z = sum_e W[e,o,x] Y[e,x]   (W scaled 2^16)   (fp16 matmuls per mode)
  out = irfft(Z / (512*512))  (G scaled 2^24)   (fp16 matmuls, final copy scale 2^-40)

All host-side prep is numpy-only relayout/casts; all FLOPs run on device.
"""
import numpy as np

import concourse.bass as bass
import concourse.tile as tile
from concourse import bacc, mybir
from concourse.bass_utils import run_bass_kernel_spmd

F32 = mybir.dt.float32
F16 = mybir.dt.float16
AF = mybir.ActivationFunctionType
OP = mybir.AluOpType

B, L, H, E, O, M = 32, 1024, 8, 64, 64, 64
NCHUNK = 8          # contraction chunks of 128 over L
NHALF = 2           # batch halves of 16 for DFT PSUM
WSHIFT = 16         # W scaled by 2^WSHIFT on host
GSHIFT = 24         # G scaled by 2^GSHIFT on host
OUT_SCALE = 2.0 ** (-WSHIFT - GSHIFT)

PI = np.float64(np.pi)
PI_HI = np.float32(3.140625)
PI_MID = np.float32(PI - np.float64(np.float32(3.140625)))
PI_LO = np.float32(PI - np.float64(np.float32(3.140625)) - np.float64(PI_MID))
MAGIC = np.float32(1.5 * 2 ** 23)   # round-to-nearest via add/sub
RH_LIM = np.nextafter(np.float32(np.pi) - np.float32(np.pi / 2), np.float32(0))


def build(debug=False, stages=9):
    nc = bacc.Bacc("TRN2", target_bir_lowering=False, debug=False, num_devices=8)

    # ---- I/O (per-core, host pre-sharded/relaid) ----
    # q/k hi+lo fp16, chunk layout: [c][p][b*64+e] = x[b, 128c+p, e]
    qh_d = nc.dram_tensor("qh", (NCHUNK, 128, B * E), F16, kind="ExternalInput")
    ql_d = nc.dram_tensor("ql", (NCHUNK, 128, B * E), F16, kind="ExternalInput")
    kh_d = nc.dram_tensor("kh", (NCHUNK, 128, B * E), F16, kind="ExternalInput")
    kl_d = nc.dram_tensor("kl", (NCHUNK, 128, B * E), F16, kind="ExternalInput")
    # DFT mats hi+lo fp16: [p][c][2m]: F[128c+p, 0:64]=cos, [64:128]=-sin
    fh_d = nc.dram_tensor("fh", (128, NCHUNK, 2 * M), F16, kind="ExternalInput")
    fl_d = nc.dram_tensor("fl", (128, NCHUNK, 2 * M), F16, kind="ExternalInput")
    # W packed fp16 (x2^16): [e][x][o]=Wr[e,o,x], [e][x][64+o]=Wi[e,o,x]
    w_d = nc.dram_tensor("wp", (E, M, 2 * O), F16, kind="ExternalInput")
    # irfft mats fp16 (x2^24): [0:64][l]=cm*cos(2pi m l/N)*S, [64:128][l]=-cm*sin(...)*S
    g_d = nc.dram_tensor("gp", (2 * M, L), F16, kind="ExternalInput")
    # transpose helpers fp32
    idq_d = nc.dram_tensor("idq", (128, 128), F32, kind="ExternalInput")  # I
    idk_d = nc.dram_tensor("idk", (128, 128), F32, kind="ExternalInput")  # [[I,0],[0,-I]]

    out_d = nc.dram_tensor("out", (B, O, L), F32, kind="ExternalOutput")
    dbg = {}
    if debug:
        assert stages >= 9
        for nm, shp, dt_ in (("d_qm", (128, B * E), F32), ("d_km", (128, B * E), F32),
                             ("d_qe", (E, B, 128), F32), ("d_ke", (E, B, 128), F32),
                             ("d_a", (128, B // 2, M), F32), ("d_b", (128, B // 2, M), F32),
                             ("d_t", (128, B // 2, 128), F16), ("d_y", (E, B, 2, M), F16),
                             ("d_z", (O, B, 2, M), F16), ("d_zp", (128, B, O), F16)):
            dbg[nm] = nc.dram_tensor(nm, shp, dt_, kind="ExternalOutput")

    with tile.TileContext(nc) as tc:
        from contextlib import ExitStack
        stack = ExitStack()
        with stack:
            consts = stack.enter_context(tc.tile_pool(name="consts", bufs=1))
            chunks = stack.enter_context(tc.tile_pool(name="chunks", bufs=3))
            coeff = stack.enter_context(tc.tile_pool(name="coeff", bufs=1))
            work = stack.enter_context(tc.tile_pool(name="work", bufs=1))
            tmp = stack.enter_context(tc.tile_pool(name="tmp", bufs=1))
            outs = stack.enter_context(tc.tile_pool(name="outs", bufs=4))
            ps_stack = ExitStack()
            dft_ps = ps_stack.enter_context(tc.tile_pool(name="dft_ps", bufs=1, space="PSUM"))

            # ---------- constants ----------
            fh_t = consts.tile([128, NCHUNK, 2 * M], F16, tag="fh")
            fl_t = consts.tile([128, NCHUNK, 2 * M], F16, tag="fl")
            w_t = consts.tile([E, M, 2 * O], F16, tag="w")
            g_t = consts.tile([2 * M, L], F16, tag="g")
            idq_t = consts.tile([128, 128], F32, tag="idq")
            idk_t = consts.tile([128, 128], F32, tag="idk")
            nc.sync.dma_start(out=fh_t, in_=fh_d[:])
            nc.sync.dma_start(out=fl_t, in_=fl_d[:])
            nc.sync.dma_start(out=w_t, in_=w_d[:])
            nc.sync.dma_start(out=g_t, in_=g_d[:])
            nc.sync.dma_start(out=idq_t, in_=idq_d[:])
            nc.sync.dma_start(out=idk_t, in_=idk_d[:])

            # ---------- stage 1+2: DFT (hi/lo 3-pass), half-0 first ----------
            # DFT uses only 4 PSUM banks (one b-half at a time, tag-reused) so
            # the transpose/attn1 pools below fit alongside and half 0's
            # downstream work genuinely overlaps half 1's DFT.
            tp_ps = ps_stack.enter_context(tc.tile_pool(name="tp_ps", bufs=2, space="PSUM"))
            at_ps = ps_stack.enter_context(tc.tile_pool(name="at_ps", bufs=2, space="PSUM"))
            qm_h = [coeff.tile([128, 1024], F32, tag=f"qmh{hf}", name=f"qm_h{hf}") for hf in range(NHALF)]
            km_h = [coeff.tile([128, 1024], F32, tag=f"kmh{hf}", name=f"km_h{hf}") for hf in range(NHALF)]
            km16_t = coeff.tile([128, B, E], F16, tag="km16")

            for hf in range(NHALF):
                qm_ps = dft_ps.tile([128, 1024], F32, tag="qmps", name=f"qm_ps{hf}", bufs=1)
                km_ps = dft_ps.tile([128, 1024], F32, tag="kmps", name=f"km_ps{hf}", bufs=1)
                csl = slice(hf * 1024, (hf + 1) * 1024)
                for c in range(NCHUNK):
                    qh_c = chunks.tile([128, 1024], F16, tag=f"qh{hf}", name=f"qh{hf}_{c}")
                    ql_c = chunks.tile([128, 1024], F16, tag=f"ql{hf}", name=f"ql{hf}_{c}")
                    kh_c = chunks.tile([128, 1024], F16, tag=f"kh{hf}", name=f"kh{hf}_{c}")
                    kl_c = chunks.tile([128, 1024], F16, tag=f"kl{hf}", name=f"kl{hf}_{c}")
                    nc.sync.dma_start(out=qh_c, in_=qh_d[c][:, csl])
                    nc.sync.dma_start(out=ql_c, in_=ql_d[c][:, csl])
                    nc.sync.dma_start(out=kh_c, in_=kh_d[c][:, csl])
                    nc.sync.dma_start(out=kl_c, in_=kl_d[c][:, csl])
                    first = c == 0
                    last = c == NCHUNK - 1
                    passes = (
                        (fh_t[:, c, :], qh_c, qm_ps, first, False),
                        (fh_t[:, c, :], ql_c, qm_ps, False, False),
                        (fh_t[:, c, :], kh_c, km_ps, first, False),
                        (fh_t[:, c, :], kl_c, km_ps, False, False),
                        (fl_t[:, c, :], qh_c, qm_ps, False, last),
                        (fl_t[:, c, :], kh_c, km_ps, False, last),
                    )
                    for lhs, rhs_t, ps, is_start, is_stop in passes:
                        for g in range(2):
                            nc.tensor.matmul(
                                ps[:, g * 512:(g + 1) * 512],
                                lhs,
                                rhs_t[:, g * 512:(g + 1) * 512],
                                start=is_start,
                                stop=is_stop,
                            )
                nc.vector.tensor_copy(qm_h[hf][:], qm_ps[:])
                nc.scalar.copy(km_h[hf][:], km_ps[:])
                nc.vector.tensor_copy(
                    km16_t[:, hf * 16:(hf + 1) * 16, :],
                    km_ps[:].rearrange("p (b e) -> p b e", e=E),
                )

            if stages >= 3:
                # ---------- stage 3: pair transposes -> Q_e, K_e (per half) ----------
                # in [2m, (b0-e|b1-e)] -> out [(b0-e|b1-e), 2m]; even b on
                # partitions 0:64, odd on 64:128; split per half for overlap.
                qe_h = [work.tile([128, 8, 128], F32, tag=f"qeh{hf}", name=f"qe_h{hf}") for hf in range(NHALF)]
                ke_h = [work.tile([128, 8, 128], F32, tag=f"keh{hf}", name=f"ke_h{hf}") for hf in range(NHALF)]
                qf_h = [work.tile([128, 8, 128], F32, tag=f"qfh{hf}", name=f"qf_h{hf}") for hf in range(NHALF)]
                for hf in range(NHALF):
                    qm_p = qm_h[hf][:].rearrange("p (bp c) -> p bp c", c=128)
                    km_p = km_h[hf][:].rearrange("p (bp c) -> p bp c", c=128)
                    for g2 in range(4):
                        tp = tp_ps.tile([128, 2, 128], F32, tag="tp")
                        tk = tp_ps.tile([128, 2, 128], F32, tag="tp", name=f"tk{hf}_{g2}")
                        for j in range(2):
                            bpl = g2 * 2 + j
                            nc.tensor.transpose(tp[:, j, :], qm_p[:, bpl, :], idq_t[:])
                            nc.tensor.transpose(tk[:, j, :], km_p[:, bpl, :], idq_t[:])
                        if g2 % 2 == 0:
                            nc.scalar.copy(qe_h[hf][:, g2 * 2:(g2 + 1) * 2, :], tp[:])
                            nc.scalar.copy(ke_h[hf][:, g2 * 2:(g2 + 1) * 2, :], tk[:])
                        else:
                            nc.vector.tensor_copy(qe_h[hf][:, g2 * 2:(g2 + 1) * 2, :], tp[:])
                            nc.vector.tensor_copy(ke_h[hf][:, g2 * 2:(g2 + 1) * 2, :], tk[:])
                    nc.vector.tensor_scalar_mul(qf_h[hf][:, :, 0:64], qe_h[hf][:, :, 64:128], -1.0)
                    nc.vector.tensor_copy(qf_h[hf][:, :, 64:128], qe_h[hf][:, :, 0:64])

                # ---------- stage 4: attn1 -> X^T psum, A/B fp32 sbuf ----------
                # per b: P_b[y, (XTr|XTi)] = Kr^T [Qr|Qi] + Ki^T [-Qi|Qr]
                # operands at base 64*(b%2); separate psum banks per base.
                a_t = work.tile([128, B // 2, M], F32, tag="a")
                b_t = work.tile([128, B // 2, M], F32, tag="b")
                for hf in range(NHALF):
                    for par in range(2):
                        base = 64 * par
                        for g4 in range(2):
                            pt = at_ps.tile([128, 4, 128], F32, tag="pt", bufs=2)
                            for j in range(4):
                                bpl = g4 * 4 + j
                                po = pt[base:base + 64, j, :]
                                ke_b = ke_h[hf][base:base + 64, bpl, :]
                                qe_b = qe_h[hf][base:base + 64, bpl, :]
                                qf_b = qf_h[hf][base:base + 64, bpl, :]
                                nc.tensor.matmul(po, ke_b[:, 0:64], qe_b, start=True, stop=False)
                                nc.tensor.matmul(po, ke_b[:, 64:128], qf_b, start=False, stop=True)
                            gsl = slice(hf * 8 + g4 * 4, hf * 8 + (g4 + 1) * 4)
                            sl = slice(base, base + 64)
                            if par == 0:
                                nc.scalar.copy(a_t[sl, gsl, :], pt[sl, :, 0:64])
                                nc.scalar.copy(b_t[sl, gsl, :], pt[sl, :, 64:128])
                            else:
                                nc.vector.tensor_copy(a_t[sl, gsl, :], pt[sl, :, 0:64])
                                nc.vector.tensor_copy(b_t[sl, gsl, :], pt[sl, :, 64:128])

                # ---------- stage 5: complex tanh (tau form) ----------
                # A=Re X^T, B=Im X^T, both [128, 1024] fp32 (b-pair packed partitions)
                av = a_t[:].rearrange("p b m -> p (b m)")
                bv = b_t[:].rearrange("p b m -> p (b m)")
                halfpi = consts.tile([128, 1], F32, tag="halfpi", name="halfpi")
                nc.vector.memset(halfpi[:], float(np.pi / 2))
                def ctt(n):
                    return tmp.tile([128, 1024], F32, tag="ct", name=f"ct_{n}", bufs=6)
                ct_n = ctt("n")
                nc.vector.tensor_scalar(ct_n[:], bv, float(1.0 / PI), float(MAGIC), OP.mult, OP.add)
                nc.vector.tensor_scalar_sub(ct_n[:], ct_n[:], float(MAGIC))
                ct_rh = ctt("rh")
                nc.vector.cody_waite_cascade(ct_rh[:], bv, ct_n[:], float(PI_HI), float(PI_MID), float(PI_LO))
                # clamp |rh| so rh+pi/2 (cos path) and 2*rh (sin path) stay in [-pi, pi]
                nc.vector.tensor_scalar(ct_rh[:], ct_rh[:], -float(RH_LIM), float(RH_LIM), OP.max, OP.min)
                ct_tau = ctt("tau")
                nc.scalar.activation(ct_tau[:], av, AF.Tanh)
                ct_s = ctt("s")
                nc.scalar.activation(ct_s[:], ct_rh[:], AF.Sin)
                ct_c = ctt("c")
                nc.scalar.activation(ct_c[:], ct_rh[:], AF.Sin, bias=halfpi[:])
                ct_s2 = ctt("s2")
                nc.scalar.activation(ct_s2[:], ct_s[:], AF.Square)
                ct_c2 = ctt("c2")
                nc.scalar.activation(ct_c2[:], ct_c[:], AF.Square)
                ct_sc = ctt("sc")
                nc.vector.tensor_mul(ct_sc[:], ct_s[:], ct_c[:])
                ct_t2 = ctt("t2")
                nc.scalar.activation(ct_t2[:], ct_tau[:], AF.Square)
                ct_d = ctt("d")
                nc.vector.tensor_mul(ct_d[:], ct_t2[:], ct_s2[:])
                nc.vector.tensor_add(ct_d[:], ct_d[:], ct_c2[:])
                ct_r = ctt("r")
                nc.vector.reciprocal(ct_r[:], ct_d[:])
                nc.vector.tensor_scalar(ct_t2[:], ct_t2[:], -1.0, 1.0, OP.mult, OP.add)
                ct_u = ctt("u")
                nc.vector.tensor_mul(ct_u[:], ct_sc[:], ct_t2[:])
                # T = [Tr | Ti] fp16 ; Tf = [-Ti | Tr]
                t_t = work.tile([128, B // 2, 128], F16, tag="t")
                tf_t = work.tile([128, B // 2, 128], F16, tag="tf")
                tau_v = ct_tau[:].rearrange("p (b m) -> p b m", m=M)
                u_v = ct_u[:].rearrange("p (b m) -> p b m", m=M)
                r_v = ct_r[:].rearrange("p (b m) -> p b m", m=M)
                nc.vector.tensor_mul(t_t[:, :, 0:64], tau_v, r_v)
                nc.vector.tensor_mul(t_t[:, :, 64:128], u_v, r_v)
                nc.vector.tensor_scalar_mul(tf_t[:, :, 0:64], t_t[:, :, 64:128], -1.0)
                nc.vector.tensor_copy(tf_t[:, :, 64:128], t_t[:, :, 0:64])

            if stages >= 6:
                ps_stack.close()
                ps_stack = ExitStack()
                sm_ps = ps_stack.enter_context(tc.tile_pool(name="sm_ps", bufs=2, space="PSUM"))
                tz_ps = ps_stack.enter_context(tc.tile_pool(name="tz_ps", bufs=2, space="PSUM"))

                # ---------- stage 6: attn2 -> Y fp16 [e, (u, ri, x)] ----------
                # one K=128 matmul per b: km16 is [Kr;Ki] partition-stacked, so
                # Yr|Yi = [Kr;Ki]^T @ [T;Tf].  Assemble TT = [T_b (0:64); Tf_b
                # (64:128)] per b: parity-matched halves via DVE, the other two
                # via partition-shifting SBUF DMAs.
                tt_t = work.tile([128, B, 128], F16, tag="tt")
                tt_v = tt_t[:].rearrange("p (b2 par) c -> p b2 par c", par=2)
                nc.vector.tensor_copy(tt_v[0:64, :, 0, :], t_t[0:64, :, :])
                nc.vector.tensor_copy(tt_v[64:128, :, 1, :], tf_t[64:128, :, :])
                nc.sync.dma_start(out=tt_v[0:64, :, 1, :], in_=t_t[64:128, :, :])
                nc.sync.dma_start(out=tt_v[64:128, :, 0, :], in_=tf_t[0:64, :, :])
                # Y batch axis stays parity-major: u = (b%2)*16 + b//2
                y_t = work.tile([E, B, 2, M], F16, tag="y")
                for u4 in range(B // 4):
                    yp = sm_ps.tile([E, 4, 128], F32, tag="yp")
                    for j in range(4):
                        u = u4 * 4 + j
                        b = 2 * (u % 16) + (u // 16)
                        nc.tensor.matmul(yp[:, j, :], km16_t[:, b, :], tt_t[:, b, :],
                                         start=True, stop=True)
                    dst = y_t[:, u4 * 4:(u4 + 1) * 4, :, :]
                    srcv = yp[:].rearrange("p b (ri m) -> p b ri m", m=M)
                    if u4 % 2 == 0:
                        nc.scalar.copy(dst, srcv)
                    else:
                        nc.vector.tensor_copy(dst, srcv)

            if stages >= 7:
                # ---------- stage 7: weights -> Z fp16 [o, (b, ri, x)] ----------
                # combine needs both PSUM partition halves on the same lanes:
                # stage full psum to SBUF, DMA-shift the upper half down, combine.
                z_t = work.tile([O, B, 2, M], F16, tag="z")
                for x8 in range(M // 8):
                    wp = sm_ps.tile([128, 8, B * 2], F32, tag="wp")
                    for j in range(8):
                        x = x8 * 8 + j
                        nc.tensor.matmul(
                            wp[:, j, :],
                            w_t[:, x, :],
                            y_t[:, :, :, x].rearrange("p b ri -> p (b ri)"),
                            start=True, stop=True,
                        )
                    wsb = outs.tile([128, 8, B * 2], F32, tag="wsb", name=f"wsb{x8}")
                    if x8 % 2 == 0:
                        nc.scalar.copy(wsb[:], wp[:])
                    else:
                        nc.vector.tensor_copy(wsb[:], wp[:])
                    wlo = outs.tile([E, 8, B * 2], F32, tag="wlo", name=f"wlo{x8}")
                    nc.sync.dma_start(out=wlo[:], in_=wsb[64:128, :, :])
                    wv0 = wsb[:].rearrange("p x (b ri) -> p x b ri", ri=2)
                    wv1 = wlo[:].rearrange("p x (b ri) -> p x b ri", ri=2)
                    # Zr = hi[o, b, 0] - lo[o, b, 1] ; Zi = hi[o, b, 1] + lo[o, b, 0]
                    nc.vector.tensor_tensor(
                        z_t[:, :, 0, x8 * 8:(x8 + 1) * 8].rearrange("p b x -> p x b"),
                        wv0[0:64, :, :, 0], wv1[:, :, :, 1], OP.subtract)
                    nc.vector.tensor_tensor(
                        z_t[:, :, 1, x8 * 8:(x8 + 1) * 8].rearrange("p b x -> p x b"),
                        wv0[0:64, :, :, 1], wv1[:, :, :, 0], OP.add)

            if stages >= 8:
                # ---------- stage 8: Z transposes -> Z' fp16 [(ri,x), (b, o)] ----------
                zp_g = [work.tile([128, 8, O], F16, tag=f"zp{g}", name=f"zp_g{g}")
                        for g in range(B // 8)]
                idk16 = consts.tile([64, 64], F16, tag="id16")
                nc.vector.tensor_copy(idk16[:], idq_t[0:64, 0:64])
                for b8 in range(B // 8):
                    zt = tz_ps.tile([128, 8, O], F16, tag="zt")
                    for j in range(8):
                        b = b8 * 8 + j
                        nc.tensor.transpose(
                            zt[:, j, :],
                            z_t[:, b, :, :].rearrange("p ri m -> p (ri m)"),
                            idk16[:],
                        )
                    nc.vector.tensor_copy(zp_g[b8][:], zt[:])

            if stages >= 9:
                # ---------- stage 9: irfft + scaled output ----------
                # per b-pair, per l-half: out[(b0-o|b1-o), 512] = Z'[:, pair]^T @ G
                for bp in range(B // 2):
                    for g in range(2):
                        opg = sm_ps.tile([128, 512], F32, tag="op")
                        nc.tensor.matmul(
                            opg[:, :],
                            zp_g[bp // 4][:, (bp % 4) * 2:(bp % 4) * 2 + 2, :]
                            .rearrange("p b o -> p (b o)"),
                            g_t[:, g * 512:(g + 1) * 512],
                            start=True, stop=True,
                        )
                        otg = outs.tile([128, 512], F32, tag="ot")
                        if bp % 2 == 0:
                            nc.scalar.mul(otg[:], opg[:], float(OUT_SCALE))
                        else:
                            nc.vector.tensor_scalar_mul(otg[:], opg[:], float(OUT_SCALE))
                        for half in range(2):
                            u = bp * 2 + half
                            b = 2 * (u % 16) + (u // 16)   # parity-major -> b
                            nc.sync.dma_start(
                                out=out_d[b, :, g * 512:(g + 1) * 512],
                                in_=otg[half * 64:(half + 1) * 64, :],
                            )
            if debug:
                for nm, t in (("d_qm", qm_t), ("d_km", km_t), ("d_qe", qe_t),
                              ("d_ke", ke_t), ("d_a", a_t), ("d_b", b_t),
                              ("d_t", t_t), ("d_y", y_t), ("d_z", z_t)):
                    nc.sync.dma_start(out=dbg[nm][:], in_=t[:])
            ps_stack.close()

    nc.compile()
    return nc


_NC_CACHE = None


def _get_nc():
    global _NC_CACHE
    if _NC_CACHE is None:
        _NC_CACHE = build()
    return _NC_CACHE


def _host_prep(q, k, Wr, Wi):
    """Build the 8 per-core input maps (numpy relayout/cast only)."""
    l = np.arange(L, dtype=np.float64)[:, None]
    m = np.arange(M, dtype=np.float64)[None, :]
    ang = 2.0 * np.pi * l * m / L
    F = np.concatenate([np.cos(ang), -np.sin(ang)], axis=1).astype(np.float32)  # [L, 2M]
    fh = F.astype(np.float16)
    fl = (F - fh.astype(np.float32)).astype(np.float16)
    fh = fh.reshape(NCHUNK, 128, 2 * M).transpose(1, 0, 2).copy()
    fl = fl.reshape(NCHUNK, 128, 2 * M).transpose(1, 0, 2).copy()

    cm = np.full(M, 2.0); cm[0] = 1.0
    ang2 = 2.0 * np.pi * m.T * np.arange(L, dtype=np.float64)[None, :] / L
    SC = 2.0 ** GSHIFT / (L * 512.0 * 512.0)
    g = np.concatenate([
        cm[:, None] * np.cos(ang2) * SC,
        -cm[:, None] * np.sin(ang2) * SC,
    ], axis=0).astype(np.float32).astype(np.float16)  # [2M, L]

    idq = np.eye(128, dtype=np.float32)
    idk = np.eye(128, dtype=np.float32)
    idk[64:, 64:] *= -1.0

    maps = []
    for h in range(H):
        def split(x):
            xs = np.ascontiguousarray(x[:, :, h, :].transpose(1, 0, 2)).reshape(L, B * E)
            hi = xs.astype(np.float16)
            lo = (xs - hi.astype(np.float32)).astype(np.float16)
            return (hi.reshape(NCHUNK, 128, B * E).copy(),
                    lo.reshape(NCHUNK, 128, B * E).copy())
        qh, ql = split(q)
        kh, kl = split(k)
        wpk = np.empty((E, M, 2 * O), np.float32)
        wpk[:, :, 0:O] = (Wr[h] * 2.0 ** WSHIFT).transpose(0, 2, 1)  # [e,o,x]->[e,x,o]
        wpk[:, :, O:] = (Wi[h] * 2.0 ** WSHIFT).transpose(0, 2, 1)
        maps.append({
            "qh": qh, "ql": ql, "kh": kh, "kl": kl,
            "fh": fh, "fl": fl,
            "wp": wpk.astype(np.float16),
            "gp": g,
            "idq": idq, "idk": idk,
        })
    return maps


def kernel(q, k, v, Wr, Wi, _trace=False):
    q = np.asarray(q, np.float32)
    k = np.asarray(k, np.float32)
    Wr = np.asarray(Wr, np.float32)
    Wi = np.asarray(Wi, np.float32)
    nc = _get_nc()
    maps = _host_prep(q, k, Wr, Wi)
    try:
        res = run_bass_kernel_spmd(nc, maps, core_ids=list(range(H)), trace=_trace)
    except ModuleNotFoundError:
        res = run_bass_kernel_spmd(nc, maps, core_ids=list(range(H)), trace=False)
    out = np.stack([res.results[h]["out"] for h in range(H)], axis=1)  # [B,H,O,L]
    if _trace:
        kernel.last_results = res
    return out.astype(np.float32)



# revision 15
# speedup vs baseline: 1.1328x; 1.1328x over previous
"""FEDformer FourierCrossAttention kernel for 8 TRN2 NeuronCores.

Sharding: one head per core (H=8 == n_cores). Each core computes, for its head:
  Q = rfft(q)[:64 modes], K = rfft(k)[:64]      (DFT-as-matmul, hi/lo fp16 3-pass)
  X^T = K^T Q (complex, contract E)             (2-batch 256-col matmuls)
  T = tanh(X) (complex, tau/sin/cos form)       (ACT tanh+sin, DVE cody-waite RR)
  Y = sum_y T[x,y] K[e,y]                       (fp16 matmuls)
  Z = sum_e W[e,o,x] Y[e,x]   (W scaled 2^16)   (dual-accumulate Wr/Wi fp16 matmuls)
  out = irfft(Z / (512*512))  (G scaled 2^24)   (fp16 matmuls, scale 2^-40 on copy)

DMA layout is optimized for the TRN2 DGE model: few, large, contiguous
transfers. Inputs are packed per (chunk, batch-half) into single tiles;
output is written per b-pair in fp16 and reordered/upcast on the host.

Batch indexing: global b = 16*hf + 4*g4 + 2*j + par, stored in the attn/tanh
stages at partition half j (pair LSB) and column group cg = 8*hf + 2*g4 + par.
"""
import numpy as np

import concourse.bass as bass
import concourse.tile as tile
from concourse import bacc, mybir
from concourse.bass_utils import run_bass_kernel_spmd

F32 = mybir.dt.float32
F16 = mybir.dt.float16
AF = mybir.ActivationFunctionType
OP = mybir.AluOpType

B, L, H, E, O, M = 32, 1024, 8, 64, 64, 64
NCHUNK = 8          # contraction chunks of 128 over L
NHALF = 2           # batch halves of 16 for DFT PSUM
WSHIFT = 16         # W scaled by 2^WSHIFT on host
GSHIFT = 24         # G scaled by 2^GSHIFT on host
OUT_SCALE = 2.0 ** (-WSHIFT - GSHIFT)
HB = B // NHALF     # 16 batches per half

PI = np.float64(np.pi)
PI_HI = np.float32(3.140625)
PI_MID = np.float32(PI - np.float64(np.float32(3.140625)))
PI_LO = np.float32(PI - np.float64(np.float32(3.140625)) - np.float64(PI_MID))
MAGIC = np.float32(1.5 * 2 ** 23)   # round-to-nearest via add/sub
RH_LIM = np.nextafter(np.float32(np.pi) - np.float32(np.pi / 2), np.float32(0))


def build(debug=False):
    nc = bacc.Bacc("TRN2", target_bir_lowering=False, debug=False, num_devices=8)

    # ---- I/O (per-core, host pre-sharded/relaid) ----
    # packed q/k hi+lo fp16: [c][hf][p][t][col]; t in {qh, ql, kh, kl},
    # col = b_local*64 + e  (b_local = b - 16*hf)
    x_d = nc.dram_tensor("xp", (NCHUNK, NHALF, 128, 4, HB * E), F16,
                         kind="ExternalInput")
    # packed fp16 consts: [p][fh(8*128) | fl(8*128) | g(1024)]
    c_d = nc.dram_tensor("cp", (128, 3 * 1024), F16, kind="ExternalInput")
    # W packed fp16 (x2^16): [e][ri][x][o] = W{ri}[e, o, x]
    w_d = nc.dram_tensor("wp", (E, 2, M, O), F16, kind="ExternalInput")
    # transpose helper fp32
    idq_d = nc.dram_tensor("idq", (128, 128), F32, kind="ExternalInput")

    # out[bp][p][l]: p = (b-pair half)*64 + o; u = 2*bp + (p>=64)
    out_d = nc.dram_tensor("out", (B // 2, 128, L), F16, kind="ExternalOutput")

    with tile.TileContext(nc) as tc:
        from contextlib import ExitStack
        stack = ExitStack()
        with stack:
            consts = stack.enter_context(tc.tile_pool(name="consts", bufs=1))
            chunks = stack.enter_context(tc.tile_pool(name="chunks", bufs=3))
            coeff = stack.enter_context(tc.tile_pool(name="coeff", bufs=1))
            work = stack.enter_context(tc.tile_pool(name="work", bufs=1))
            tmp = stack.enter_context(tc.tile_pool(name="tmp", bufs=1))
            outs = stack.enter_context(tc.tile_pool(name="outs", bufs=4))
            ps_stack = ExitStack()
            dft_ps = ps_stack.enter_context(tc.tile_pool(name="dft_ps", bufs=1, space="PSUM"))
            tp_ps = ps_stack.enter_context(tc.tile_pool(name="tp_ps", bufs=2, space="PSUM"))
            at_ps = ps_stack.enter_context(tc.tile_pool(name="at_ps", bufs=2, space="PSUM"))

            # ---------- constants (3 DMAs spread over scalar/vector queues) ----------
            c_t = consts.tile([128, 3 * 1024], F16, tag="cp")
            w_t = consts.tile([E, 2, M, O], F16, tag="w")
            idq_t = consts.tile([128, 128], F32, tag="idq")
            nc.scalar.dma_start(out=c_t, in_=c_d[:])
            nc.scalar.dma_start(out=w_t, in_=w_d[:])
            nc.scalar.dma_start(out=idq_t, in_=idq_d[:])
            fh_t = c_t[:, 0:1024].rearrange("p (c m) -> p c m", m=2 * M)
            fl_t = c_t[:, 1024:2048].rearrange("p (c m) -> p c m", m=2 * M)
            g_t = c_t[:, 2048:3072]

            # ---------- per-half state ----------
            qm_h = [coeff.tile([128, 1024], F32, tag=f"qmh{hf}", name=f"qm_h{hf}")
                    for hf in range(NHALF)]
            km_h = [coeff.tile([128, 1024], F32, tag=f"kmh{hf}", name=f"km_h{hf}")
                    for hf in range(NHALF)]
            km16_t = coeff.tile([128, B, E], F16, tag="km16")
            # layout [p=(par,e), ri, bp, y] so stationary attn1 slices merge
            # into a single contiguous free dim (BIR requirement)
            qe_h = [work.tile([128, 2, 8, 64], F32, tag=f"qeh{hf}", name=f"qe_h{hf}") for hf in range(NHALF)]
            ke_h = [work.tile([128, 2, 8, 64], F32, tag=f"keh{hf}", name=f"ke_h{hf}") for hf in range(NHALF)]
            qf_h = [work.tile([128, 2, 8, 64], F32, tag=f"qfh{hf}", name=f"qf_h{hf}") for hf in range(NHALF)]
            # A/B packed: ab[p = 64*j + y, cg, 0:64 = Re X^T, 64:128 = Im X^T]
            ab_t = work.tile([128, 16, 128], F32, tag="ab")

            for hf in range(NHALF):
                # ---------- stage 1+2: DFT (hi/lo 3-pass) ----------
                qm_ps = dft_ps.tile([128, 1024], F32, tag="qmps", name=f"qm_ps{hf}", bufs=1)
                km_ps = dft_ps.tile([128, 1024], F32, tag="kmps", name=f"km_ps{hf}", bufs=1)
                for c in range(NCHUNK):
                    x_c = chunks.tile([128, 4, HB * E], F16, tag="x", name=f"x{hf}_{c}")
                    nc.sync.dma_start(out=x_c, in_=x_d[c, hf])
                    first = c == 0
                    last = c == NCHUNK - 1
                    passes = (
                        (fh_t[:, c, :], 0, qm_ps, first, False),
                        (fh_t[:, c, :], 1, qm_ps, False, False),
                        (fh_t[:, c, :], 2, km_ps, first, False),
                        (fh_t[:, c, :], 3, km_ps, False, False),
                        (fl_t[:, c, :], 0, qm_ps, False, last),
                        (fl_t[:, c, :], 2, km_ps, False, last),
                    )
                    for lhs, ti, ps, is_start, is_stop in passes:
                        for g in range(2):
                            nc.tensor.matmul(
                                ps[:, g * 512:(g + 1) * 512],
                                lhs,
                                x_c[:, ti, g * 512:(g + 1) * 512],
                                start=is_start,
                                stop=is_stop,
                            )
                nc.vector.tensor_copy(qm_h[hf][:], qm_ps[:])
                nc.scalar.copy(km_h[hf][:], km_ps[:])
                nc.vector.tensor_copy(
                    km16_t[:, hf * HB:(hf + 1) * HB, :],
                    km_ps[:].rearrange("p (b e) -> p b e", e=E),
                )

                # ---------- stage 3: pair transposes -> Q_e, K_e ----------
                # in [2m, (b0-e|b1-e)] -> out [(b0-e|b1-e), 2m]; even b on
                # partitions 0:64, odd on 64:128.
                qm_p = qm_h[hf][:].rearrange("p (bp c) -> p bp c", c=128)
                km_p = km_h[hf][:].rearrange("p (bp c) -> p bp c", c=128)
                for g2 in range(4):
                    tp = tp_ps.tile([128, 2, 128], F32, tag="tp")
                    tk = tp_ps.tile([128, 2, 128], F32, tag="tp", name=f"tk{hf}_{g2}")
                    for j in range(2):
                        bpl = g2 * 2 + j
                        nc.tensor.transpose(tp[:, j, :], qm_p[:, bpl, :], idq_t[:])
                        nc.tensor.transpose(tk[:, j, :], km_p[:, bpl, :], idq_t[:])
                    tpv = tp[:].rearrange("p j (ri y) -> p ri j y", ri=2)
                    tkv = tk[:].rearrange("p j (ri y) -> p ri j y", ri=2)
                    if g2 % 2 == 0:
                        nc.scalar.copy(qe_h[hf][:, :, g2 * 2:(g2 + 1) * 2, :], tpv)
                        nc.scalar.copy(ke_h[hf][:, :, g2 * 2:(g2 + 1) * 2, :], tkv)
                    else:
                        nc.vector.tensor_copy(qe_h[hf][:, :, g2 * 2:(g2 + 1) * 2, :], tpv)
                        nc.vector.tensor_copy(ke_h[hf][:, :, g2 * 2:(g2 + 1) * 2, :], tkv)
                nc.vector.tensor_scalar_mul(qf_h[hf][:, 0, :, :], qe_h[hf][:, 1, :, :], -1.0)
                nc.vector.tensor_copy(qf_h[hf][:, 1, :, :], qe_h[hf][:, 0, :, :])

                # ---------- stage 4: attn1 -> X^T psum, A/B fp32 sbuf ----------
                # 2 same-parity b per matmul pair (256 cols each):
                #   pt[(j, y), (j', ri, x)] = sum_e Kr_bj[e, y] Q_bj'[e, (ri x)]
                #                           + sum_e Ki_bj[e, y] Qf_bj'[e, (ri x)]
                # useful quadrants j == j'; partition-aligned extraction.
                for par in range(2):
                    base = 64 * par
                    sl = slice(base, base + 64)
                    for g4 in range(4):
                        # pairs at this parity: batch pair index pr = 2*g4 + j,
                        # living at ke[64*par:, :, pr, :]; take pr = 2*g4, 2*g4+1.
                        # out pt[(j, y), (ri', j', y')]; useful quadrants j==j'.
                        pt = at_ps.tile([128, 2, 2, 64], F32, tag="pt", bufs=2)
                        psl = slice(2 * g4, 2 * g4 + 2)
                        nc.tensor.matmul(pt[:], ke_h[hf][sl, 0, psl, :],
                                         qe_h[hf][sl, :, psl, :],
                                         start=True, stop=False)
                        nc.tensor.matmul(pt[:], ke_h[hf][sl, 1, psl, :],
                                         qf_h[hf][sl, :, psl, :],
                                         start=False, stop=True)
                        cg = 8 * hf + 2 * g4 + par
                        if (par + g4) % 2 == 0:
                            nc.scalar.copy(ab_t[0:64, cg, :].rearrange("p (ri y) -> p ri y", ri=2), pt[0:64, :, 0, :])
                            nc.vector.tensor_copy(ab_t[64:128, cg, :].rearrange("p (ri y) -> p ri y", ri=2), pt[64:128, :, 1, :])
                        else:
                            nc.vector.tensor_copy(ab_t[0:64, cg, :].rearrange("p (ri y) -> p ri y", ri=2), pt[0:64, :, 0, :])
                            nc.scalar.copy(ab_t[64:128, cg, :].rearrange("p (ri y) -> p ri y", ri=2), pt[64:128, :, 1, :])

            # ---------- stage 5: complex tanh (tau form) ----------
            # A=Re X^T, B=Im X^T, strided views of ab_t [128, 16, 64]
            av = ab_t[:, :, 0:64]
            bv = ab_t[:, :, 64:128]
            halfpi = consts.tile([128, 1], F32, tag="halfpi", name="halfpi")
            nc.vector.memset(halfpi[:], float(np.pi / 2))
            def ctt(n):
                return tmp.tile([128, 1024], F32, tag="ct", name=f"ct_{n}", bufs=6)
            def v2(t):
                return t[:].rearrange("p (g m) -> p g m", m=64)
            ct_n = ctt("n")
            nc.vector.tensor_scalar(v2(ct_n), bv, float(1.0 / PI), float(MAGIC), OP.mult, OP.add)
            nc.vector.tensor_scalar_sub(ct_n[:], ct_n[:], float(MAGIC))
            ct_rh = ctt("rh")
            nc.vector.cody_waite_cascade(v2(ct_rh), bv, ct_n[:], float(PI_HI), float(PI_MID), float(PI_LO))
            # clamp |rh| so rh+pi/2 (cos path) and 2*rh (sin path) stay in [-pi, pi]
            nc.vector.tensor_scalar(ct_rh[:], ct_rh[:], -float(RH_LIM), float(RH_LIM), OP.max, OP.min)
            ct_tau = ctt("tau")
            nc.scalar.activation(v2(ct_tau), av, AF.Tanh)
            ct_s = ctt("s")
            nc.scalar.activation(ct_s[:], ct_rh[:], AF.Sin)
            ct_c = ctt("c")
            nc.scalar.activation(ct_c[:], ct_rh[:], AF.Sin, bias=halfpi[:])
            ct_s2 = ctt("s2")
            nc.scalar.activation(ct_s2[:], ct_s[:], AF.Square)
            ct_c2 = ctt("c2")
            nc.scalar.activation(ct_c2[:], ct_c[:], AF.Square)
            ct_sc = ctt("sc")
            nc.vector.tensor_mul(ct_sc[:], ct_s[:], ct_c[:])
            ct_t2 = ctt("t2")
            nc.scalar.activation(ct_t2[:], ct_tau[:], AF.Square)
            ct_d = ctt("d")
            nc.vector.tensor_mul(ct_d[:], ct_t2[:], ct_s2[:])
            nc.vector.tensor_add(ct_d[:], ct_d[:], ct_c2[:])
            ct_r = ctt("r")
            nc.vector.reciprocal(ct_r[:], ct_d[:])
            nc.vector.tensor_scalar(ct_t2[:], ct_t2[:], -1.0, 1.0, OP.mult, OP.add)
            ct_u = ctt("u")
            nc.vector.tensor_mul(ct_u[:], ct_sc[:], ct_t2[:])
            # T = [Tr | Ti] fp16 ; Tf = [-Ti | Tr]   (same (j, cg) layout)
            t_t = work.tile([128, 16, 128], F16, tag="t")
            tf_t = work.tile([128, 16, 128], F16, tag="tf")
            nc.vector.tensor_mul(t_t[:, :, 0:64], v2(ct_tau), v2(ct_r))
            nc.vector.tensor_mul(t_t[:, :, 64:128], v2(ct_u), v2(ct_r))
            nc.vector.tensor_scalar_mul(tf_t[:, :, 0:64], t_t[:, :, 64:128], -1.0)
            nc.vector.tensor_copy(tf_t[:, :, 64:128], t_t[:, :, 0:64])

            ps_stack.close()
            ps_stack = ExitStack()
            sm_ps = ps_stack.enter_context(tc.tile_pool(name="sm_ps", bufs=2, space="PSUM"))
            tz_ps = ps_stack.enter_context(tc.tile_pool(name="tz_ps", bufs=2, space="PSUM"))

            # ---------- stage 6: attn2 -> Y fp16 [e, (u, ri, x)] ----------
            # one K=128 matmul per b: km16 is [Kr;Ki] partition-stacked, so
            # Yr|Yi = [Kr;Ki]^T @ [T;Tf].  Assemble TT[:, b] = [T_b; Tf_b]:
            # global b = 16hf + 4g4 + 2j + par lives at t[64j:64j+64, cg] with
            # cg = 8hf + 2g4 + par; so the b axis factors as (hg=(hf,g4), j,
            # par).  Partition-parity-matched halves go via DVE, the other two
            # via partition-shifting SBUF DMAs.
            tt_t = work.tile([128, B, 128], F16, tag="tt")
            tt_v = tt_t[:].rearrange("p (hg j par) c -> p hg j par c", j=2, par=2)
            def cg_view(t):
                return t.rearrange("p (hg par) c -> p hg par c", par=2)
            nc.vector.tensor_copy(tt_v[0:64, :, 0, :, :], cg_view(t_t[0:64, :, :]))
            nc.vector.tensor_copy(tt_v[64:128, :, 1, :, :], cg_view(tf_t[64:128, :, :]))
            nc.sync.dma_start(out=tt_v[0:64, :, 1, :, :], in_=cg_view(t_t[64:128, :, :]))
            nc.sync.dma_start(out=tt_v[64:128, :, 0, :, :], in_=cg_view(tf_t[0:64, :, :]))
            # Y batch axis u: b = 2*(u%16) + (u//16)
            y_t = work.tile([E, B, 2, M], F16, tag="y")
            for u4 in range(B // 4):
                yp = sm_ps.tile([E, 4, 128], F32, tag="yp")
                for j in range(4):
                    u = u4 * 4 + j
                    b = 2 * (u % 16) + (u // 16)
                    nc.tensor.matmul(yp[:, j, :], km16_t[:, b, :], tt_t[:, b, :],
                                     start=True, stop=True)
                dst = y_t[:, u4 * 4:(u4 + 1) * 4, :, :]
                srcv = yp[:].rearrange("p b (ri m) -> p b ri m", m=M)
                if u4 % 2 == 0:
                    nc.scalar.copy(dst, srcv)
                else:
                    nc.vector.tensor_copy(dst, srcv)
            # Yf = [-Yi | Yr] for the dual-accumulate weight stage
            yf_t = work.tile([E, B, 2, M], F16, tag="yf")
            nc.vector.tensor_scalar_mul(yf_t[:, :, 0, :], y_t[:, :, 1, :], -1.0)
            nc.vector.tensor_copy(yf_t[:, :, 1, :], y_t[:, :, 0, :])

            # ---------- stage 7: weights -> Z fp16 [o, (b, ri, x)] ----------
            # Zr = Wr^T Yr - Wi^T Yi ; Zi = Wr^T Yi + Wi^T Yr, via two
            # accumulating matmuls: Wr^T @ [Yr|Yi] + Wi^T @ [-Yi|Yr].
            z_t = work.tile([O, B, 2, M], F16, tag="z")
            for x8 in range(M // 8):
                wp = sm_ps.tile([O, 8, B * 2], F32, tag="wp")
                for j in range(8):
                    x = x8 * 8 + j
                    nc.tensor.matmul(
                        wp[:, j, :],
                        w_t[:, 0, x, :],
                        y_t[:, :, :, x].rearrange("p b ri -> p (b ri)"),
                        start=True, stop=False,
                    )
                    nc.tensor.matmul(
                        wp[:, j, :],
                        w_t[:, 1, x, :],
                        yf_t[:, :, :, x].rearrange("p b ri -> p (b ri)"),
                        start=False, stop=True,
                    )
                dst = z_t[:, :, :, x8 * 8:(x8 + 1) * 8].rearrange("p b ri x -> p x b ri")
                srcv = wp[:].rearrange("p x (b ri) -> p x b ri", ri=2)
                if x8 % 2 == 0:
                    nc.scalar.copy(dst, srcv)
                else:
                    nc.vector.tensor_copy(dst, srcv)

            # ---------- stage 8: Z transposes -> Z' fp16 [(ri,x), (b, o)] ----------
            zp_g = [work.tile([128, 8, O], F16, tag=f"zp{g}", name=f"zp_g{g}")
                    for g in range(B // 8)]
            idk16 = consts.tile([64, 64], F16, tag="id16")
            nc.vector.tensor_copy(idk16[:], idq_t[0:64, 0:64])
            for b8 in range(B // 8):
                zt = tz_ps.tile([128, 8, O], F16, tag="zt")
                for j in range(8):
                    b = b8 * 8 + j
                    nc.tensor.transpose(
                        zt[:, j, :],
                        z_t[:, b, :, :].rearrange("p ri m -> p (ri m)"),
                        idk16[:],
                    )
                nc.vector.tensor_copy(zp_g[b8][:], zt[:])

            # ---------- stage 9: irfft + scaled fp16 output ----------
            # per b-pair: out[(b0-o|b1-o), 0:1024] = Z'[:, pair]^T @ G, one
            # output DMA per b-pair; host reorders/upcasts.
            for bp in range(B // 2):
                otg = outs.tile([128, 1024], F16, tag="ot")
                for g in range(2):
                    opg = sm_ps.tile([128, 512], F32, tag="op")
                    nc.tensor.matmul(
                        opg[:, :],
                        zp_g[bp // 4][:, (bp % 4) * 2:(bp % 4) * 2 + 2, :]
                        .rearrange("p b o -> p (b o)"),
                        g_t[:, g * 512:(g + 1) * 512],
                        start=True, stop=True,
                    )
                    # no scale here: fp16 can't hold out*2^-40 (underflow);
                    # host multiplies by OUT_SCALE after the fp32 upcast.
                    if (bp + g) % 2 == 0:
                        nc.scalar.copy(otg[:, g * 512:(g + 1) * 512], opg[:])
                    else:
                        nc.vector.tensor_copy(otg[:, g * 512:(g + 1) * 512], opg[:])
                nc.sync.dma_start(out=out_d[bp], in_=otg[:])
            ps_stack.close()

    nc.compile()
    return nc


_NC_CACHE = None


def _get_nc():
    global _NC_CACHE
    if _NC_CACHE is None:
        _NC_CACHE = build()
    return _NC_CACHE


def _host_prep(q, k, Wr, Wi):
    """Build the 8 per-core input maps (numpy relayout/cast only)."""
    l = np.arange(L, dtype=np.float64)[:, None]
    m = np.arange(M, dtype=np.float64)[None, :]
    ang = 2.0 * np.pi * l * m / L
    F = np.concatenate([np.cos(ang), -np.sin(ang)], axis=1).astype(np.float32)  # [L, 2M]
    fh = F.astype(np.float16)
    fl = (F - fh.astype(np.float32)).astype(np.float16)
    # fh/fl as [p][(c, 2m)]
    fh = fh.reshape(NCHUNK, 128, 2 * M).transpose(1, 0, 2).reshape(128, 1024)
    fl = fl.reshape(NCHUNK, 128, 2 * M).transpose(1, 0, 2).reshape(128, 1024)

    cm = np.full(M, 2.0); cm[0] = 1.0
    ang2 = 2.0 * np.pi * m.T * np.arange(L, dtype=np.float64)[None, :] / L
    SC = 2.0 ** GSHIFT / (L * 512.0 * 512.0)
    g = np.concatenate([
        cm[:, None] * np.cos(ang2) * SC,
        -cm[:, None] * np.sin(ang2) * SC,
    ], axis=0).astype(np.float32).astype(np.float16)  # [2M, L]

    cpack = np.concatenate([fh, fl, g.astype(np.float16)], axis=1)  # [128, 3072]

    idq = np.eye(128, dtype=np.float32)

    maps = []
    for h in range(H):
        def split(x):
            xs = np.ascontiguousarray(x[:, :, h, :].transpose(1, 0, 2)).reshape(L, B * E)
            hi = xs.astype(np.float16)
            lo = (xs - hi.astype(np.float32)).astype(np.float16)
            return hi, lo
        qh, ql = split(q)
        kh, kl = split(k)
        # pack [c][hf][p][t][col]
        xp = np.empty((NCHUNK, NHALF, 128, 4, HB * E), np.float16)
        for t, src in enumerate((qh, ql, kh, kl)):
            s = src.reshape(NCHUNK, 128, NHALF, HB * E)
            xp[:, :, :, t, :] = s.transpose(0, 2, 1, 3)
        wpk = np.empty((E, 2, M, O), np.float32)
        wpk[:, 0] = (Wr[h] * 2.0 ** WSHIFT).transpose(0, 2, 1)  # [e,o,x]->[e,x,o]
        wpk[:, 1] = (Wi[h] * 2.0 ** WSHIFT).transpose(0, 2, 1)
        maps.append({
            "xp": xp,
            "cp": cpack,
            "wp": wpk.astype(np.float16),
            "idq": idq,
        })
    return maps


def kernel(q, k, v, Wr, Wi, _trace=False):
    q = np.asarray(q, np.float32)
    k = np.asarray(k, np.float32)
    Wr = np.asarray(Wr, np.float32)
    Wi = np.asarray(Wi, np.float32)
    nc = _get_nc()
    maps = _host_prep(q, k, Wr, Wi)
    try:
        res = run_bass_kernel_spmd(nc, maps, core_ids=list(range(H)), trace=_trace)
    except ModuleNotFoundError:
        res = run_bass_kernel_spmd(nc, maps, core_ids=list(range(H)), trace=False)
    # out_d[bp][p][l]: u = 2*bp + (p//64), o = p%64; b = 2*(u%16) + u//16
    out = np.empty((B, H, O, L), np.float32)
    for h in range(H):
        o = np.asarray(res.results[h]["out"], np.float32).reshape(B, O, L)
        o *= np.float32(OUT_SCALE)
        u = np.arange(B)
        b = 2 * (u % 16) + u // 16
        out[b, h] = o
    if _trace:
        kernel.last_results = res
    return out.astype(np.float32)


# revision 19
# speedup vs baseline: 1.1707x; 1.0334x over previous
"""FEDformer FourierCrossAttention kernel for 8 TRN2 NeuronCores.

Sharding: one head per core (H=8 == n_cores). Each core computes, for its head:
  Q = rfft(q)[:64 modes], K = rfft(k)[:64]      (DFT-as-matmul, hi/lo fp16 3-pass)
  X^T = K^T Q (complex, contract E)             (2-batch 256-col matmuls)
  T = tanh(X) (complex, tau/sin/cos form)       (ACT tanh+sin, DVE cody-waite RR)
  Y = sum_y T[x,y] K[e,y]                       (fp16 matmuls)
  Z = sum_e W[e,o,x] Y[e,x]   (W scaled 2^16)   (dual-accumulate Wr/Wi fp16 matmuls)
  out = irfft(Z / (512*512))  (G scaled 2^24)   (fp16 matmuls, scale 2^-40 on copy)

DMA layout is optimized for the TRN2 DGE model: few, large, contiguous
transfers. Inputs are packed per (chunk, batch-half) into single tiles;
output is written per b-pair in fp16 and reordered/upcast on the host.

Batch indexing: global b = 16*hf + 4*g4 + 2*j + par, stored in the attn/tanh
stages at partition half j (pair LSB) and column group cg = 8*hf + 2*g4 + par.
"""
import numpy as np

import concourse.bass as bass
import concourse.tile as tile
from concourse import bacc, mybir
from concourse.bass_utils import run_bass_kernel_spmd

F32 = mybir.dt.float32
F16 = mybir.dt.float16
AF = mybir.ActivationFunctionType
OP = mybir.AluOpType

B, L, H, E, O, M = 32, 1024, 8, 64, 64, 64
NCHUNK = 8          # contraction chunks of 128 over L
NHALF = 2           # batch halves of 16 for DFT PSUM
WSHIFT = 16         # W scaled by 2^WSHIFT on host
GSHIFT = 24         # G scaled by 2^GSHIFT on host
OUT_SCALE = 2.0 ** (-WSHIFT - GSHIFT)
HB = B // NHALF     # 16 batches per half

PI = np.float64(np.pi)
PI_HI = np.float32(3.140625)
PI_MID = np.float32(PI - np.float64(np.float32(3.140625)))
PI_LO = np.float32(PI - np.float64(np.float32(3.140625)) - np.float64(PI_MID))
MAGIC = np.float32(1.5 * 2 ** 23)   # round-to-nearest via add/sub
RH_LIM = np.nextafter(np.float32(np.pi) - np.float32(np.pi / 2), np.float32(0))


def build(debug=False):
    nc = bacc.Bacc("TRN2", target_bir_lowering=False, debug=False, num_devices=8)

    # ---- I/O (per-core, host pre-sharded/relaid) ----
    # packed q/k hi+lo fp16: [c][hf][p][t][col]; t in {qh, ql, kh, kl},
    # col = b_local*64 + e  (b_local = b - 16*hf)
    x_d = nc.dram_tensor("xp", (NCHUNK, NHALF, 128, 4, HB * E), F16,
                         kind="ExternalInput")
    # packed fp16 consts: [p][fh(8*128) | fl(8*128) | g(1024)]
    c_d = nc.dram_tensor("cp", (128, 3 * 1024), F16, kind="ExternalInput")
    # W packed fp16 (x2^16): [e][ri][x][o] = W{ri}[e, o, x]
    w_d = nc.dram_tensor("wp", (E, 2, M, O), F16, kind="ExternalInput")
    # transpose helper fp32
    idq_d = nc.dram_tensor("idq", (128, 128), F32, kind="ExternalInput")

    # out[bp][p][l]: p = (b-pair half)*64 + o; u = 2*bp + (p>=64)
    out_d = nc.dram_tensor("out", (B // 2, 128, L), F16, kind="ExternalOutput")

    with tile.TileContext(nc) as tc:
        from contextlib import ExitStack
        stack = ExitStack()
        with stack:
            consts = stack.enter_context(tc.tile_pool(name="consts", bufs=1))
            chunks = stack.enter_context(tc.tile_pool(name="chunks", bufs=3))
            coeff = stack.enter_context(tc.tile_pool(name="coeff", bufs=1))
            work = stack.enter_context(tc.tile_pool(name="work", bufs=1))
            tmp = stack.enter_context(tc.tile_pool(name="tmp", bufs=1))
            outs = stack.enter_context(tc.tile_pool(name="outs", bufs=4))
            ps_stack = ExitStack()
            dft_ps = ps_stack.enter_context(tc.tile_pool(name="dft_ps", bufs=1, space="PSUM"))
            tp_ps = ps_stack.enter_context(tc.tile_pool(name="tp_ps", bufs=2, space="PSUM"))
            at_ps = ps_stack.enter_context(tc.tile_pool(name="at_ps", bufs=2, space="PSUM"))

            # ---------- constants (3 DMAs spread over scalar/vector queues) ----------
            c_t = consts.tile([128, 3 * 1024], F16, tag="cp")
            w_t = consts.tile([E, 2, M, O], F16, tag="w")
            idq_t = consts.tile([128, 128], F32, tag="idq")
            nc.scalar.dma_start(out=c_t, in_=c_d[:])
            nc.scalar.dma_start(out=w_t, in_=w_d[:])
            nc.scalar.dma_start(out=idq_t, in_=idq_d[:])
            fh_t = c_t[:, 0:1024].rearrange("p (c m) -> p c m", m=2 * M)
            fl_t = c_t[:, 1024:2048].rearrange("p (c m) -> p c m", m=2 * M)
            g_t = c_t[:, 2048:3072]

            # ---------- per-half state ----------
            qm_h = [coeff.tile([128, 1024], F32, tag=f"qmh{hf}", name=f"qm_h{hf}")
                    for hf in range(NHALF)]
            km_h = [coeff.tile([128, 1024], F32, tag=f"kmh{hf}", name=f"km_h{hf}")
                    for hf in range(NHALF)]
            km16_t = coeff.tile([128, B, E], F16, tag="km16")
            # layout [p=(par,e), ri, bp, y] so stationary attn1 slices merge
            # into a single contiguous free dim (BIR requirement)
            qe_h = [work.tile([128, 2, 8, 64], F32, tag=f"qeh{hf}", name=f"qe_h{hf}") for hf in range(NHALF)]
            ke_h = [work.tile([128, 2, 8, 64], F32, tag=f"keh{hf}", name=f"ke_h{hf}") for hf in range(NHALF)]
            qf_h = [work.tile([128, 2, 8, 64], F32, tag=f"qfh{hf}", name=f"qf_h{hf}") for hf in range(NHALF)]
            # A/B packed: ab[p = 64*j + y, cg, 0:64 = Re X^T, 64:128 = Im X^T]
            ab_t = work.tile([128, 16, 128], F32, tag="ab")

            # ---------- stage 5+6, per half: complex tanh + attn2 ----------
            # runs for half hf right after its attn1, overlapping the other
            # half's (DMA-paced) DFT.
            halfpi = consts.tile([128, 1], F32, tag="halfpi", name="halfpi")
            nc.vector.memset(halfpi[:], float(np.pi / 2))
            t_t = work.tile([128, 16, 128], F16, tag="t")
            tf_t = work.tile([128, 16, 128], F16, tag="tf")
            tt_t = work.tile([128, B, 128], F16, tag="tt")
            tt_v = tt_t[:].rearrange("p (hg j par) c -> p hg j par c", j=2, par=2)
            y_t = work.tile([E, B, 2, M], F16, tag="y")
            yf_t = work.tile([E, B, 2, M], F16, tag="yf")

            def tanh_attn2_half(hf):
                cgs = slice(8 * hf, 8 * hf + 8)
                # A=Re X^T, B=Im X^T, strided views of ab_t [128, 8, 64]
                av = ab_t[:, cgs, 0:64]
                bv = ab_t[:, cgs, 64:128]
                def ctt(n):
                    return tmp.tile([128, 512], F32, tag="ct", name=f"ct_{n}{hf}", bufs=6)
                def v2(t):
                    return t[:].rearrange("p (g m) -> p g m", m=64)
                ct_n = ctt("n")
                nc.vector.tensor_scalar(v2(ct_n), bv, float(1.0 / PI), float(MAGIC), OP.mult, OP.add)
                nc.vector.tensor_scalar_sub(ct_n[:], ct_n[:], float(MAGIC))
                ct_rh = ctt("rh")
                nc.vector.cody_waite_cascade(v2(ct_rh), bv, ct_n[:], float(PI_HI), float(PI_MID), float(PI_LO))
                # clamp |rh| so rh+pi/2 (cos) and 2*rh (sin) stay in [-pi, pi]
                nc.vector.tensor_scalar(ct_rh[:], ct_rh[:], -float(RH_LIM), float(RH_LIM), OP.max, OP.min)
                ct_tau = ctt("tau")
                nc.scalar.activation(v2(ct_tau), av, AF.Tanh)
                ct_s = ctt("s")
                nc.scalar.activation(ct_s[:], ct_rh[:], AF.Sin)
                ct_c = ctt("c")
                nc.scalar.activation(ct_c[:], ct_rh[:], AF.Sin, bias=halfpi[:])
                ct_s2 = ctt("s2")
                nc.scalar.activation(ct_s2[:], ct_s[:], AF.Square)
                ct_c2 = ctt("c2")
                nc.scalar.activation(ct_c2[:], ct_c[:], AF.Square)
                ct_sc = ctt("sc")
                nc.vector.tensor_mul(ct_sc[:], ct_s[:], ct_c[:])
                ct_t2 = ctt("t2")
                nc.scalar.activation(ct_t2[:], ct_tau[:], AF.Square)
                ct_d = ctt("d")
                nc.vector.tensor_mul(ct_d[:], ct_t2[:], ct_s2[:])
                nc.vector.tensor_add(ct_d[:], ct_d[:], ct_c2[:])
                ct_r = ctt("r")
                nc.vector.reciprocal(ct_r[:], ct_d[:])
                nc.vector.tensor_scalar(ct_t2[:], ct_t2[:], -1.0, 1.0, OP.mult, OP.add)
                ct_u = ctt("u")
                nc.vector.tensor_mul(ct_u[:], ct_sc[:], ct_t2[:])
                # T = [Tr | Ti] fp16 ; Tf = [-Ti | Tr]   (same (j, cg) layout)
                nc.vector.tensor_mul(t_t[:, cgs, 0:64], v2(ct_tau), v2(ct_r))
                nc.vector.tensor_mul(t_t[:, cgs, 64:128], v2(ct_u), v2(ct_r))
                nc.vector.tensor_scalar_mul(tf_t[:, cgs, 0:64], t_t[:, cgs, 64:128], -1.0)
                nc.vector.tensor_copy(tf_t[:, cgs, 64:128], t_t[:, cgs, 0:64])

                # ----- attn2: TT assembly + Y for this half's batches -----
                # global b = 16hf + 4g4 + 2j + par lives at t[64j:64j+64, cg],
                # cg = 8hf + 2g4 + par; b factors as (hg=(hf,g4), j, par).
                # Parity-matched halves via DVE, others via SWDGE SBUF DMAs.
                hgs = slice(4 * hf, 4 * hf + 4)
                def cg_view(t):
                    return t.rearrange("p (hg par) c -> p hg par c", par=2)
                nc.vector.tensor_copy(tt_v[0:64, hgs, 0, :, :], cg_view(t_t[0:64, cgs, :]))
                nc.vector.tensor_copy(tt_v[64:128, hgs, 1, :, :], cg_view(tf_t[64:128, cgs, :]))
                nc.gpsimd.dma_start(out=tt_v[0:64, hgs, 1, :, :], in_=cg_view(t_t[64:128, cgs, :]))
                nc.gpsimd.dma_start(out=tt_v[64:128, hgs, 0, :, :], in_=cg_view(tf_t[0:64, cgs, :]))
                # Y batch axis u: b = 2*(u%16) + (u//16); this half's batches
                # (b%16 < 8*(hf+1)) are u4 groups {0,1,4,5} (hf=0), {2,3,6,7}.
                for u4 in ((0, 1, 4, 5) if hf == 0 else (2, 3, 6, 7)):
                    yp = at_ps.tile([E, 4, 128], F32, tag="pt", bufs=2, name=f"yp{u4}")
                    for j in range(4):
                        u = u4 * 4 + j
                        b = 2 * (u % 16) + (u // 16)
                        nc.tensor.matmul(yp[:, j, :], km16_t[:, b, :], tt_t[:, b, :],
                                         start=True, stop=True)
                    dst = y_t[:, u4 * 4:(u4 + 1) * 4, :, :]
                    srcv = yp[:].rearrange("p b (ri m) -> p b ri m", m=M)
                    if u4 % 2 == 0:
                        nc.scalar.copy(dst, srcv)
                    else:
                        nc.vector.tensor_copy(dst, srcv)
                # Yf = [-Yi | Yr] for the dual-accumulate weight stage
                for u0, u1 in ((0, 8), (16, 24)) if hf == 0 else ((8, 16), (24, 32)):
                    nc.vector.tensor_scalar_mul(yf_t[:, u0:u1, 0, :], y_t[:, u0:u1, 1, :], -1.0)
                    nc.vector.tensor_copy(yf_t[:, u0:u1, 1, :], y_t[:, u0:u1, 0, :])

            for hf in range(NHALF):
                # ---------- stage 1+2: DFT (hi/lo 3-pass) ----------
                qm_ps = dft_ps.tile([128, 1024], F32, tag="qmps", name=f"qm_ps{hf}", bufs=1)
                km_ps = dft_ps.tile([128, 1024], F32, tag="kmps", name=f"km_ps{hf}", bufs=1)
                for c in range(NCHUNK):
                    x_c = chunks.tile([128, 4, HB * E], F16, tag="x", name=f"x{hf}_{c}")
                    nc.sync.dma_start(out=x_c, in_=x_d[c, hf])
                    first = c == 0
                    last = c == NCHUNK - 1
                    passes = (
                        (fh_t[:, c, :], 0, qm_ps, first, False),
                        (fh_t[:, c, :], 1, qm_ps, False, False),
                        (fh_t[:, c, :], 2, km_ps, first, False),
                        (fh_t[:, c, :], 3, km_ps, False, False),
                        (fl_t[:, c, :], 0, qm_ps, False, last),
                        (fl_t[:, c, :], 2, km_ps, False, last),
                    )
                    for lhs, ti, ps, is_start, is_stop in passes:
                        for g in range(2):
                            nc.tensor.matmul(
                                ps[:, g * 512:(g + 1) * 512],
                                lhs,
                                x_c[:, ti, g * 512:(g + 1) * 512],
                                start=is_start,
                                stop=is_stop,
                            )
                nc.vector.tensor_copy(qm_h[hf][:], qm_ps[:])
                nc.scalar.copy(km_h[hf][:], km_ps[:])
                nc.vector.tensor_copy(
                    km16_t[:, hf * HB:(hf + 1) * HB, :],
                    km_ps[:].rearrange("p (b e) -> p b e", e=E),
                )

                # ---------- stage 3: pair transposes -> Q_e, K_e ----------
                # in [2m, (b0-e|b1-e)] -> out [(b0-e|b1-e), 2m]; even b on
                # partitions 0:64, odd on 64:128.
                qm_p = qm_h[hf][:].rearrange("p (bp c) -> p bp c", c=128)
                km_p = km_h[hf][:].rearrange("p (bp c) -> p bp c", c=128)
                for g2 in range(4):
                    tp = tp_ps.tile([128, 2, 128], F32, tag="tp")
                    tk = tp_ps.tile([128, 2, 128], F32, tag="tp", name=f"tk{hf}_{g2}")
                    for j in range(2):
                        bpl = g2 * 2 + j
                        nc.tensor.transpose(tp[:, j, :], qm_p[:, bpl, :], idq_t[:])
                        nc.tensor.transpose(tk[:, j, :], km_p[:, bpl, :], idq_t[:])
                    tpv = tp[:].rearrange("p j (ri y) -> p ri j y", ri=2)
                    tkv = tk[:].rearrange("p j (ri y) -> p ri j y", ri=2)
                    if g2 % 2 == 0:
                        nc.scalar.copy(qe_h[hf][:, :, g2 * 2:(g2 + 1) * 2, :], tpv)
                        nc.scalar.copy(ke_h[hf][:, :, g2 * 2:(g2 + 1) * 2, :], tkv)
                    else:
                        nc.vector.tensor_copy(qe_h[hf][:, :, g2 * 2:(g2 + 1) * 2, :], tpv)
                        nc.vector.tensor_copy(ke_h[hf][:, :, g2 * 2:(g2 + 1) * 2, :], tkv)
                nc.vector.tensor_scalar_mul(qf_h[hf][:, 0, :, :], qe_h[hf][:, 1, :, :], -1.0)
                nc.vector.tensor_copy(qf_h[hf][:, 1, :, :], qe_h[hf][:, 0, :, :])

                # ---------- stage 4: attn1 -> X^T psum, A/B fp32 sbuf ----------
                # 2 same-parity b per matmul pair (256 cols each):
                #   pt[(j, y), (j', ri, x)] = sum_e Kr_bj[e, y] Q_bj'[e, (ri x)]
                #                           + sum_e Ki_bj[e, y] Qf_bj'[e, (ri x)]
                # useful quadrants j == j'; partition-aligned extraction.
                for par in range(2):
                    base = 64 * par
                    sl = slice(base, base + 64)
                    for g4 in range(4):
                        # pairs at this parity: batch pair index pr = 2*g4 + j,
                        # living at ke[64*par:, :, pr, :]; take pr = 2*g4, 2*g4+1.
                        # out pt[(j, y), (ri', j', y')]; useful quadrants j==j'.
                        pt = at_ps.tile([128, 2, 2, 64], F32, tag="pt", bufs=2)
                        psl = slice(2 * g4, 2 * g4 + 2)
                        nc.tensor.matmul(pt[:], ke_h[hf][sl, 0, psl, :],
                                         qe_h[hf][sl, :, psl, :],
                                         start=True, stop=False)
                        nc.tensor.matmul(pt[:], ke_h[hf][sl, 1, psl, :],
                                         qf_h[hf][sl, :, psl, :],
                                         start=False, stop=True)
                        cg = 8 * hf + 2 * g4 + par
                        if (par + g4) % 2 == 0:
                            nc.scalar.copy(ab_t[0:64, cg, :].rearrange("p (ri y) -> p ri y", ri=2), pt[0:64, :, 0, :])
                            nc.vector.tensor_copy(ab_t[64:128, cg, :].rearrange("p (ri y) -> p ri y", ri=2), pt[64:128, :, 1, :])
                        else:
                            nc.vector.tensor_copy(ab_t[0:64, cg, :].rearrange("p (ri y) -> p ri y", ri=2), pt[0:64, :, 0, :])
                            nc.scalar.copy(ab_t[64:128, cg, :].rearrange("p (ri y) -> p ri y", ri=2), pt[64:128, :, 1, :])

                tanh_attn2_half(hf)

            ps_stack.close()
            ps_stack = ExitStack()
            sm_ps = ps_stack.enter_context(tc.tile_pool(name="sm_ps", bufs=2, space="PSUM"))
            tz_ps = ps_stack.enter_context(tc.tile_pool(name="tz_ps", bufs=2, space="PSUM"))

            # ---------- stage 7: weights -> Z fp16 [o, (b, ri, x)] ----------
            # Zr = Wr^T Yr - Wi^T Yi ; Zi = Wr^T Yi + Wi^T Yr, via two
            # accumulating matmuls: Wr^T @ [Yr|Yi] + Wi^T @ [-Yi|Yr].
            z_t = work.tile([O, B, 2, M], F16, tag="z")
            for x8 in range(M // 8):
                wp = sm_ps.tile([O, 8, B * 2], F32, tag="wp")
                for j in range(8):
                    x = x8 * 8 + j
                    nc.tensor.matmul(
                        wp[:, j, :],
                        w_t[:, 0, x, :],
                        y_t[:, :, :, x].rearrange("p b ri -> p (b ri)"),
                        start=True, stop=False,
                    )
                    nc.tensor.matmul(
                        wp[:, j, :],
                        w_t[:, 1, x, :],
                        yf_t[:, :, :, x].rearrange("p b ri -> p (b ri)"),
                        start=False, stop=True,
                    )
                dst = z_t[:, :, :, x8 * 8:(x8 + 1) * 8].rearrange("p b ri x -> p x b ri")
                srcv = wp[:].rearrange("p x (b ri) -> p x b ri", ri=2)
                if x8 % 2 == 0:
                    nc.scalar.copy(dst, srcv)
                else:
                    nc.vector.tensor_copy(dst, srcv)

            # ---------- stage 8: Z transposes -> Z' fp16 [(ri,x), (b, o)] ----------
            zp_g = [work.tile([128, 8, O], F16, tag=f"zp{g}", name=f"zp_g{g}")
                    for g in range(B // 8)]
            idk16 = consts.tile([64, 64], F16, tag="id16")
            nc.vector.tensor_copy(idk16[:], idq_t[0:64, 0:64])
            for b8 in range(B // 8):
                zt = tz_ps.tile([128, 8, O], F16, tag="zt")
                for j in range(8):
                    b = b8 * 8 + j
                    nc.tensor.transpose(
                        zt[:, j, :],
                        z_t[:, b, :, :].rearrange("p ri m -> p (ri m)"),
                        idk16[:],
                    )
                nc.vector.tensor_copy(zp_g[b8][:], zt[:])

            # ---------- stage 9: irfft + scaled fp16 output ----------
            # per b-pair: out[(b0-o|b1-o), 0:1024] = Z'[:, pair]^T @ G, one
            # output DMA per b-pair; host reorders/upcasts.
            for bp in range(B // 2):
                otg = outs.tile([128, 1024], F16, tag="ot")
                for g in range(2):
                    opg = sm_ps.tile([128, 512], F32, tag="op")
                    nc.tensor.matmul(
                        opg[:, :],
                        zp_g[bp // 4][:, (bp % 4) * 2:(bp % 4) * 2 + 2, :]
                        .rearrange("p b o -> p (b o)"),
                        g_t[:, g * 512:(g + 1) * 512],
                        start=True, stop=True,
                    )
                    # no scale here: fp16 can't hold out*2^-40 (underflow);
                    # host multiplies by OUT_SCALE after the fp32 upcast.
                    if (bp + g) % 2 == 0:
                        nc.scalar.copy(otg[:, g * 512:(g + 1) * 512], opg[:])
                    else:
                        nc.vector.tensor_copy(otg[:, g * 512:(g + 1) * 512], opg[:])
                nc.sync.dma_start(out=out_d[bp], in_=otg[:])
            ps_stack.close()

    nc.compile()
    return nc


_NC_CACHE = None


def _get_nc():
    global _NC_CACHE
    if _NC_CACHE is None:
        _NC_CACHE = build()
    return _NC_CACHE


def _host_prep(q, k, Wr, Wi):
    """Build the 8 per-core input maps (numpy relayout/cast only)."""
    l = np.arange(L, dtype=np.float64)[:, None]
    m = np.arange(M, dtype=np.float64)[None, :]
    ang = 2.0 * np.pi * l * m / L
    F = np.concatenate([np.cos(ang), -np.sin(ang)], axis=1).astype(np.float32)  # [L, 2M]
    fh = F.astype(np.float16)
    fl = (F - fh.astype(np.float32)).astype(np.float16)
    # fh/fl as [p][(c, 2m)]
    fh = fh.reshape(NCHUNK, 128, 2 * M).transpose(1, 0, 2).reshape(128, 1024)
    fl = fl.reshape(NCHUNK, 128, 2 * M).transpose(1, 0, 2).reshape(128, 1024)

    cm = np.full(M, 2.0); cm[0] = 1.0
    ang2 = 2.0 * np.pi * m.T * np.arange(L, dtype=np.float64)[None, :] / L
    SC = 2.0 ** GSHIFT / (L * 512.0 * 512.0)
    g = np.concatenate([
        cm[:, None] * np.cos(ang2) * SC,
        -cm[:, None] * np.sin(ang2) * SC,
    ], axis=0).astype(np.float32).astype(np.float16)  # [2M, L]

    cpack = np.concatenate([fh, fl, g.astype(np.float16)], axis=1)  # [128, 3072]

    idq = np.eye(128, dtype=np.float32)

    maps = []
    for h in range(H):
        def split(x):
            xs = np.ascontiguousarray(x[:, :, h, :].transpose(1, 0, 2)).reshape(L, B * E)
            hi = xs.astype(np.float16)
            lo = (xs - hi.astype(np.float32)).astype(np.float16)
            return hi, lo
        qh, ql = split(q)
        kh, kl = split(k)
        # pack [c][hf][p][t][col]
        xp = np.empty((NCHUNK, NHALF, 128, 4, HB * E), np.float16)
        for t, src in enumerate((qh, ql, kh, kl)):
            s = src.reshape(NCHUNK, 128, NHALF, HB * E)
            xp[:, :, :, t, :] = s.transpose(0, 2, 1, 3)
        wpk = np.empty((E, 2, M, O), np.float32)
        wpk[:, 0] = (Wr[h] * 2.0 ** WSHIFT).transpose(0, 2, 1)  # [e,o,x]->[e,x,o]
        wpk[:, 1] = (Wi[h] * 2.0 ** WSHIFT).transpose(0, 2, 1)
        maps.append({
            "xp": xp,
            "cp": cpack,
            "wp": wpk.astype(np.float16),
            "idq": idq,
        })
    return maps


def kernel(q, k, v, Wr, Wi, _trace=False):
    q = np.asarray(q, np.float32)
    k = np.asarray(k, np.float32)
    Wr = np.asarray(Wr, np.float32)
    Wi = np.asarray(Wi, np.float32)
    nc = _get_nc()
    maps = _host_prep(q, k, Wr, Wi)
    try:
        res = run_bass_kernel_spmd(nc, maps, core_ids=list(range(H)), trace=_trace)
    except ModuleNotFoundError:
        res = run_bass_kernel_spmd(nc, maps, core_ids=list(range(H)), trace=False)
    # out_d[bp][p][l]: u = 2*bp + (p//64), o = p%64; b = 2*(u%16) + u//16
    out = np.empty((B, H, O, L), np.float32)
    for h in range(H):
        o = np.asarray(res.results[h]["out"], np.float32).reshape(B, O, L)
        o *= np.float32(OUT_SCALE)
        u = np.arange(B)
        b = 2 * (u % 16) + u // 16
        out[b, h] = o
    if _trace:
        kernel.last_results = res
    return out.astype(np.float32)


# revision 21
# speedup vs baseline: 1.2402x; 1.0594x over previous
"""FEDformer FourierCrossAttention kernel for 8 TRN2 NeuronCores.

Sharding: one head per core (H=8 == n_cores). Each core computes, for its head:
  Q = rfft(q)[:64 modes], K = rfft(k)[:64]      (DFT-as-matmul, hi/lo fp16 3-pass)
  X^T = K^T Q (complex, contract E)             (2-batch 256-col matmuls)
  T = tanh(X) (complex, tau/sin/cos form)       (ACT tanh+sin, DVE cody-waite RR)
  Y = sum_y T[x,y] K[e,y]                       (fp16 matmuls)
  Z = sum_e W[e,o,x] Y[e,x]   (W scaled 2^16)   (dual-accumulate Wr/Wi fp16 matmuls)
  out = irfft(Z / (512*512))  (G scaled 2^24)   (fp16 matmuls; 2^-40 applied on host)

The whole pipeline is split per batch-half (hf): half 0's attn/tanh/output
stages overlap half 1's DMA-paced DFT, and half 0's weight/irfft stages fill
the PE-idle window of half 1's tanh chain.

Batch indexing: global b = 16*hf + 4*g4 + 2*j + par, stored in the attn/tanh
stages at partition half j (pair LSB) and column group cg = 8*hf + 2*g4 + par.
Y/Z/out stages use plain global b ordering.
"""
import numpy as np

import concourse.bass as bass
import concourse.tile as tile
from concourse import bacc, mybir
from concourse.bass_utils import run_bass_kernel_spmd

F32 = mybir.dt.float32
F16 = mybir.dt.float16
AF = mybir.ActivationFunctionType
OP = mybir.AluOpType

B, L, H, E, O, M = 32, 1024, 8, 64, 64, 64
NCHUNK = 8          # contraction chunks of 128 over L
NHALF = 2           # batch halves of 16 for DFT PSUM
WSHIFT = 16         # W scaled by 2^WSHIFT on host
GSHIFT = 24         # G scaled by 2^GSHIFT on host
OUT_SCALE = 2.0 ** (-WSHIFT - GSHIFT)
HB = B // NHALF     # 16 batches per half

PI = np.float64(np.pi)
PI_HI = np.float32(3.140625)
PI_MID = np.float32(PI - np.float64(np.float32(3.140625)))
PI_LO = np.float32(PI - np.float64(np.float32(3.140625)) - np.float64(PI_MID))
MAGIC = np.float32(1.5 * 2 ** 23)   # round-to-nearest via add/sub
RH_LIM = np.nextafter(np.float32(np.pi) - np.float32(np.pi / 2), np.float32(0))


def build(debug=False):
    nc = bacc.Bacc("TRN2", target_bir_lowering=False, debug=False, num_devices=8)

    # ---- I/O (per-core, host pre-sharded/relaid) ----
    # packed q/k hi+lo fp16: [c][hf][p][t][col]; t in {qh, ql, kh, kl},
    # col = b_local*64 + e  (b_local = b - 16*hf)
    x_d = nc.dram_tensor("xp", (NCHUNK, NHALF, 128, 4, HB * E), F16,
                         kind="ExternalInput")
    # packed fp16 consts: [p][fh(8*128) | fl(8*128) | g(1024)]
    c_d = nc.dram_tensor("cp", (128, 3 * 1024), F16, kind="ExternalInput")
    # W packed fp16 (x2^16): [e][ri][x][o] = W{ri}[e, o, x]
    w_d = nc.dram_tensor("wp", (E, 2, M, O), F16, kind="ExternalInput")
    # transpose helper fp32
    idq_d = nc.dram_tensor("idq", (128, 128), F32, kind="ExternalInput")

    # out[bp][p][l]: p = (pair half)*64 + o; global b = 2*bp + (p>=64)
    out_d = nc.dram_tensor("out", (B // 2, 128, L), F16, kind="ExternalOutput")

    with tile.TileContext(nc) as tc:
        from contextlib import ExitStack
        stack = ExitStack()
        with stack:
            consts = stack.enter_context(tc.tile_pool(name="consts", bufs=1))
            chunks = stack.enter_context(tc.tile_pool(name="chunks", bufs=3))
            coeff = stack.enter_context(tc.tile_pool(name="coeff", bufs=1))
            work = stack.enter_context(tc.tile_pool(name="work", bufs=1))
            tmp = stack.enter_context(tc.tile_pool(name="tmp", bufs=1))
            outs = stack.enter_context(tc.tile_pool(name="outs", bufs=4))
            dft_ps = stack.enter_context(tc.tile_pool(name="dft_ps", bufs=1, space="PSUM"))
            tp_ps = stack.enter_context(tc.tile_pool(name="tp_ps", bufs=2, space="PSUM"))
            at_ps = stack.enter_context(tc.tile_pool(name="at_ps", bufs=2, space="PSUM"))

            # ---------- constants ----------
            c_t = consts.tile([128, 3 * 1024], F16, tag="cp")
            w_t = consts.tile([E, 2, M, O], F16, tag="w")
            idq_t = consts.tile([128, 128], F32, tag="idq")
            nc.scalar.dma_start(out=c_t, in_=c_d[:])
            nc.scalar.dma_start(out=w_t, in_=w_d[:])
            nc.scalar.dma_start(out=idq_t, in_=idq_d[:])
            fh_t = c_t[:, 0:1024].rearrange("p (c m) -> p c m", m=2 * M)
            fl_t = c_t[:, 1024:2048].rearrange("p (c m) -> p c m", m=2 * M)
            g_t = c_t[:, 2048:3072]

            # ---------- persistent state ----------
            qm_h = [coeff.tile([128, 1024], F32, tag=f"qmh{hf}", name=f"qm_h{hf}")
                    for hf in range(NHALF)]
            km_h = [coeff.tile([128, 1024], F32, tag=f"kmh{hf}", name=f"km_h{hf}")
                    for hf in range(NHALF)]
            km16_t = coeff.tile([128, B, E], F16, tag="km16")
            # layout [p=(par,e), ri, bp, y] so stationary attn1 slices merge
            # into a single contiguous free dim (BIR requirement)
            qe_h = [work.tile([128, 2, 8, 64], F32, tag=f"qeh{hf}", name=f"qe_h{hf}") for hf in range(NHALF)]
            ke_h = [work.tile([128, 2, 8, 64], F32, tag=f"keh{hf}", name=f"ke_h{hf}") for hf in range(NHALF)]
            qf_h = [work.tile([128, 2, 8, 64], F32, tag=f"qfh{hf}", name=f"qf_h{hf}") for hf in range(NHALF)]
            # A/B packed: ab[p = 64*j + y, cg, 0:64 = Re X^T, 64:128 = Im X^T]
            ab_t = work.tile([128, 16, 128], F32, tag="ab")
            halfpi = consts.tile([128, 1], F32, tag="halfpi", name="halfpi")
            nc.vector.memset(halfpi[:], float(np.pi / 2))
            t_t = work.tile([128, 16, 128], F16, tag="t")
            tf_t = work.tile([128, 16, 128], F16, tag="tf")
            tt_t = work.tile([128, B, 128], F16, tag="tt")
            tt_v = tt_t[:].rearrange("p (hg j par) c -> p hg j par c", j=2, par=2)
            y_t = work.tile([E, B, 2, M], F16, tag="y")
            yf_t = work.tile([E, B, 2, M], F16, tag="yf")
            z_t = work.tile([O, B, 2, M], F16, tag="z")
            zp_g = [work.tile([128, 8, O], F16, tag=f"zp{g}", name=f"zp_g{g}")
                    for g in range(B // 8)]
            idk16 = consts.tile([64, 64], F16, tag="id16")

            # ---------- stage 5+6, per half: complex tanh + attn2 ----------
            def tanh_attn2_half(hf):
                cgs = slice(8 * hf, 8 * hf + 8)
                # A=Re X^T, B=Im X^T, strided views of ab_t [128, 8, 64]
                av = ab_t[:, cgs, 0:64]
                bv = ab_t[:, cgs, 64:128]
                def ctt(n):
                    return tmp.tile([128, 512], F32, tag="ct", name=f"ct_{n}{hf}", bufs=6)
                def v2(t):
                    return t[:].rearrange("p (g m) -> p g m", m=64)
                ct_n = ctt("n")
                nc.vector.tensor_scalar(v2(ct_n), bv, float(1.0 / PI), float(MAGIC), OP.mult, OP.add)
                nc.vector.tensor_scalar_sub(ct_n[:], ct_n[:], float(MAGIC))
                ct_rh = ctt("rh")
                nc.vector.cody_waite_cascade(v2(ct_rh), bv, ct_n[:], float(PI_HI), float(PI_MID), float(PI_LO))
                # clamp |rh| so rh+pi/2 (cos) and 2*rh (sin) stay in [-pi, pi]
                nc.vector.tensor_scalar(ct_rh[:], ct_rh[:], -float(RH_LIM), float(RH_LIM), OP.max, OP.min)
                ct_tau = ctt("tau")
                nc.scalar.activation(v2(ct_tau), av, AF.Tanh)
                ct_s = ctt("s")
                nc.scalar.activation(ct_s[:], ct_rh[:], AF.Sin)
                ct_c = ctt("c")
                nc.scalar.activation(ct_c[:], ct_rh[:], AF.Sin, bias=halfpi[:])
                # squares on DVE (keeps ACT on the Tanh/Sin table set: a table
                # reload costs 1.3us and lands on the critical chain)
                ct_s2 = ctt("s2")
                nc.vector.tensor_mul(ct_s2[:], ct_s[:], ct_s[:])
                ct_c2 = ctt("c2")
                nc.vector.tensor_mul(ct_c2[:], ct_c[:], ct_c[:])
                ct_sc = ctt("sc")
                nc.vector.tensor_mul(ct_sc[:], ct_s[:], ct_c[:])
                ct_t2 = ctt("t2")
                nc.vector.tensor_mul(ct_t2[:], ct_tau[:], ct_tau[:])
                ct_d = ctt("d")
                nc.vector.tensor_mul(ct_d[:], ct_t2[:], ct_s2[:])
                nc.vector.tensor_add(ct_d[:], ct_d[:], ct_c2[:])
                ct_r = ctt("r")
                nc.vector.reciprocal(ct_r[:], ct_d[:])
                nc.vector.tensor_scalar(ct_t2[:], ct_t2[:], -1.0, 1.0, OP.mult, OP.add)
                ct_u = ctt("u")
                nc.vector.tensor_mul(ct_u[:], ct_sc[:], ct_t2[:])
                # T = [Tr | Ti] fp16 ; Tf = [-Ti | Tr]   (same (j, cg) layout)
                nc.vector.tensor_mul(t_t[:, cgs, 0:64], v2(ct_tau), v2(ct_r))
                nc.vector.tensor_mul(t_t[:, cgs, 64:128], v2(ct_u), v2(ct_r))
                nc.vector.tensor_scalar_mul(tf_t[:, cgs, 0:64], t_t[:, cgs, 64:128], -1.0)
                nc.vector.tensor_copy(tf_t[:, cgs, 64:128], t_t[:, cgs, 0:64])

                # ----- attn2: TT assembly + Y for this half's batches -----
                # global b = 16hf + 4g4 + 2j + par lives at t[64j:64j+64, cg],
                # cg = 8hf + 2g4 + par; b factors as (hg=(hf,g4), j, par).
                # Parity-matched halves via DVE, others via SWDGE SBUF DMAs.
                hgs = slice(4 * hf, 4 * hf + 4)
                def cg_view(t):
                    return t.rearrange("p (hg par) c -> p hg par c", par=2)
                nc.vector.tensor_copy(tt_v[0:64, hgs, 0, :, :], cg_view(t_t[0:64, cgs, :]))
                nc.vector.tensor_copy(tt_v[64:128, hgs, 1, :, :], cg_view(tf_t[64:128, cgs, :]))
                nc.gpsimd.dma_start(out=tt_v[0:64, hgs, 1, :, :], in_=cg_view(t_t[64:128, cgs, :]))
                nc.gpsimd.dma_start(out=tt_v[64:128, hgs, 0, :, :], in_=cg_view(tf_t[0:64, cgs, :]))
                # Y in plain global-b order
                for b4 in range(4 * hf, 4 * hf + 4):
                    yp = at_ps.tile([E, 4, 128], F32, tag="pt", bufs=2, name=f"yp{b4}")
                    for j in range(4):
                        b = b4 * 4 + j
                        nc.tensor.matmul(yp[:, j, :], km16_t[:, b, :], tt_t[:, b, :],
                                         start=True, stop=True)
                    dst = y_t[:, b4 * 4:(b4 + 1) * 4, :, :]
                    srcv = yp[:].rearrange("p b (ri m) -> p b ri m", m=M)
                    if b4 % 2 == 0:
                        nc.scalar.copy(dst, srcv)
                    else:
                        nc.vector.tensor_copy(dst, srcv)
                # Yf = [-Yi | Yr] for the dual-accumulate weight stage
                hb = slice(16 * hf, 16 * hf + 16)
                nc.vector.tensor_scalar_mul(yf_t[:, hb, 0, :], y_t[:, hb, 1, :], -1.0)
                nc.vector.tensor_copy(yf_t[:, hb, 1, :], y_t[:, hb, 0, :])

            # ---------- stages 7-9, per half ----------
            # Zr = Wr^T Yr - Wi^T Yi ; Zi = Wr^T Yi + Wi^T Yr, via two
            # accumulating matmuls: Wr^T @ [Yr|Yi] + Wi^T @ [-Yi|Yr].
            # Then Z transposes -> Z' [(ri,x), (b, o)] and irfft out = Z'^T G.
            # PSUM comes from the transpose tag (free once transposes done).
            def stage789_half(hf):
                b0 = 16 * hf
                for x8 in range(M // 8):
                    wp = tp_ps.tile([O, 8, HB * 2], F32, tag="tp", bufs=2,
                                    name=f"wp{hf}_{x8}")
                    for j in range(8):
                        x = x8 * 8 + j
                        yv = y_t[:, b0:b0 + HB, :, x].rearrange("p b ri -> p (b ri)")
                        yfv = yf_t[:, b0:b0 + HB, :, x].rearrange("p b ri -> p (b ri)")
                        nc.tensor.matmul(wp[:, j, :], w_t[:, 0, x, :], yv,
                                         start=True, stop=False)
                        nc.tensor.matmul(wp[:, j, :], w_t[:, 1, x, :], yfv,
                                         start=False, stop=True)
                    dst = z_t[:, b0:b0 + HB, :, x8 * 8:(x8 + 1) * 8].rearrange("p b ri x -> p x b ri")
                    srcv = wp[:].rearrange("p x (b ri) -> p x b ri", ri=2)
                    if x8 % 2 == 0:
                        nc.scalar.copy(dst, srcv)
                    else:
                        nc.vector.tensor_copy(dst, srcv)

                for b8 in range(2 * hf, 2 * hf + 2):
                    zt = tp_ps.tile([128, 8, O], F16, tag="tp", bufs=2,
                                    name=f"zt{b8}")
                    for j in range(8):
                        b = b8 * 8 + j
                        nc.tensor.transpose(
                            zt[:, j, :],
                            z_t[:, b, :, :].rearrange("p ri m -> p (ri m)"),
                            idk16[:],
                        )
                    if b8 % 2 == 0:
                        nc.vector.tensor_copy(zp_g[b8][:], zt[:])
                    else:
                        nc.scalar.copy(zp_g[b8][:], zt[:])

                # irfft + fp16 output (host applies OUT_SCALE; fp16 can't
                # hold out*2^-40 without underflow)
                for bp in range(8 * hf, 8 * hf + 8):
                    otg = outs.tile([128, 1024], F16, tag="ot", name=f"ot{bp}")
                    for g in range(2):
                        opg = tp_ps.tile([128, 512], F32, tag="tp", bufs=2,
                                         name=f"op{bp}_{g}")
                        nc.tensor.matmul(
                            opg[:, :],
                            zp_g[bp // 4][:, (bp % 4) * 2:(bp % 4) * 2 + 2, :]
                            .rearrange("p b o -> p (b o)"),
                            g_t[:, g * 512:(g + 1) * 512],
                            start=True, stop=True,
                        )
                        if (bp + g) % 2 == 0:
                            nc.scalar.copy(otg[:, g * 512:(g + 1) * 512], opg[:])
                        else:
                            nc.vector.tensor_copy(otg[:, g * 512:(g + 1) * 512], opg[:])
                    nc.sync.dma_start(out=out_d[bp], in_=otg[:])

            # ---------- main per-half pipeline ----------
            for hf in range(NHALF):
                # ----- stage 1+2: DFT (hi/lo 3-pass) -----
                qm_ps = dft_ps.tile([128, 1024], F32, tag="qmps", name=f"qm_ps{hf}", bufs=1)
                km_ps = dft_ps.tile([128, 1024], F32, tag="kmps", name=f"km_ps{hf}", bufs=1)
                for c in range(NCHUNK):
                    x_c = chunks.tile([128, 4, HB * E], F16, tag="x", name=f"x{hf}_{c}")
                    nc.sync.dma_start(out=x_c, in_=x_d[c, hf])
                    first = c == 0
                    last = c == NCHUNK - 1
                    passes = (
                        (fh_t[:, c, :], 0, qm_ps, first, False),
                        (fh_t[:, c, :], 1, qm_ps, False, False),
                        (fh_t[:, c, :], 2, km_ps, first, False),
                        (fh_t[:, c, :], 3, km_ps, False, False),
                        (fl_t[:, c, :], 0, qm_ps, False, last),
                        (fl_t[:, c, :], 2, km_ps, False, last),
                    )
                    for lhs, ti, ps, is_start, is_stop in passes:
                        for g in range(2):
                            nc.tensor.matmul(
                                ps[:, g * 512:(g + 1) * 512],
                                lhs,
                                x_c[:, ti, g * 512:(g + 1) * 512],
                                start=is_start,
                                stop=is_stop,
                            )
                nc.vector.tensor_copy(qm_h[hf][:], qm_ps[:])
                nc.scalar.copy(km_h[hf][:], km_ps[:])
                nc.vector.tensor_copy(
                    km16_t[:, hf * HB:(hf + 1) * HB, :],
                    km_ps[:].rearrange("p (b e) -> p b e", e=E),
                )
                if hf == 0:
                    nc.vector.tensor_copy(idk16[:], idq_t[0:64, 0:64])

                # ----- stage 3: pair transposes -> Q_e, K_e -----
                # in [2m, (b0-e|b1-e)] -> out [(b0-e|b1-e), 2m]; even b on
                # partitions 0:64, odd on 64:128.
                qm_p = qm_h[hf][:].rearrange("p (bp c) -> p bp c", c=128)
                km_p = km_h[hf][:].rearrange("p (bp c) -> p bp c", c=128)
                for g2 in range(4):
                    tp = tp_ps.tile([128, 2, 128], F32, tag="tp")
                    tk = tp_ps.tile([128, 2, 128], F32, tag="tp", name=f"tk{hf}_{g2}")
                    for j in range(2):
                        bpl = g2 * 2 + j
                        nc.tensor.transpose(tp[:, j, :], qm_p[:, bpl, :], idq_t[:])
                        nc.tensor.transpose(tk[:, j, :], km_p[:, bpl, :], idq_t[:])
                    tpv = tp[:].rearrange("p j (ri y) -> p ri j y", ri=2)
                    tkv = tk[:].rearrange("p j (ri y) -> p ri j y", ri=2)
                    if g2 % 2 == 0:
                        nc.scalar.copy(qe_h[hf][:, :, g2 * 2:(g2 + 1) * 2, :], tpv)
                        nc.scalar.copy(ke_h[hf][:, :, g2 * 2:(g2 + 1) * 2, :], tkv)
                    else:
                        nc.vector.tensor_copy(qe_h[hf][:, :, g2 * 2:(g2 + 1) * 2, :], tpv)
                        nc.vector.tensor_copy(ke_h[hf][:, :, g2 * 2:(g2 + 1) * 2, :], tkv)
                nc.vector.tensor_scalar_mul(qf_h[hf][:, 0, :, :], qe_h[hf][:, 1, :, :], -1.0)
                nc.vector.tensor_copy(qf_h[hf][:, 1, :, :], qe_h[hf][:, 0, :, :])

                # ----- stage 4: attn1 -> X^T psum, A/B fp32 sbuf -----
                # 2 same-parity b per matmul pair (256 cols each):
                #   pt[(j, y), (ri', j', y')] = sum_e Kr_bj[e, y] Q_bj'[e, ri' y']
                #                            + sum_e Ki_bj[e, y] Qf_bj'[e, ri' y']
                # useful quadrants j == j'; partition-aligned extraction.
                for par in range(2):
                    base = 64 * par
                    sl = slice(base, base + 64)
                    for g4 in range(4):
                        pt = at_ps.tile([128, 2, 2, 64], F32, tag="pt", bufs=2)
                        psl = slice(2 * g4, 2 * g4 + 2)
                        nc.tensor.matmul(pt[:], ke_h[hf][sl, 0, psl, :],
                                         qe_h[hf][sl, :, psl, :],
                                         start=True, stop=False)
                        nc.tensor.matmul(pt[:], ke_h[hf][sl, 1, psl, :],
                                         qf_h[hf][sl, :, psl, :],
                                         start=False, stop=True)
                        cg = 8 * hf + 2 * g4 + par
                        if (par + g4) % 2 == 0:
                            nc.scalar.copy(ab_t[0:64, cg, :].rearrange("p (ri y) -> p ri y", ri=2), pt[0:64, :, 0, :])
                            nc.vector.tensor_copy(ab_t[64:128, cg, :].rearrange("p (ri y) -> p ri y", ri=2), pt[64:128, :, 1, :])
                        else:
                            nc.vector.tensor_copy(ab_t[0:64, cg, :].rearrange("p (ri y) -> p ri y", ri=2), pt[0:64, :, 0, :])
                            nc.scalar.copy(ab_t[64:128, cg, :].rearrange("p (ri y) -> p ri y", ri=2), pt[64:128, :, 1, :])

                tanh_attn2_half(hf)
                if hf == 1:
                    stage789_half(0)

            stage789_half(1)

    nc.compile()
    return nc


_NC_CACHE = None


def _get_nc():
    global _NC_CACHE
    if _NC_CACHE is None:
        _NC_CACHE = build()
    return _NC_CACHE


def _host_prep(q, k, Wr, Wi):
    """Build the 8 per-core input maps (numpy relayout/cast only)."""
    l = np.arange(L, dtype=np.float64)[:, None]
    m = np.arange(M, dtype=np.float64)[None, :]
    ang = 2.0 * np.pi * l * m / L
    F = np.concatenate([np.cos(ang), -np.sin(ang)], axis=1).astype(np.float32)  # [L, 2M]
    fh = F.astype(np.float16)
    fl = (F - fh.astype(np.float32)).astype(np.float16)
    # fh/fl as [p][(c, 2m)]
    fh = fh.reshape(NCHUNK, 128, 2 * M).transpose(1, 0, 2).reshape(128, 1024)
    fl = fl.reshape(NCHUNK, 128, 2 * M).transpose(1, 0, 2).reshape(128, 1024)

    cm = np.full(M, 2.0); cm[0] = 1.0
    ang2 = 2.0 * np.pi * m.T * np.arange(L, dtype=np.float64)[None, :] / L
    SC = 2.0 ** GSHIFT / (L * 512.0 * 512.0)
    g = np.concatenate([
        cm[:, None] * np.cos(ang2) * SC,
        -cm[:, None] * np.sin(ang2) * SC,
    ], axis=0).astype(np.float32).astype(np.float16)  # [2M, L]

    cpack = np.concatenate([fh, fl, g.astype(np.float16)], axis=1)  # [128, 3072]

    idq = np.eye(128, dtype=np.float32)

    maps = []
    for h in range(H):
        def split(x):
            xs = np.ascontiguousarray(x[:, :, h, :].transpose(1, 0, 2)).reshape(L, B * E)
            hi = xs.astype(np.float16)
            lo = (xs - hi.astype(np.float32)).astype(np.float16)
            return hi, lo
        qh, ql = split(q)
        kh, kl = split(k)
        # pack [c][hf][p][t][col]
        xp = np.empty((NCHUNK, NHALF, 128, 4, HB * E), np.float16)
        for t, src in enumerate((qh, ql, kh, kl)):
            s = src.reshape(NCHUNK, 128, NHALF, HB * E)
            xp[:, :, :, t, :] = s.transpose(0, 2, 1, 3)
        wpk = np.empty((E, 2, M, O), np.float32)
        wpk[:, 0] = (Wr[h] * 2.0 ** WSHIFT).transpose(0, 2, 1)  # [e,o,x]->[e,x,o]
        wpk[:, 1] = (Wi[h] * 2.0 ** WSHIFT).transpose(0, 2, 1)
        maps.append({
            "xp": xp,
            "cp": cpack,
            "wp": wpk.astype(np.float16),
            "idq": idq,
        })
    return maps


def kernel(q, k, v, Wr, Wi, _trace=False):
    q = np.asarray(q, np.float32)
    k = np.asarray(k, np.float32)
    Wr = np.asarray(Wr, np.float32)
    Wi = np.asarray(Wi, np.float32)
    nc = _get_nc()
    maps = _host_prep(q, k, Wr, Wi)
    try:
        res = run_bass_kernel_spmd(nc, maps, core_ids=list(range(H)), trace=_trace)
    except ModuleNotFoundError:
        res = run_bass_kernel_spmd(nc, maps, core_ids=list(range(H)), trace=False)
    # out_d[bp][p][l]: b = 2*bp + (p//64), o = p%64 -> plain b order
    out = np.empty((B, H, O, L), np.float32)
    for h in range(H):
        o = np.asarray(res.results[h]["out"], np.float32).reshape(B, O, L)
        o *= np.float32(OUT_SCALE)
        out[:, h] = o
    if _trace:
        kernel.last_results = res
    return out.astype(np.float32)


# revision 22
# speedup vs baseline: 1.2448x; 1.0037x over previous
"""FEDformer FourierCrossAttention kernel for 8 TRN2 NeuronCores.

Sharding: one head per core (H=8 == n_cores). Each core computes, for its head:
  Q = rfft(q)[:64 modes], K = rfft(k)[:64]      (DFT-as-matmul, hi/lo fp16 3-pass)
  X^T = K^T Q (complex, contract E)             (2-batch 256-col matmuls)
  T = tanh(X) (complex, tau/sin/cos form)       (ACT tanh+sin, DVE cody-waite RR)
  Y = sum_y T[x,y] K[e,y]                       (fp16 matmuls)
  Z = sum_e W[e,o,x] Y[e,x]   (W scaled 2^16)   (dual-accumulate Wr/Wi fp16 matmuls)
  out = irfft(Z / (512*512))  (G scaled 2^24)   (fp16 matmuls; 2^-40 applied on host)

The whole pipeline is split per batch-half (hf): half 0's attn/tanh/output
stages overlap half 1's DMA-paced DFT, and half 0's weight/irfft stages fill
the PE-idle window of half 1's tanh chain.

Batch indexing: global b = 16*hf + 4*g4 + 2*j + par, stored in the attn/tanh
stages at partition half j (pair LSB) and column group cg = 8*hf + 2*g4 + par.
Y/Z/out stages use plain global b ordering.
"""
import numpy as np

import concourse.bass as bass
import concourse.tile as tile
from concourse import bacc, mybir
from concourse.bass_utils import run_bass_kernel_spmd

F32 = mybir.dt.float32
F16 = mybir.dt.float16
AF = mybir.ActivationFunctionType
OP = mybir.AluOpType

B, L, H, E, O, M = 32, 1024, 8, 64, 64, 64
NCHUNK = 8          # contraction chunks of 128 over L
NHALF = 2           # batch halves of 16 for DFT PSUM
WSHIFT = 16         # W scaled by 2^WSHIFT on host
GSHIFT = 24         # G scaled by 2^GSHIFT on host
OUT_SCALE = 2.0 ** (-WSHIFT - GSHIFT)
HB = B // NHALF     # 16 batches per half

PI = np.float64(np.pi)
PI_HI = np.float32(3.140625)
PI_MID = np.float32(PI - np.float64(np.float32(3.140625)))
PI_LO = np.float32(PI - np.float64(np.float32(3.140625)) - np.float64(PI_MID))
MAGIC = np.float32(1.5 * 2 ** 23)   # round-to-nearest via add/sub
RH_LIM = np.nextafter(np.float32(np.pi) - np.float32(np.pi / 2), np.float32(0))


def build(debug=False):
    nc = bacc.Bacc("TRN2", target_bir_lowering=False, debug=False, num_devices=8)

    # ---- I/O (per-core, host pre-sharded/relaid) ----
    # packed q/k hi+lo fp16: [c][hf][p][t][col]; t in {qh, ql, kh, kl},
    # col = b_local*64 + e  (b_local = b - 16*hf)
    x_d = nc.dram_tensor("xp", (NCHUNK, NHALF, 128, 4, HB * E), F16,
                         kind="ExternalInput")
    # packed fp16 consts: [p][fh(8*128) | fl(8*128) | g(1024)]
    c_d = nc.dram_tensor("cp", (128, 3 * 1024), F16, kind="ExternalInput")
    # W packed fp16 (x2^16): [e][ri][x][o] = W{ri}[e, o, x]
    w_d = nc.dram_tensor("wp", (E, 2, M, O), F16, kind="ExternalInput")
    # transpose helper fp32
    idq_d = nc.dram_tensor("idq", (128, 128), F32, kind="ExternalInput")

    # out[bp][p][l]: p = (pair half)*64 + o; global b = 2*bp + (p>=64)
    out_d = nc.dram_tensor("out", (B // 2, 128, L), F16, kind="ExternalOutput")

    with tile.TileContext(nc) as tc:
        from contextlib import ExitStack
        stack = ExitStack()
        with stack:
            consts = stack.enter_context(tc.tile_pool(name="consts", bufs=1))
            chunks = stack.enter_context(tc.tile_pool(name="chunks", bufs=4))
            coeff = stack.enter_context(tc.tile_pool(name="coeff", bufs=1))
            work = stack.enter_context(tc.tile_pool(name="work", bufs=1))
            tmp = stack.enter_context(tc.tile_pool(name="tmp", bufs=1))
            outs = stack.enter_context(tc.tile_pool(name="outs", bufs=4))
            dft_ps = stack.enter_context(tc.tile_pool(name="dft_ps", bufs=1, space="PSUM"))
            tp_ps = stack.enter_context(tc.tile_pool(name="tp_ps", bufs=2, space="PSUM"))
            at_ps = stack.enter_context(tc.tile_pool(name="at_ps", bufs=2, space="PSUM"))

            # ---------- constants ----------
            c_t = consts.tile([128, 3 * 1024], F16, tag="cp")
            w_t = consts.tile([E, 2, M, O], F16, tag="w")
            idq_t = consts.tile([128, 128], F32, tag="idq")
            nc.scalar.dma_start(out=c_t, in_=c_d[:])
            nc.scalar.dma_start(out=w_t, in_=w_d[:])
            nc.scalar.dma_start(out=idq_t, in_=idq_d[:])
            fh_t = c_t[:, 0:1024].rearrange("p (c m) -> p c m", m=2 * M)
            fl_t = c_t[:, 1024:2048].rearrange("p (c m) -> p c m", m=2 * M)
            g_t = c_t[:, 2048:3072]

            # ---------- persistent state ----------
            qm_h = [coeff.tile([128, 1024], F32, tag=f"qmh{hf}", name=f"qm_h{hf}")
                    for hf in range(NHALF)]
            km_h = [coeff.tile([128, 1024], F32, tag=f"kmh{hf}", name=f"km_h{hf}")
                    for hf in range(NHALF)]
            km16_t = coeff.tile([128, B, E], F16, tag="km16")
            # layout [p=(par,e), ri, bp, y] so stationary attn1 slices merge
            # into a single contiguous free dim (BIR requirement)
            qe_h = [work.tile([128, 2, 8, 64], F32, tag=f"qeh{hf}", name=f"qe_h{hf}") for hf in range(NHALF)]
            ke_h = [work.tile([128, 2, 8, 64], F32, tag=f"keh{hf}", name=f"ke_h{hf}") for hf in range(NHALF)]
            qf_h = [work.tile([128, 2, 8, 64], F32, tag=f"qfh{hf}", name=f"qf_h{hf}") for hf in range(NHALF)]
            # A/B packed: ab[p = 64*j + y, cg, 0:64 = Re X^T, 64:128 = Im X^T]
            ab_t = work.tile([128, 16, 128], F32, tag="ab")
            halfpi = consts.tile([128, 1], F32, tag="halfpi", name="halfpi")
            nc.vector.memset(halfpi[:], float(np.pi / 2))
            t_t = work.tile([128, 16, 128], F16, tag="t")
            tf_t = work.tile([128, 16, 128], F16, tag="tf")
            tt_t = work.tile([128, B, 128], F16, tag="tt")
            tt_v = tt_t[:].rearrange("p (hg j par) c -> p hg j par c", j=2, par=2)
            y_t = work.tile([E, B, 2, M], F16, tag="y")
            yf_t = work.tile([E, B, 2, M], F16, tag="yf")
            z_t = work.tile([O, B, 2, M], F16, tag="z")
            zp_g = [work.tile([128, 8, O], F16, tag=f"zp{g}", name=f"zp_g{g}")
                    for g in range(B // 8)]
            idk16 = consts.tile([64, 64], F16, tag="id16")

            # ---------- stage 5+6, per half: complex tanh + attn2 ----------
            def tanh_half(hf):
                cgs = slice(8 * hf, 8 * hf + 8)
                # A=Re X^T, B=Im X^T, strided views of ab_t [128, 8, 64]
                av = ab_t[:, cgs, 0:64]
                bv = ab_t[:, cgs, 64:128]
                def ctt(n):
                    return tmp.tile([128, 512], F32, tag="ct", name=f"ct_{n}{hf}", bufs=6)
                def v2(t):
                    return t[:].rearrange("p (g m) -> p g m", m=64)
                ct_n = ctt("n")
                nc.vector.tensor_scalar(v2(ct_n), bv, float(1.0 / PI), float(MAGIC), OP.mult, OP.add)
                nc.vector.tensor_scalar_sub(ct_n[:], ct_n[:], float(MAGIC))
                ct_rh = ctt("rh")
                nc.vector.cody_waite_cascade(v2(ct_rh), bv, ct_n[:], float(PI_HI), float(PI_MID), float(PI_LO))
                # clamp |rh| so rh+pi/2 (cos) and 2*rh (sin) stay in [-pi, pi]
                nc.vector.tensor_scalar(ct_rh[:], ct_rh[:], -float(RH_LIM), float(RH_LIM), OP.max, OP.min)
                ct_tau = ctt("tau")
                nc.scalar.activation(v2(ct_tau), av, AF.Tanh)
                ct_s = ctt("s")
                nc.scalar.activation(ct_s[:], ct_rh[:], AF.Sin)
                ct_c = ctt("c")
                nc.scalar.activation(ct_c[:], ct_rh[:], AF.Sin, bias=halfpi[:])
                # squares on DVE (keeps ACT on the Tanh/Sin table set: a table
                # reload costs 1.3us and lands on the critical chain)
                ct_s2 = ctt("s2")
                nc.vector.tensor_mul(ct_s2[:], ct_s[:], ct_s[:])
                ct_c2 = ctt("c2")
                nc.vector.tensor_mul(ct_c2[:], ct_c[:], ct_c[:])
                ct_sc = ctt("sc")
                nc.vector.tensor_mul(ct_sc[:], ct_s[:], ct_c[:])
                ct_t2 = ctt("t2")
                nc.vector.tensor_mul(ct_t2[:], ct_tau[:], ct_tau[:])
                ct_d = ctt("d")
                nc.vector.tensor_mul(ct_d[:], ct_t2[:], ct_s2[:])
                nc.vector.tensor_add(ct_d[:], ct_d[:], ct_c2[:])
                ct_r = ctt("r")
                nc.vector.reciprocal(ct_r[:], ct_d[:])
                nc.vector.tensor_scalar(ct_t2[:], ct_t2[:], -1.0, 1.0, OP.mult, OP.add)
                ct_u = ctt("u")
                nc.vector.tensor_mul(ct_u[:], ct_sc[:], ct_t2[:])
                # T = [Tr | Ti] fp16 ; Tf = [-Ti | Tr]   (same (j, cg) layout)
                nc.vector.tensor_mul(t_t[:, cgs, 0:64], v2(ct_tau), v2(ct_r))
                nc.vector.tensor_mul(t_t[:, cgs, 64:128], v2(ct_u), v2(ct_r))
                nc.vector.tensor_scalar_mul(tf_t[:, cgs, 0:64], t_t[:, cgs, 64:128], -1.0)
                nc.vector.tensor_copy(tf_t[:, cgs, 64:128], t_t[:, cgs, 0:64])

                # TT assembly for this half (feeds attn2, emitted later):
                # global b = 16hf + 4g4 + 2j + par lives at t[64j:64j+64, cg],
                # cg = 8hf + 2g4 + par; b factors as (hg=(hf,g4), j, par).
                # Parity-matched halves via DVE, others via SWDGE SBUF DMAs.
                hgs = slice(4 * hf, 4 * hf + 4)
                def cg_view(t):
                    return t.rearrange("p (hg par) c -> p hg par c", par=2)
                nc.vector.tensor_copy(tt_v[0:64, hgs, 0, :, :], cg_view(t_t[0:64, cgs, :]))
                nc.vector.tensor_copy(tt_v[64:128, hgs, 1, :, :], cg_view(tf_t[64:128, cgs, :]))
                nc.gpsimd.dma_start(out=tt_v[0:64, hgs, 1, :, :], in_=cg_view(t_t[64:128, cgs, :]))
                nc.gpsimd.dma_start(out=tt_v[64:128, hgs, 0, :, :], in_=cg_view(tf_t[0:64, cgs, :]))

            # ---------- attn2 (PE side), per half ----------
            # emitted in dependency-ready order: PE queues are in-order, so a
            # matmul waiting on the tanh chain must not be emitted before PE
            # work whose inputs are already available.
            def attn2_half(hf, copy_eng):
                # Y in plain global-b order
                for b4 in range(4 * hf, 4 * hf + 4):
                    yp = at_ps.tile([E, 4, 128], F32, tag="pt", bufs=2, name=f"yp{b4}")
                    for j in range(4):
                        b = b4 * 4 + j
                        nc.tensor.matmul(yp[:, j, :], km16_t[:, b, :], tt_t[:, b, :],
                                         start=True, stop=True)
                    dst = y_t[:, b4 * 4:(b4 + 1) * 4, :, :]
                    srcv = yp[:].rearrange("p b (ri m) -> p b ri m", m=M)
                    if copy_eng == "act" or b4 % 2 == 0:
                        nc.scalar.copy(dst, srcv)
                    else:
                        nc.vector.tensor_copy(dst, srcv)
                # Yf = [-Yi | Yr] for the dual-accumulate weight stage
                hb = slice(16 * hf, 16 * hf + 16)
                if copy_eng == "act":
                    nc.scalar.mul(yf_t[:, hb, 0, :], y_t[:, hb, 1, :], -1.0)
                    nc.scalar.copy(yf_t[:, hb, 1, :], y_t[:, hb, 0, :])
                else:
                    nc.vector.tensor_scalar_mul(yf_t[:, hb, 0, :], y_t[:, hb, 1, :], -1.0)
                    nc.vector.tensor_copy(yf_t[:, hb, 1, :], y_t[:, hb, 0, :])

            # ---------- stages 7-9, per half ----------
            # Zr = Wr^T Yr - Wi^T Yi ; Zi = Wr^T Yi + Wi^T Yr, via two
            # accumulating matmuls: Wr^T @ [Yr|Yi] + Wi^T @ [-Yi|Yr].
            # Then Z transposes -> Z' [(ri,x), (b, o)] and irfft out = Z'^T G.
            # PSUM comes from the transpose tag (free once transposes done).
            def stage789_half(hf, copy_eng):
                b0 = 16 * hf
                for x8 in range(M // 8):
                    wp = tp_ps.tile([O, 8, HB * 2], F32, tag="tp", bufs=2,
                                    name=f"wp{hf}_{x8}")
                    for j in range(8):
                        x = x8 * 8 + j
                        yv = y_t[:, b0:b0 + HB, :, x].rearrange("p b ri -> p (b ri)")
                        yfv = yf_t[:, b0:b0 + HB, :, x].rearrange("p b ri -> p (b ri)")
                        nc.tensor.matmul(wp[:, j, :], w_t[:, 0, x, :], yv,
                                         start=True, stop=False)
                        nc.tensor.matmul(wp[:, j, :], w_t[:, 1, x, :], yfv,
                                         start=False, stop=True)
                    dst = z_t[:, b0:b0 + HB, :, x8 * 8:(x8 + 1) * 8].rearrange("p b ri x -> p x b ri")
                    srcv = wp[:].rearrange("p x (b ri) -> p x b ri", ri=2)
                    if copy_eng == "act" or x8 % 2 == 0:
                        nc.scalar.copy(dst, srcv)
                    else:
                        nc.vector.tensor_copy(dst, srcv)

                for b8 in range(2 * hf, 2 * hf + 2):
                    zt = tp_ps.tile([128, 8, O], F16, tag="tp", bufs=2,
                                    name=f"zt{b8}")
                    for j in range(8):
                        b = b8 * 8 + j
                        nc.tensor.transpose(
                            zt[:, j, :],
                            z_t[:, b, :, :].rearrange("p ri m -> p (ri m)"),
                            idk16[:],
                        )
                    if copy_eng == "act" or b8 % 2 == 1:
                        nc.scalar.copy(zp_g[b8][:], zt[:])
                    else:
                        nc.vector.tensor_copy(zp_g[b8][:], zt[:])

                # irfft + fp16 output (host applies OUT_SCALE; fp16 can't
                # hold out*2^-40 without underflow)
                for bp in range(8 * hf, 8 * hf + 8):
                    otg = outs.tile([128, 1024], F16, tag="ot", name=f"ot{bp}")
                    for g in range(2):
                        opg = tp_ps.tile([128, 512], F32, tag="tp", bufs=2,
                                         name=f"op{bp}_{g}")
                        nc.tensor.matmul(
                            opg[:, :],
                            zp_g[bp // 4][:, (bp % 4) * 2:(bp % 4) * 2 + 2, :]
                            .rearrange("p b o -> p (b o)"),
                            g_t[:, g * 512:(g + 1) * 512],
                            start=True, stop=True,
                        )
                        if copy_eng == "act" or (bp + g) % 2 == 0:
                            nc.scalar.copy(otg[:, g * 512:(g + 1) * 512], opg[:])
                        else:
                            nc.vector.tensor_copy(otg[:, g * 512:(g + 1) * 512], opg[:])
                    nc.sync.dma_start(out=out_d[bp], in_=otg[:])

            # ---------- main per-half pipeline ----------
            for hf in range(NHALF):
                # ----- stage 1+2: DFT (hi/lo 3-pass) -----
                qm_ps = dft_ps.tile([128, 1024], F32, tag="qmps", name=f"qm_ps{hf}", bufs=1)
                km_ps = dft_ps.tile([128, 1024], F32, tag="kmps", name=f"km_ps{hf}", bufs=1)
                for c in range(NCHUNK):
                    x_c = chunks.tile([128, 4, HB * E], F16, tag="x", name=f"x{hf}_{c}")
                    nc.sync.dma_start(out=x_c, in_=x_d[c, hf])
                    first = c == 0
                    last = c == NCHUNK - 1
                    passes = (
                        (fh_t[:, c, :], 0, qm_ps, first, False),
                        (fh_t[:, c, :], 1, qm_ps, False, False),
                        (fh_t[:, c, :], 2, km_ps, first, False),
                        (fh_t[:, c, :], 3, km_ps, False, False),
                        (fl_t[:, c, :], 0, qm_ps, False, last),
                        (fl_t[:, c, :], 2, km_ps, False, last),
                    )
                    for lhs, ti, ps, is_start, is_stop in passes:
                        for g in range(2):
                            nc.tensor.matmul(
                                ps[:, g * 512:(g + 1) * 512],
                                lhs,
                                x_c[:, ti, g * 512:(g + 1) * 512],
                                start=is_start,
                                stop=is_stop,
                            )
                nc.vector.tensor_copy(qm_h[hf][:], qm_ps[:])
                nc.scalar.copy(km_h[hf][:], km_ps[:])
                nc.vector.tensor_copy(
                    km16_t[:, hf * HB:(hf + 1) * HB, :],
                    km_ps[:].rearrange("p (b e) -> p b e", e=E),
                )
                if hf == 0:
                    nc.vector.tensor_copy(idk16[:], idq_t[0:64, 0:64])

                # ----- stage 3: pair transposes -> Q_e, K_e -----
                # in [2m, (b0-e|b1-e)] -> out [(b0-e|b1-e), 2m]; even b on
                # partitions 0:64, odd on 64:128.
                qm_p = qm_h[hf][:].rearrange("p (bp c) -> p bp c", c=128)
                km_p = km_h[hf][:].rearrange("p (bp c) -> p bp c", c=128)
                for g2 in range(4):
                    tp = tp_ps.tile([128, 2, 128], F32, tag="tp")
                    tk = tp_ps.tile([128, 2, 128], F32, tag="tp", name=f"tk{hf}_{g2}")
                    for j in range(2):
                        bpl = g2 * 2 + j
                        nc.tensor.transpose(tp[:, j, :], qm_p[:, bpl, :], idq_t[:])
                        nc.tensor.transpose(tk[:, j, :], km_p[:, bpl, :], idq_t[:])
                    tpv = tp[:].rearrange("p j (ri y) -> p ri j y", ri=2)
                    tkv = tk[:].rearrange("p j (ri y) -> p ri j y", ri=2)
                    if g2 % 2 == 0:
                        nc.scalar.copy(qe_h[hf][:, :, g2 * 2:(g2 + 1) * 2, :], tpv)
                        nc.scalar.copy(ke_h[hf][:, :, g2 * 2:(g2 + 1) * 2, :], tkv)
                    else:
                        nc.vector.tensor_copy(qe_h[hf][:, :, g2 * 2:(g2 + 1) * 2, :], tpv)
                        nc.vector.tensor_copy(ke_h[hf][:, :, g2 * 2:(g2 + 1) * 2, :], tkv)
                nc.vector.tensor_scalar_mul(qf_h[hf][:, 0, :, :], qe_h[hf][:, 1, :, :], -1.0)
                nc.vector.tensor_copy(qf_h[hf][:, 1, :, :], qe_h[hf][:, 0, :, :])

                # ----- stage 4: attn1 -> X^T psum, A/B fp32 sbuf -----
                # 2 same-parity b per matmul pair (256 cols each):
                #   pt[(j, y), (ri', j', y')] = sum_e Kr_bj[e, y] Q_bj'[e, ri' y']
                #                            + sum_e Ki_bj[e, y] Qf_bj'[e, ri' y']
                # useful quadrants j == j'; partition-aligned extraction.
                for par in range(2):
                    base = 64 * par
                    sl = slice(base, base + 64)
                    for g4 in range(4):
                        pt = at_ps.tile([128, 2, 2, 64], F32, tag="pt", bufs=2)
                        psl = slice(2 * g4, 2 * g4 + 2)
                        nc.tensor.matmul(pt[:], ke_h[hf][sl, 0, psl, :],
                                         qe_h[hf][sl, :, psl, :],
                                         start=True, stop=False)
                        nc.tensor.matmul(pt[:], ke_h[hf][sl, 1, psl, :],
                                         qf_h[hf][sl, :, psl, :],
                                         start=False, stop=True)
                        cg = 8 * hf + 2 * g4 + par
                        if (par + g4) % 2 == 0:
                            nc.scalar.copy(ab_t[0:64, cg, :].rearrange("p (ri y) -> p ri y", ri=2), pt[0:64, :, 0, :])
                            nc.vector.tensor_copy(ab_t[64:128, cg, :].rearrange("p (ri y) -> p ri y", ri=2), pt[64:128, :, 1, :])
                        else:
                            nc.vector.tensor_copy(ab_t[0:64, cg, :].rearrange("p (ri y) -> p ri y", ri=2), pt[0:64, :, 0, :])
                            nc.scalar.copy(ab_t[64:128, cg, :].rearrange("p (ri y) -> p ri y", ri=2), pt[64:128, :, 1, :])

                tanh_half(hf)

            # PE-side emission in dependency-ready order: h0's attn2 and
            # stages 7-9 fill the PE-idle window of h1's tanh chain; their
            # psum->sbuf copies ride on ACT, which the tanh chain barely uses.
            attn2_half(0, "act")
            stage789_half(0, "act")
            attn2_half(1, "mix")
            stage789_half(1, "mix")

    nc.compile()
    return nc


_NC_CACHE = None


def _get_nc():
    global _NC_CACHE
    if _NC_CACHE is None:
        _NC_CACHE = build()
    return _NC_CACHE


def _host_prep(q, k, Wr, Wi):
    """Build the 8 per-core input maps (numpy relayout/cast only)."""
    l = np.arange(L, dtype=np.float64)[:, None]
    m = np.arange(M, dtype=np.float64)[None, :]
    ang = 2.0 * np.pi * l * m / L
    F = np.concatenate([np.cos(ang), -np.sin(ang)], axis=1).astype(np.float32)  # [L, 2M]
    fh = F.astype(np.float16)
    fl = (F - fh.astype(np.float32)).astype(np.float16)
    # fh/fl as [p][(c, 2m)]
    fh = fh.reshape(NCHUNK, 128, 2 * M).transpose(1, 0, 2).reshape(128, 1024)
    fl = fl.reshape(NCHUNK, 128, 2 * M).transpose(1, 0, 2).reshape(128, 1024)

    cm = np.full(M, 2.0); cm[0] = 1.0
    ang2 = 2.0 * np.pi * m.T * np.arange(L, dtype=np.float64)[None, :] / L
    SC = 2.0 ** GSHIFT / (L * 512.0 * 512.0)
    g = np.concatenate([
        cm[:, None] * np.cos(ang2) * SC,
        -cm[:, None] * np.sin(ang2) * SC,
    ], axis=0).astype(np.float32).astype(np.float16)  # [2M, L]

    cpack = np.concatenate([fh, fl, g.astype(np.float16)], axis=1)  # [128, 3072]

    idq = np.eye(128, dtype=np.float32)

    maps = []
    for h in range(H):
        def split(x):
            xs = np.ascontiguousarray(x[:, :, h, :].transpose(1, 0, 2)).reshape(L, B * E)
            hi = xs.astype(np.float16)
            lo = (xs - hi.astype(np.float32)).astype(np.float16)
            return hi, lo
        qh, ql = split(q)
        kh, kl = split(k)
        # pack [c][hf][p][t][col]
        xp = np.empty((NCHUNK, NHALF, 128, 4, HB * E), np.float16)
        for t, src in enumerate((qh, ql, kh, kl)):
            s = src.reshape(NCHUNK, 128, NHALF, HB * E)
            xp[:, :, :, t, :] = s.transpose(0, 2, 1, 3)
        wpk = np.empty((E, 2, M, O), np.float32)
        wpk[:, 0] = (Wr[h] * 2.0 ** WSHIFT).transpose(0, 2, 1)  # [e,o,x]->[e,x,o]
        wpk[:, 1] = (Wi[h] * 2.0 ** WSHIFT).transpose(0, 2, 1)
        maps.append({
            "xp": xp,
            "cp": cpack,
            "wp": wpk.astype(np.float16),
            "idq": idq,
        })
    return maps


def kernel(q, k, v, Wr, Wi, _trace=False):
    q = np.asarray(q, np.float32)
    k = np.asarray(k, np.float32)
    Wr = np.asarray(Wr, np.float32)
    Wi = np.asarray(Wi, np.float32)
    nc = _get_nc()
    maps = _host_prep(q, k, Wr, Wi)
    try:
        res = run_bass_kernel_spmd(nc, maps, core_ids=list(range(H)), trace=_trace)
    except ModuleNotFoundError:
        res = run_bass_kernel_spmd(nc, maps, core_ids=list(range(H)), trace=False)
    # out_d[bp][p][l]: b = 2*bp + (p//64), o = p%64 -> plain b order
    out = np.empty((B, H, O, L), np.float32)
    for h in range(H):
        o = np.asarray(res.results[h]["out"], np.float32).reshape(B, O, L)
        o *= np.float32(OUT_SCALE)
        out[:, h] = o
    if _trace:
        kernel.last_results = res
    return out.astype(np.float32)


# revision 24
# speedup vs baseline: 1.2558x; 1.0089x over previous
"""FEDformer FourierCrossAttention kernel for 8 TRN2 NeuronCores.

Sharding: one head per core (H=8 == n_cores). Each core computes, for its head:
  Q = rfft(q)[:64 modes], K = rfft(k)[:64]      (DFT-as-matmul, hi/lo fp16 3-pass)
  X^T = K^T Q (complex, contract E)             (2-batch 256-col matmuls)
  T = tanh(X) (complex, tau/sin/cos form)       (ACT tanh+sin, DVE cody-waite RR)
  Y = sum_y T[x,y] K[e,y]                       (fp16 matmuls)
  Z = sum_e W[e,o,x] Y[e,x]   (W scaled 2^16)   (dual-accumulate Wr/Wi fp16 matmuls)
  out = irfft(Z / (512*512))  (G scaled 2^24)   (fp16 matmuls; 2^-40 applied on host)

The whole pipeline is split per batch-half (hf): half 0's attn/tanh/output
stages overlap half 1's DMA-paced DFT, and half 0's weight/irfft stages fill
the PE-idle window of half 1's tanh chain.

Batch indexing: global b = 16*hf + 4*g4 + 2*j + par, stored in the attn/tanh
stages at partition half j (pair LSB) and column group cg = 8*hf + 2*g4 + par.
Y/Z/out stages use plain global b ordering.
"""
import numpy as np

import concourse.bass as bass
import concourse.tile as tile
from concourse import bacc, mybir
from concourse.bass_utils import run_bass_kernel_spmd

F32 = mybir.dt.float32
F16 = mybir.dt.float16
F32R = mybir.dt.float32r
AF = mybir.ActivationFunctionType
OP = mybir.AluOpType

B, L, H, E, O, M = 32, 1024, 8, 64, 64, 64
NCHUNK = 8          # contraction chunks of 128 over L
NHALF = 2           # batch halves of 16 for DFT PSUM
WSHIFT = 16         # W scaled by 2^WSHIFT on host
GSHIFT = 24         # G scaled by 2^GSHIFT on host
OUT_SCALE = 2.0 ** (-WSHIFT - GSHIFT)
HB = B // NHALF     # 16 batches per half

PI = np.float64(np.pi)
PI_HI = np.float32(3.140625)
PI_MID = np.float32(PI - np.float64(np.float32(3.140625)))
PI_LO = np.float32(PI - np.float64(np.float32(3.140625)) - np.float64(PI_MID))
MAGIC = np.float32(1.5 * 2 ** 23)   # round-to-nearest via add/sub
RH_LIM = np.nextafter(np.float32(np.pi) - np.float32(np.pi / 2), np.float32(0))


def build(debug=False):
    nc = bacc.Bacc("TRN2", target_bir_lowering=False, debug=False, num_devices=8)

    # ---- I/O (per-core, host pre-sharded/relaid) ----
    # packed q/k hi+lo fp16: [c][hf][p][t][col]; t in {qh, ql, kh, kl},
    # col = b_local*64 + e  (b_local = b - 16*hf)
    x_d = nc.dram_tensor("xp", (NCHUNK, NHALF, 128, 4, HB * E), F16,
                         kind="ExternalInput")
    # packed fp16 consts: [p][fh(8*128) | fl(8*128) | g(1024)]
    c_d = nc.dram_tensor("cp", (128, 3 * 1024), F16, kind="ExternalInput")
    # W packed fp16 (x2^16): [e][ri][x][o] = W{ri}[e, o, x]
    w_d = nc.dram_tensor("wp", (E, 2, M, O), F16, kind="ExternalInput")
    # transpose helper fp32
    idq_d = nc.dram_tensor("idq", (128, 128), F32, kind="ExternalInput")

    # out[bp][p][l]: p = (pair half)*64 + o; global b = 2*bp + (p>=64)
    out_d = nc.dram_tensor("out", (B // 2, 128, L), F16, kind="ExternalOutput")

    with tile.TileContext(nc) as tc:
        from contextlib import ExitStack
        stack = ExitStack()
        with stack:
            consts = stack.enter_context(tc.tile_pool(name="consts", bufs=1))
            chunks = stack.enter_context(tc.tile_pool(name="chunks", bufs=4))
            coeff = stack.enter_context(tc.tile_pool(name="coeff", bufs=1))
            work = stack.enter_context(tc.tile_pool(name="work", bufs=1))
            tmp = stack.enter_context(tc.tile_pool(name="tmp", bufs=1))
            outs = stack.enter_context(tc.tile_pool(name="outs", bufs=4))
            dft_ps = stack.enter_context(tc.tile_pool(name="dft_ps", bufs=1, space="PSUM"))
            tp_ps = stack.enter_context(tc.tile_pool(name="tp_ps", bufs=2, space="PSUM"))
            at_ps = stack.enter_context(tc.tile_pool(name="at_ps", bufs=2, space="PSUM"))

            # ---------- constants ----------
            c_t = consts.tile([128, 3 * 1024], F16, tag="cp")
            w_t = consts.tile([E, 2, M, O], F16, tag="w")
            idq_t = consts.tile([128, 128], F32, tag="idq")
            nc.scalar.dma_start(out=c_t, in_=c_d[:])
            nc.scalar.dma_start(out=w_t, in_=w_d[:])
            nc.scalar.dma_start(out=idq_t, in_=idq_d[:])
            fh_t = c_t[:, 0:1024].rearrange("p (c m) -> p c m", m=2 * M)
            fl_t = c_t[:, 1024:2048].rearrange("p (c m) -> p c m", m=2 * M)
            g_t = c_t[:, 2048:3072]

            # ---------- persistent state ----------
            qm_h = [coeff.tile([128, 1024], F32, tag=f"qmh{hf}", name=f"qm_h{hf}")
                    for hf in range(NHALF)]
            km_h = [coeff.tile([128, 1024], F32, tag=f"kmh{hf}", name=f"km_h{hf}")
                    for hf in range(NHALF)]
            km16_t = coeff.tile([128, B, E], F16, tag="km16")
            # layout [p=(par,e), ri, bp, y] so stationary attn1 slices merge
            # into a single contiguous free dim (BIR requirement)
            qe_h = [work.tile([128, 2, 8, 64], F32, tag=f"qeh{hf}", name=f"qe_h{hf}") for hf in range(NHALF)]
            ke_h = [work.tile([128, 2, 8, 64], F32, tag=f"keh{hf}", name=f"ke_h{hf}") for hf in range(NHALF)]
            qf_h = [work.tile([128, 2, 8, 64], F32, tag=f"qfh{hf}", name=f"qf_h{hf}") for hf in range(NHALF)]
            # A/B packed: ab[p = 64*j + y, cg, 0:64 = Re X^T, 64:128 = Im X^T]
            ab_t = work.tile([128, 16, 128], F32, tag="ab")
            halfpi = consts.tile([128, 1], F32, tag="halfpi", name="halfpi")
            nc.vector.memset(halfpi[:], float(np.pi / 2))
            t_t = work.tile([128, 16, 128], F16, tag="t")
            tf_t = work.tile([128, 16, 128], F16, tag="tf")
            tt_t = work.tile([128, B, 128], F16, tag="tt")
            tt_v = tt_t[:].rearrange("p (hg j par) c -> p hg j par c", j=2, par=2)
            y_t = work.tile([E, B, 2, M], F16, tag="y")
            yf_t = work.tile([E, B, 2, M], F16, tag="yf")
            z_t = work.tile([O, B, 2, M], F16, tag="z")
            zp_g = [work.tile([128, 8, O], F16, tag=f"zp{g}", name=f"zp_g{g}")
                    for g in range(B // 8)]
            idk16 = consts.tile([64, 64], F16, tag="id16")

            # ---------- stage 5+6, per half: complex tanh + attn2 ----------
            def tanh_half(hf):
                cgs = slice(8 * hf, 8 * hf + 8)
                # A=Re X^T, B=Im X^T, strided views of ab_t [128, 8, 64]
                av = ab_t[:, cgs, 0:64]
                bv = ab_t[:, cgs, 64:128]
                def ctt(n):
                    return tmp.tile([128, 512], F32, tag="ct", name=f"ct_{n}{hf}", bufs=6)
                def v2(t):
                    return t[:].rearrange("p (g m) -> p g m", m=64)
                ct_n = ctt("n")
                nc.vector.tensor_scalar(v2(ct_n), bv, float(1.0 / PI), float(MAGIC), OP.mult, OP.add)
                nc.vector.tensor_scalar_sub(ct_n[:], ct_n[:], float(MAGIC))
                ct_rh = ctt("rh")
                nc.vector.cody_waite_cascade(v2(ct_rh), bv, ct_n[:], float(PI_HI), float(PI_MID), float(PI_LO))
                # clamp |rh| so rh+pi/2 (cos) and 2*rh (sin) stay in [-pi, pi]
                nc.vector.tensor_scalar(ct_rh[:], ct_rh[:], -float(RH_LIM), float(RH_LIM), OP.max, OP.min)
                ct_tau = ctt("tau")
                nc.scalar.activation(v2(ct_tau), av, AF.Tanh)
                ct_s = ctt("s")
                nc.scalar.activation(ct_s[:], ct_rh[:], AF.Sin)
                ct_c = ctt("c")
                nc.scalar.activation(ct_c[:], ct_rh[:], AF.Sin, bias=halfpi[:])
                # squares on DVE (keeps ACT on the Tanh/Sin table set: a table
                # reload costs 1.3us and lands on the critical chain)
                ct_s2 = ctt("s2")
                nc.vector.tensor_mul(ct_s2[:], ct_s[:], ct_s[:])
                ct_c2 = ctt("c2")
                nc.vector.tensor_mul(ct_c2[:], ct_c[:], ct_c[:])
                ct_sc = ctt("sc")
                nc.vector.tensor_mul(ct_sc[:], ct_s[:], ct_c[:])
                ct_t2 = ctt("t2")
                nc.vector.tensor_mul(ct_t2[:], ct_tau[:], ct_tau[:])
                ct_d = ctt("d")
                nc.vector.tensor_mul(ct_d[:], ct_t2[:], ct_s2[:])
                nc.vector.tensor_add(ct_d[:], ct_d[:], ct_c2[:])
                ct_r = ctt("r")
                nc.vector.reciprocal(ct_r[:], ct_d[:])
                nc.vector.tensor_scalar(ct_t2[:], ct_t2[:], -1.0, 1.0, OP.mult, OP.add)
                ct_u = ctt("u")
                nc.vector.tensor_mul(ct_u[:], ct_sc[:], ct_t2[:])
                # T = [Tr | Ti] fp16 ; Tf = [-Ti | Tr]   (same (j, cg) layout)
                nc.vector.tensor_mul(t_t[:, cgs, 0:64], v2(ct_tau), v2(ct_r))
                nc.vector.tensor_mul(t_t[:, cgs, 64:128], v2(ct_u), v2(ct_r))
                nc.vector.tensor_scalar_mul(tf_t[:, cgs, 0:64], t_t[:, cgs, 64:128], -1.0)
                nc.vector.tensor_copy(tf_t[:, cgs, 64:128], t_t[:, cgs, 0:64])

                # TT assembly for this half (feeds attn2, emitted later):
                # global b = 16hf + 4g4 + 2j + par lives at t[64j:64j+64, cg],
                # cg = 8hf + 2g4 + par; b factors as (hg=(hf,g4), j, par).
                # Parity-matched halves via DVE, others via SWDGE SBUF DMAs.
                hgs = slice(4 * hf, 4 * hf + 4)
                def cg_view(t):
                    return t.rearrange("p (hg par) c -> p hg par c", par=2)
                nc.vector.tensor_copy(tt_v[0:64, hgs, 0, :, :], cg_view(t_t[0:64, cgs, :]))
                nc.vector.tensor_copy(tt_v[64:128, hgs, 1, :, :], cg_view(tf_t[64:128, cgs, :]))
                nc.gpsimd.dma_start(out=tt_v[0:64, hgs, 1, :, :], in_=cg_view(t_t[64:128, cgs, :]))
                nc.gpsimd.dma_start(out=tt_v[64:128, hgs, 0, :, :], in_=cg_view(tf_t[0:64, cgs, :]))

            # ---------- attn2 (PE side), per half ----------
            # emitted in dependency-ready order: PE queues are in-order, so a
            # matmul waiting on the tanh chain must not be emitted before PE
            # work whose inputs are already available.
            def attn2_half(hf, copy_eng):
                # Y in plain global-b order
                for b4 in range(4 * hf, 4 * hf + 4):
                    yp = at_ps.tile([E, 4, 128], F32, tag="pt", bufs=2, name=f"yp{b4}")
                    for j in range(4):
                        b = b4 * 4 + j
                        nc.tensor.matmul(yp[:, j, :], km16_t[:, b, :], tt_t[:, b, :],
                                         start=True, stop=True)
                    dst = y_t[:, b4 * 4:(b4 + 1) * 4, :, :]
                    srcv = yp[:].rearrange("p b (ri m) -> p b ri m", m=M)
                    if copy_eng == "act" or b4 % 2 == 0:
                        nc.scalar.copy(dst, srcv)
                    else:
                        nc.vector.tensor_copy(dst, srcv)
                # Yf = [-Yi | Yr] for the dual-accumulate weight stage
                hb = slice(16 * hf, 16 * hf + 16)
                if copy_eng == "act":
                    nc.scalar.mul(yf_t[:, hb, 0, :], y_t[:, hb, 1, :], -1.0)
                    nc.scalar.copy(yf_t[:, hb, 1, :], y_t[:, hb, 0, :])
                else:
                    nc.vector.tensor_scalar_mul(yf_t[:, hb, 0, :], y_t[:, hb, 1, :], -1.0)
                    nc.vector.tensor_copy(yf_t[:, hb, 1, :], y_t[:, hb, 0, :])

            # ---------- stages 7-9, per half ----------
            # Zr = Wr^T Yr - Wi^T Yi ; Zi = Wr^T Yi + Wi^T Yr, via two
            # accumulating matmuls: Wr^T @ [Yr|Yi] + Wi^T @ [-Yi|Yr].
            # Then Z transposes -> Z' [(ri,x), (b, o)] and irfft out = Z'^T G.
            # PSUM comes from the transpose tag (free once transposes done).
            def stage789_half(hf, z_eng, out_eng):
                b0 = 16 * hf
                for x8 in range(M // 8):
                    wp = tp_ps.tile([O, 8, HB * 2], F32, tag="tp", bufs=2,
                                    name=f"wp{hf}_{x8}")
                    for j in range(8):
                        x = x8 * 8 + j
                        yv = y_t[:, b0:b0 + HB, :, x].rearrange("p b ri -> p (b ri)")
                        yfv = yf_t[:, b0:b0 + HB, :, x].rearrange("p b ri -> p (b ri)")
                        nc.tensor.matmul(wp[:, j, :], w_t[:, 0, x, :], yv,
                                         start=True, stop=False)
                        nc.tensor.matmul(wp[:, j, :], w_t[:, 1, x, :], yfv,
                                         start=False, stop=True)
                    dst = z_t[:, b0:b0 + HB, :, x8 * 8:(x8 + 1) * 8].rearrange("p b ri x -> p x b ri")
                    srcv = wp[:].rearrange("p x (b ri) -> p x b ri", ri=2)
                    if z_eng == "act" or x8 % 2 == 0:
                        nc.scalar.copy(dst, srcv)
                    else:
                        nc.vector.tensor_copy(dst, srcv)

                for b8 in range(2 * hf, 2 * hf + 2):
                    zt = tp_ps.tile([128, 8, O], F16, tag="tp", bufs=2,
                                    name=f"zt{b8}")
                    for j in range(8):
                        b = b8 * 8 + j
                        nc.tensor.transpose(
                            zt[:, j, :],
                            z_t[:, b, :, :].rearrange("p ri m -> p (ri m)"),
                            idk16[:],
                        )
                    if z_eng == "act" or b8 % 2 == 1:
                        nc.scalar.copy(zp_g[b8][:], zt[:])
                    else:
                        nc.vector.tensor_copy(zp_g[b8][:], zt[:])

                # irfft + fp16 output (host applies OUT_SCALE; fp16 can't
                # hold out*2^-40 without underflow)
                for bp in range(8 * hf, 8 * hf + 8):
                    otg = outs.tile([128, 1024], F16, tag="ot", name=f"ot{bp}")
                    for g in range(2):
                        opg = tp_ps.tile([128, 512], F32, tag="tp", bufs=2,
                                         name=f"op{bp}_{g}")
                        nc.tensor.matmul(
                            opg[:, :],
                            zp_g[bp // 4][:, (bp % 4) * 2:(bp % 4) * 2 + 2, :]
                            .rearrange("p b o -> p (b o)"),
                            g_t[:, g * 512:(g + 1) * 512],
                            start=True, stop=True,
                        )
                        if out_eng == "act" or (bp + g) % 2 == 0:
                            nc.scalar.copy(otg[:, g * 512:(g + 1) * 512], opg[:])
                        else:
                            nc.vector.tensor_copy(otg[:, g * 512:(g + 1) * 512], opg[:])
                    nc.sync.dma_start(out=out_d[bp], in_=otg[:])

            # ---------- main per-half pipeline ----------
            def dft_half(hf):
                # ----- stage 1+2: DFT (hi/lo 3-pass) -----
                qm_ps = dft_ps.tile([128, 1024], F32, tag="qmps", name=f"qm_ps{hf}", bufs=1)
                km_ps = dft_ps.tile([128, 1024], F32, tag="kmps", name=f"km_ps{hf}", bufs=1)
                for c in range(NCHUNK):
                    x_c = chunks.tile([128, 4, HB * E], F16, tag="x", name=f"x{hf}_{c}")
                    nc.sync.dma_start(out=x_c, in_=x_d[c, hf])
                    first = c == 0
                    last = c == NCHUNK - 1
                    passes = (
                        (fh_t[:, c, :], 0, qm_ps, first, False),
                        (fh_t[:, c, :], 1, qm_ps, False, False),
                        (fh_t[:, c, :], 2, km_ps, first, False),
                        (fh_t[:, c, :], 3, km_ps, False, False),
                        (fl_t[:, c, :], 0, qm_ps, False, last),
                        (fl_t[:, c, :], 2, km_ps, False, last),
                    )
                    for lhs, ti, ps, is_start, is_stop in passes:
                        for g in range(2):
                            nc.tensor.matmul(
                                ps[:, g * 512:(g + 1) * 512],
                                lhs,
                                x_c[:, ti, g * 512:(g + 1) * 512],
                                start=is_start,
                                stop=is_stop,
                            )
                nc.vector.tensor_copy(qm_h[hf][:], qm_ps[:])
                nc.scalar.copy(km_h[hf][:], km_ps[:])
                nc.vector.tensor_copy(
                    km16_t[:, hf * HB:(hf + 1) * HB, :],
                    km_ps[:].rearrange("p (b e) -> p b e", e=E),
                )
                if hf == 0:
                    nc.vector.tensor_copy(idk16[:], idq_t[0:64, 0:64])

            def tr_half(hf):
                # ----- stage 3: pair transposes -> Q_e, K_e -----
                # in [2m, (b0-e|b1-e)] -> out [(b0-e|b1-e), 2m]; even b on
                # partitions 0:64, odd on 64:128.
                qm_p = qm_h[hf][:].rearrange("p (bp c) -> p bp c", c=128)
                km_p = km_h[hf][:].rearrange("p (bp c) -> p bp c", c=128)
                for g2 in range(4):
                    tp = tp_ps.tile([128, 2, 128], F32, tag="tp", name=f"tp{hf}_{g2}")
                    tk = tp_ps.tile([128, 2, 128], F32, tag="tp", name=f"tk{hf}_{g2}")
                    for j in range(2):
                        bpl = g2 * 2 + j
                        nc.tensor.transpose(tp[:, j, :], qm_p[:, bpl, :], idq_t[:])
                        nc.tensor.transpose(tk[:, j, :], km_p[:, bpl, :], idq_t[:])
                    tpv = tp[:].rearrange("p j (ri y) -> p ri j y", ri=2)
                    tkv = tk[:].rearrange("p j (ri y) -> p ri j y", ri=2)
                    if g2 % 2 == 0:
                        nc.scalar.copy(qe_h[hf][:, :, g2 * 2:(g2 + 1) * 2, :], tpv)
                        nc.scalar.copy(ke_h[hf][:, :, g2 * 2:(g2 + 1) * 2, :], tkv)
                    else:
                        nc.vector.tensor_copy(qe_h[hf][:, :, g2 * 2:(g2 + 1) * 2, :], tpv)
                        nc.vector.tensor_copy(ke_h[hf][:, :, g2 * 2:(g2 + 1) * 2, :], tkv)
                nc.vector.tensor_scalar_mul(qf_h[hf][:, 0, :, :], qe_h[hf][:, 1, :, :], -1.0)
                nc.vector.tensor_copy(qf_h[hf][:, 1, :, :], qe_h[hf][:, 0, :, :])

            def attn1_half(hf):
                # ----- stage 4: attn1 -> X^T psum, A/B fp32 sbuf -----
                # 2 same-parity b per matmul pair (256 cols each); useful
                # quadrants j == j'; partition-aligned extraction.
                for par in range(2):
                    base = 64 * par
                    sl = slice(base, base + 64)
                    for g4 in range(4):
                        pt = at_ps.tile([128, 2, 2, 64], F32, tag="pt", bufs=2,
                                        name=f"pt{hf}_{par}_{g4}")
                        psl = slice(2 * g4, 2 * g4 + 2)
                        nc.tensor.matmul(pt[:], ke_h[hf][sl, 0, psl, :],
                                         qe_h[hf][sl, :, psl, :],
                                         start=True, stop=False)
                        nc.tensor.matmul(pt[:], ke_h[hf][sl, 1, psl, :],
                                         qf_h[hf][sl, :, psl, :],
                                         start=False, stop=True)
                        cg = 8 * hf + 2 * g4 + par
                        if (par + g4) % 2 == 0:
                            nc.scalar.copy(ab_t[0:64, cg, :].rearrange("p (ri y) -> p ri y", ri=2), pt[0:64, :, 0, :])
                            nc.vector.tensor_copy(ab_t[64:128, cg, :].rearrange("p (ri y) -> p ri y", ri=2), pt[64:128, :, 1, :])
                        else:
                            nc.vector.tensor_copy(ab_t[0:64, cg, :].rearrange("p (ri y) -> p ri y", ri=2), pt[0:64, :, 0, :])
                            nc.scalar.copy(ab_t[64:128, cg, :].rearrange("p (ri y) -> p ri y", ri=2), pt[64:128, :, 1, :])

            # Emission order = PE dependency-ready order (PE queues are
            # in-order; a stalled head blocks everything behind it).
            dft_half(0)
            tr_half(0)
            attn1_half(0)
            tanh_half(0)
            dft_half(1)
            attn2_half(0, "act")     # fills the DFT(1)->tr(1) psum-copy gap
            tr_half(1)
            attn1_half(1)
            tanh_half(1)
            stage789_half(0, "act", "mix")   # fills tanh(1)'s PE-idle window
            attn2_half(1, "mix")
            stage789_half(1, "mix", "mix")

    nc.compile()
    return nc


_NC_CACHE = None


def _get_nc():
    global _NC_CACHE
    if _NC_CACHE is None:
        _NC_CACHE = build()
    return _NC_CACHE


def _host_prep(q, k, Wr, Wi):
    """Build the 8 per-core input maps (numpy relayout/cast only)."""
    l = np.arange(L, dtype=np.float64)[:, None]
    m = np.arange(M, dtype=np.float64)[None, :]
    ang = 2.0 * np.pi * l * m / L
    F = np.concatenate([np.cos(ang), -np.sin(ang)], axis=1).astype(np.float32)  # [L, 2M]
    fh = F.astype(np.float16)
    fl = (F - fh.astype(np.float32)).astype(np.float16)
    # fh/fl as [p][(c, 2m)]
    fh = fh.reshape(NCHUNK, 128, 2 * M).transpose(1, 0, 2).reshape(128, 1024)
    fl = fl.reshape(NCHUNK, 128, 2 * M).transpose(1, 0, 2).reshape(128, 1024)

    cm = np.full(M, 2.0); cm[0] = 1.0
    ang2 = 2.0 * np.pi * m.T * np.arange(L, dtype=np.float64)[None, :] / L
    SC = 2.0 ** GSHIFT / (L * 512.0 * 512.0)
    g = np.concatenate([
        cm[:, None] * np.cos(ang2) * SC,
        -cm[:, None] * np.sin(ang2) * SC,
    ], axis=0).astype(np.float32).astype(np.float16)  # [2M, L]

    cpack = np.concatenate([fh, fl, g.astype(np.float16)], axis=1)  # [128, 3072]

    idq = np.eye(128, dtype=np.float32)

    maps = []
    for h in range(H):
        def split(x):
            xs = np.ascontiguousarray(x[:, :, h, :].transpose(1, 0, 2)).reshape(L, B * E)
            hi = xs.astype(np.float16)
            lo = (xs - hi.astype(np.float32)).astype(np.float16)
            return hi, lo
        qh, ql = split(q)
        kh, kl = split(k)
        # pack [c][hf][p][t][col]
        xp = np.empty((NCHUNK, NHALF, 128, 4, HB * E), np.float16)
        for t, src in enumerate((qh, ql, kh, kl)):
            s = src.reshape(NCHUNK, 128, NHALF, HB * E)
            xp[:, :, :, t, :] = s.transpose(0, 2, 1, 3)
        wpk = np.empty((E, 2, M, O), np.float32)
        wpk[:, 0] = (Wr[h] * 2.0 ** WSHIFT).transpose(0, 2, 1)  # [e,o,x]->[e,x,o]
        wpk[:, 1] = (Wi[h] * 2.0 ** WSHIFT).transpose(0, 2, 1)
        maps.append({
            "xp": xp,
            "cp": cpack,
            "wp": wpk.astype(np.float16),
            "idq": idq,
        })
    return maps


def kernel(q, k, v, Wr, Wi, _trace=False):
    q = np.asarray(q, np.float32)
    k = np.asarray(k, np.float32)
    Wr = np.asarray(Wr, np.float32)
    Wi = np.asarray(Wi, np.float32)
    nc = _get_nc()
    maps = _host_prep(q, k, Wr, Wi)
    try:
        res = run_bass_kernel_spmd(nc, maps, core_ids=list(range(H)), trace=_trace)
    except ModuleNotFoundError:
        res = run_bass_kernel_spmd(nc, maps, core_ids=list(range(H)), trace=False)
    # out_d[bp][p][l]: b = 2*bp + (p//64), o = p%64 -> plain b order
    out = np.empty((B, H, O, L), np.float32)
    for h in range(H):
        o = np.asarray(res.results[h]["out"], np.float32).reshape(B, O, L)
        o *= np.float32(OUT_SCALE)
        out[:, h] = o
    if _trace:
        kernel.last_results = res
    return out.astype(np.float32)


# revision 27
# speedup vs baseline: 1.2854x; 1.0235x over previous
"""FEDformer FourierCrossAttention kernel for 8 TRN2 NeuronCores.

Sharding: one head per core (H=8 == n_cores). Each core computes, for its head:
  Q = rfft(q)[:64 modes], K = rfft(k)[:64]      (DFT-as-matmul, hi/lo fp16 3-pass)
  X^T = K^T Q (complex, contract E)             (2-batch 256-col matmuls)
  T = tanh(X) (complex, tau/sin/cos form)       (ACT tanh+sin, DVE cody-waite RR)
  Y = sum_y T[x,y] K[e,y]                       (fp16 matmuls)
  Z = sum_e W[e,o,x] Y[e,x]   (W scaled 2^16)   (dual-accumulate Wr/Wi fp16 matmuls)
  out = irfft(Z / (512*512))  (G scaled 2^24)   (fp16 matmuls; 2^-40 applied on host)

The whole pipeline is split per batch-half (hf): half 0's attn/tanh/output
stages overlap half 1's DMA-paced DFT, and half 0's weight/irfft stages fill
the PE-idle window of half 1's tanh chain.

Batch indexing: global b = 16*hf + 4*g4 + 2*j + par, stored in the attn/tanh
stages at partition half j (pair LSB) and column group cg = 8*hf + 2*g4 + par.
Y/Z/out stages use plain global b ordering.
"""
import numpy as np

import concourse.bass as bass
import concourse.tile as tile
from concourse import bacc, mybir
from concourse.bass_utils import run_bass_kernel_spmd

F32 = mybir.dt.float32
F16 = mybir.dt.float16
F32R = mybir.dt.float32r
AF = mybir.ActivationFunctionType
OP = mybir.AluOpType

B, L, H, E, O, M = 32, 1024, 8, 64, 64, 64
NCHUNK = 8          # contraction chunks of 128 over L
NHALF = 2           # batch halves of 16 for DFT PSUM
WSHIFT = 16         # W scaled by 2^WSHIFT on host
GSHIFT = 24         # G scaled by 2^GSHIFT on host
OUT_SCALE = 2.0 ** (-WSHIFT - GSHIFT)
HB = B // NHALF     # 16 batches per half

PI = np.float64(np.pi)
PI_HI = np.float32(3.140625)
PI_MID = np.float32(PI - np.float64(np.float32(3.140625)))
PI_LO = np.float32(PI - np.float64(np.float32(3.140625)) - np.float64(PI_MID))
MAGIC = np.float32(1.5 * 2 ** 23)   # round-to-nearest via add/sub
RH_LIM = np.nextafter(np.float32(np.pi) - np.float32(np.pi / 2), np.float32(0))


def build(debug=False):
    nc = bacc.Bacc("TRN2", target_bir_lowering=False, debug=False, num_devices=8)

    # ---- I/O (per-core, host pre-sharded/relaid) ----
    # packed q/k hi+lo fp16: [c][hf][p][t][col]; t in {qh, ql, kh, kl},
    # col = b_local*64 + e  (b_local = b - 16*hf)
    x_d = nc.dram_tensor("xp", (NCHUNK, NHALF, 128, 4, HB * E), F16,
                         kind="ExternalInput")
    # packed fp16 consts: [p][fh(8*128) | fl(8*128) | g(1024)]
    c_d = nc.dram_tensor("cp", (128, 3 * 1024), F16, kind="ExternalInput")
    # W packed fp16 (x2^16): [e][ri][x][o] = W{ri}[e, o, x]
    w_d = nc.dram_tensor("wp", (E, 2, M, O), F16, kind="ExternalInput")
    # transpose helper fp32
    idq_d = nc.dram_tensor("idq", (128, 128), F32, kind="ExternalInput")

    # out[bp][p][l]: p = (pair half)*64 + o; global b = 2*bp + (p>=64)
    out_d = nc.dram_tensor("out", (B // 2, 128, L), F16, kind="ExternalOutput")

    with tile.TileContext(nc) as tc:
        from contextlib import ExitStack
        stack = ExitStack()
        with stack:
            consts = stack.enter_context(tc.tile_pool(name="consts", bufs=1))
            chunks = stack.enter_context(tc.tile_pool(name="chunks", bufs=4))
            coeff = stack.enter_context(tc.tile_pool(name="coeff", bufs=1))
            work = stack.enter_context(tc.tile_pool(name="work", bufs=1))
            tmp = stack.enter_context(tc.tile_pool(name="tmp", bufs=1))
            outs = stack.enter_context(tc.tile_pool(name="outs", bufs=4))
            dft_ps = stack.enter_context(tc.tile_pool(name="dft_ps", bufs=1, space="PSUM"))
            tp_ps = stack.enter_context(tc.tile_pool(name="tp_ps", bufs=2, space="PSUM"))
            at_ps = stack.enter_context(tc.tile_pool(name="at_ps", bufs=2, space="PSUM"))

            # ---------- constants ----------
            c_t = consts.tile([128, 3 * 1024], F16, tag="cp")
            w_t = consts.tile([E, 2, M, O], F16, tag="w")
            idq_t = consts.tile([128, 128], F32, tag="idq")
            # pre-load ACT function set 18 (covers tanh+sin+square+copy):
            # the auto-placer is first-fit (tanh->set0, sin->set9) and would
            # otherwise thrash 1.3us table loads on every tanh<->sin switch.
            _ld = mybir.InstLoadActFuncSet(name=nc.get_next_instruction_name(), ins=[], outs=[])
            _ld.act_func_set_id = 18
            nc.scalar.add_instruction(_ld)
            nc.scalar.dma_start(out=c_t, in_=c_d[:])
            nc.scalar.dma_start(out=w_t, in_=w_d[:])
            nc.scalar.dma_start(out=idq_t, in_=idq_d[:])
            fh_t = c_t[:, 0:1024].rearrange("p (c m) -> p c m", m=2 * M)
            fl_t = c_t[:, 1024:2048].rearrange("p (c m) -> p c m", m=2 * M)
            g_t = c_t[:, 2048:3072]

            # ---------- persistent state ----------
            qm_h = [coeff.tile([128, 1024], F32, tag=f"qmh{hf}", name=f"qm_h{hf}")
                    for hf in range(NHALF)]
            km_h = [coeff.tile([128, 1024], F32, tag=f"kmh{hf}", name=f"km_h{hf}")
                    for hf in range(NHALF)]
            km16_t = coeff.tile([128, B, E], F16, tag="km16")
            # layout [p=(par,e), ri, bp, y] so stationary attn1 slices merge
            # into a single contiguous free dim (BIR requirement)
            qe_h = [work.tile([128, 2, 8, 64], F32, tag=f"qeh{hf}", name=f"qe_h{hf}") for hf in range(NHALF)]
            ke_h = [work.tile([128, 2, 8, 64], F32, tag=f"keh{hf}", name=f"ke_h{hf}") for hf in range(NHALF)]
            qf_h = [work.tile([128, 2, 8, 64], F32, tag=f"qfh{hf}", name=f"qf_h{hf}") for hf in range(NHALF)]
            # A/B packed: ab[p = 64*j + y, cg, 0:64 = Re X^T, 64:128 = Im X^T]
            ab_t = work.tile([128, 16, 128], F32, tag="ab")
            halfpi = consts.tile([128, 1], F32, tag="halfpi", name="halfpi")
            nc.vector.memset(halfpi[:], float(np.pi / 2))
            t_t = work.tile([128, 16, 128], F16, tag="t")
            tf_t = work.tile([128, 16, 128], F16, tag="tf")
            tt_t = work.tile([128, B, 128], F16, tag="tt")
            tt_v = tt_t[:].rearrange("p (hg j par) c -> p hg j par c", j=2, par=2)
            y_t = work.tile([E, B, 2, M], F16, tag="y")
            yf_t = work.tile([E, B, 2, M], F16, tag="yf")
            z_t = work.tile([O, B, 2, M], F16, tag="z")
            zp_g = [work.tile([128, 8, O], F16, tag=f"zp{g}", name=f"zp_g{g}")
                    for g in range(B // 8)]
            idk16 = consts.tile([64, 64], F16, tag="id16")

            # ---------- stage 5+6, per half: complex tanh + attn2 ----------
            def tanh_half(hf):
                cgs = slice(8 * hf, 8 * hf + 8)
                # A=Re X^T, B=Im X^T, strided views of ab_t [128, 8, 64]
                av = ab_t[:, cgs, 0:64]
                bv = ab_t[:, cgs, 64:128]
                def ctt(n):
                    return tmp.tile([128, 512], F32, tag="ct", name=f"ct_{n}{hf}", bufs=6)
                def v2(t):
                    return t[:].rearrange("p (g m) -> p g m", m=64)
                ct_n = ctt("n")
                nc.vector.tensor_scalar(v2(ct_n), bv, float(1.0 / PI), float(MAGIC), OP.mult, OP.add)
                nc.vector.tensor_scalar_sub(ct_n[:], ct_n[:], float(MAGIC))
                ct_rh = ctt("rh")
                nc.vector.cody_waite_cascade(v2(ct_rh), bv, ct_n[:], float(PI_HI), float(PI_MID), float(PI_LO))
                # clamp |rh| so rh+pi/2 (cos) and 2*rh (sin) stay in [-pi, pi]
                nc.vector.tensor_scalar(ct_rh[:], ct_rh[:], -float(RH_LIM), float(RH_LIM), OP.max, OP.min)
                ct_tau = ctt("tau")
                nc.scalar.activation(v2(ct_tau), av, AF.Tanh)
                ct_s = ctt("s")
                nc.scalar.activation(ct_s[:], ct_rh[:], AF.Sin)
                ct_c = ctt("c")
                nc.scalar.activation(ct_c[:], ct_rh[:], AF.Sin, bias=halfpi[:])
                # squares on DVE (keeps ACT on the Tanh/Sin table set: a table
                # reload costs 1.3us and lands on the critical chain)
                ct_s2 = ctt("s2")
                nc.vector.tensor_mul(ct_s2[:], ct_s[:], ct_s[:])
                ct_c2 = ctt("c2")
                nc.vector.tensor_mul(ct_c2[:], ct_c[:], ct_c[:])
                ct_sc = ctt("sc")
                nc.vector.tensor_mul(ct_sc[:], ct_s[:], ct_c[:])
                ct_t2 = ctt("t2")
                nc.vector.tensor_mul(ct_t2[:], ct_tau[:], ct_tau[:])
                ct_d = ctt("d")
                nc.vector.tensor_mul(ct_d[:], ct_t2[:], ct_s2[:])
                nc.vector.tensor_add(ct_d[:], ct_d[:], ct_c2[:])
                ct_r = ctt("r")
                nc.vector.reciprocal(ct_r[:], ct_d[:])
                nc.vector.tensor_scalar(ct_t2[:], ct_t2[:], -1.0, 1.0, OP.mult, OP.add)
                ct_u = ctt("u")
                nc.vector.tensor_mul(ct_u[:], ct_sc[:], ct_t2[:])
                # T = [Tr | Ti] fp16 ; Tf = [-Ti | Tr]   (same (j, cg) layout)
                nc.vector.tensor_mul(t_t[:, cgs, 0:64], v2(ct_tau), v2(ct_r))
                nc.vector.tensor_mul(t_t[:, cgs, 64:128], v2(ct_u), v2(ct_r))
                nc.vector.tensor_scalar_mul(tf_t[:, cgs, 0:64], t_t[:, cgs, 64:128], -1.0)
                nc.vector.tensor_copy(tf_t[:, cgs, 64:128], t_t[:, cgs, 0:64])

                # TT assembly for this half (feeds attn2, emitted later):
                # global b = 16hf + 4g4 + 2j + par lives at t[64j:64j+64, cg],
                # cg = 8hf + 2g4 + par; b factors as (hg=(hf,g4), j, par).
                # Parity-matched halves via DVE, others via SWDGE SBUF DMAs.
                hgs = slice(4 * hf, 4 * hf + 4)
                def cg_view(t):
                    return t.rearrange("p (hg par) c -> p hg par c", par=2)
                nc.vector.tensor_copy(tt_v[0:64, hgs, 0, :, :], cg_view(t_t[0:64, cgs, :]))
                nc.vector.tensor_copy(tt_v[64:128, hgs, 1, :, :], cg_view(tf_t[64:128, cgs, :]))
                nc.gpsimd.dma_start(out=tt_v[0:64, hgs, 1, :, :], in_=cg_view(t_t[64:128, cgs, :]))
                nc.gpsimd.dma_start(out=tt_v[64:128, hgs, 0, :, :], in_=cg_view(tf_t[0:64, cgs, :]))

            # ---------- attn2 (PE side), per half ----------
            # emitted in dependency-ready order: PE queues are in-order, so a
            # matmul waiting on the tanh chain must not be emitted before PE
            # work whose inputs are already available.
            def attn2_half(hf, copy_eng):
                # Y in plain global-b order
                for b4 in range(4 * hf, 4 * hf + 4):
                    yp = at_ps.tile([E, 4, 128], F32, tag="pt", bufs=2, name=f"yp{b4}")
                    for j in range(4):
                        b = b4 * 4 + j
                        nc.tensor.matmul(yp[:, j, :], km16_t[:, b, :], tt_t[:, b, :],
                                         start=True, stop=True)
                    dst = y_t[:, b4 * 4:(b4 + 1) * 4, :, :]
                    srcv = yp[:].rearrange("p b (ri m) -> p b ri m", m=M)
                    if copy_eng == "act" or b4 % 2 == 0:
                        nc.scalar.copy(dst, srcv)
                    else:
                        nc.vector.tensor_copy(dst, srcv)
                # Yf = [-Yi | Yr] for the dual-accumulate weight stage
                hb = slice(16 * hf, 16 * hf + 16)
                if copy_eng == "act":
                    nc.scalar.mul(yf_t[:, hb, 0, :], y_t[:, hb, 1, :], -1.0)
                    nc.scalar.copy(yf_t[:, hb, 1, :], y_t[:, hb, 0, :])
                else:
                    nc.vector.tensor_scalar_mul(yf_t[:, hb, 0, :], y_t[:, hb, 1, :], -1.0)
                    nc.vector.tensor_copy(yf_t[:, hb, 1, :], y_t[:, hb, 0, :])

            # ---------- stages 7-9, per half ----------
            # Zr = Wr^T Yr - Wi^T Yi ; Zi = Wr^T Yi + Wi^T Yr, via two
            # accumulating matmuls: Wr^T @ [Yr|Yi] + Wi^T @ [-Yi|Yr].
            # Then Z transposes -> Z' [(ri,x), (b, o)] and irfft out = Z'^T G.
            # PSUM comes from the transpose tag (free once transposes done).
            def stage789_half(hf, z_eng, out_eng):
                b0 = 16 * hf
                for x8 in range(M // 8):
                    wp = tp_ps.tile([O, 8, HB * 2], F32, tag="tp", bufs=2,
                                    name=f"wp{hf}_{x8}")
                    for j in range(8):
                        x = x8 * 8 + j
                        yv = y_t[:, b0:b0 + HB, :, x].rearrange("p b ri -> p (b ri)")
                        yfv = yf_t[:, b0:b0 + HB, :, x].rearrange("p b ri -> p (b ri)")
                        nc.tensor.matmul(wp[:, j, :], w_t[:, 0, x, :], yv,
                                         start=True, stop=False)
                        nc.tensor.matmul(wp[:, j, :], w_t[:, 1, x, :], yfv,
                                         start=False, stop=True)
                    dst = z_t[:, b0:b0 + HB, :, x8 * 8:(x8 + 1) * 8].rearrange("p b ri x -> p x b ri")
                    srcv = wp[:].rearrange("p x (b ri) -> p x b ri", ri=2)
                    if z_eng == "act" or x8 % 2 == 0:
                        nc.scalar.copy(dst, srcv)
                    else:
                        nc.vector.tensor_copy(dst, srcv)

                for b8 in range(2 * hf, 2 * hf + 2):
                    zt = tp_ps.tile([128, 8, O], F16, tag="tp", bufs=2,
                                    name=f"zt{b8}")
                    for j in range(8):
                        b = b8 * 8 + j
                        nc.tensor.transpose(
                            zt[:, j, :],
                            z_t[:, b, :, :].rearrange("p ri m -> p (ri m)"),
                            idk16[:],
                        )
                    if z_eng == "act" or b8 % 2 == 1:
                        nc.scalar.copy(zp_g[b8][:], zt[:])
                    else:
                        nc.vector.tensor_copy(zp_g[b8][:], zt[:])

                # irfft + staged fp16 output (host applies OUT_SCALE;
                # fp16 can't hold out*2^-40 without underflow)
                for bp in range(8 * hf, 8 * hf + 8):
                    otg = outs.tile([128, 1024], F16, tag="ot", name=f"ot{bp}")
                    for g in range(2):
                        opg = tp_ps.tile([128, 512], F32, tag="tp", bufs=2,
                                         name=f"op{bp}_{g}")
                        nc.tensor.matmul(
                            opg[:, :],
                            zp_g[bp // 4][:, (bp % 4) * 2:(bp % 4) * 2 + 2, :]
                            .rearrange("p b o -> p (b o)"),
                            g_t[:, g * 512:(g + 1) * 512],
                            start=True, stop=True,
                        )
                        if out_eng == "act" or (bp + g) % 2 == 0:
                            nc.scalar.copy(otg[:, g * 512:(g + 1) * 512], opg[:])
                        else:
                            nc.vector.tensor_copy(otg[:, g * 512:(g + 1) * 512], opg[:])
                    nc.sync.dma_start(out=out_d[bp], in_=otg[:])

            # ---------- main per-half pipeline ----------
            def dft_half(hf):
                # ----- stage 1+2: DFT (hi/lo 3-pass) -----
                qm_ps = dft_ps.tile([128, 1024], F32, tag="qmps", name=f"qm_ps{hf}", bufs=1)
                km_ps = dft_ps.tile([128, 1024], F32, tag="kmps", name=f"km_ps{hf}", bufs=1)
                for c in range(NCHUNK):
                    x_c = chunks.tile([128, 4, HB * E], F16, tag="x", name=f"x{hf}_{c}")
                    nc.sync.dma_start(out=x_c, in_=x_d[c, hf])
                    first = c == 0
                    last = c == NCHUNK - 1
                    passes = (
                        (fh_t[:, c, :], 0, qm_ps, first, False),
                        (fh_t[:, c, :], 1, qm_ps, False, False),
                        (fh_t[:, c, :], 2, km_ps, first, False),
                        (fh_t[:, c, :], 3, km_ps, False, False),
                        (fl_t[:, c, :], 0, qm_ps, False, last),
                        (fl_t[:, c, :], 2, km_ps, False, last),
                    )
                    for lhs, ti, ps, is_start, is_stop in passes:
                        for g in range(2):
                            nc.tensor.matmul(
                                ps[:, g * 512:(g + 1) * 512],
                                lhs,
                                x_c[:, ti, g * 512:(g + 1) * 512],
                                start=is_start,
                                stop=is_stop,
                            )
                nc.vector.tensor_copy(qm_h[hf][:], qm_ps[:])
                nc.scalar.copy(km_h[hf][:], km_ps[:])
                nc.vector.tensor_copy(
                    km16_t[:, hf * HB:(hf + 1) * HB, :],
                    km_ps[:].rearrange("p (b e) -> p b e", e=E),
                )
                if hf == 0:
                    nc.vector.tensor_copy(idk16[:], idq_t[0:64, 0:64])

            def tr_half(hf):
                # ----- stage 3: pair transposes -> Q_e, K_e -----
                # in [2m, (b0-e|b1-e)] -> out [(b0-e|b1-e), 2m]; even b on
                # partitions 0:64, odd on 64:128.
                qm_p = qm_h[hf][:].rearrange("p (bp c) -> p bp c", c=128)
                km_p = km_h[hf][:].rearrange("p (bp c) -> p bp c", c=128)
                for g2 in range(4):
                    tp = tp_ps.tile([128, 2, 128], F32, tag="tp", name=f"tp{hf}_{g2}")
                    tk = tp_ps.tile([128, 2, 128], F32, tag="tp", name=f"tk{hf}_{g2}")
                    for j in range(2):
                        bpl = g2 * 2 + j
                        nc.tensor.transpose(tp[:, j, :], qm_p[:, bpl, :], idq_t[:])
                        nc.tensor.transpose(tk[:, j, :], km_p[:, bpl, :], idq_t[:])
                    tpv = tp[:].rearrange("p j (ri y) -> p ri j y", ri=2)
                    tkv = tk[:].rearrange("p j (ri y) -> p ri j y", ri=2)
                    if g2 % 2 == 0:
                        nc.scalar.copy(qe_h[hf][:, :, g2 * 2:(g2 + 1) * 2, :], tpv)
                        nc.scalar.copy(ke_h[hf][:, :, g2 * 2:(g2 + 1) * 2, :], tkv)
                    else:
                        nc.vector.tensor_copy(qe_h[hf][:, :, g2 * 2:(g2 + 1) * 2, :], tpv)
                        nc.vector.tensor_copy(ke_h[hf][:, :, g2 * 2:(g2 + 1) * 2, :], tkv)
                nc.vector.tensor_scalar_mul(qf_h[hf][:, 0, :, :], qe_h[hf][:, 1, :, :], -1.0)
                nc.vector.tensor_copy(qf_h[hf][:, 1, :, :], qe_h[hf][:, 0, :, :])

            def attn1_half(hf):
                # ----- stage 4: attn1 -> X^T psum, A/B fp32 sbuf -----
                # 2 same-parity b per matmul pair (256 cols each); useful
                # quadrants j == j'; partition-aligned extraction.
                for par in range(2):
                    base = 64 * par
                    sl = slice(base, base + 64)
                    for g4 in range(4):
                        pt = at_ps.tile([128, 2, 2, 64], F32, tag="pt", bufs=2,
                                        name=f"pt{hf}_{par}_{g4}")
                        psl = slice(2 * g4, 2 * g4 + 2)
                        nc.tensor.matmul(pt[:], ke_h[hf][sl, 0, psl, :],
                                         qe_h[hf][sl, :, psl, :],
                                         start=True, stop=False)
                        nc.tensor.matmul(pt[:], ke_h[hf][sl, 1, psl, :],
                                         qf_h[hf][sl, :, psl, :],
                                         start=False, stop=True)
                        cg = 8 * hf + 2 * g4 + par
                        if (par + g4) % 2 == 0:
                            nc.scalar.copy(ab_t[0:64, cg, :].rearrange("p (ri y) -> p ri y", ri=2), pt[0:64, :, 0, :])
                            nc.vector.tensor_copy(ab_t[64:128, cg, :].rearrange("p (ri y) -> p ri y", ri=2), pt[64:128, :, 1, :])
                        else:
                            nc.vector.tensor_copy(ab_t[0:64, cg, :].rearrange("p (ri y) -> p ri y", ri=2), pt[0:64, :, 0, :])
                            nc.scalar.copy(ab_t[64:128, cg, :].rearrange("p (ri y) -> p ri y", ri=2), pt[64:128, :, 1, :])

            # Emission order = PE dependency-ready order (PE queues are
            # in-order; a stalled head blocks everything behind it).
            dft_half(0)
            tr_half(0)
            attn1_half(0)
            tanh_half(0)
            dft_half(1)
            attn2_half(0, "act")     # fills the DFT(1)->tr(1) psum-copy gap
            tr_half(1)
            attn1_half(1)
            tanh_half(1)
            stage789_half(0, "act", "mix")   # fills tanh(1)'s PE-idle window
            attn2_half(1, "mix")
            stage789_half(1, "mix", "mix")

    nc.compile()
    return nc


_NC_CACHE = None


def _get_nc():
    global _NC_CACHE
    if _NC_CACHE is None:
        _NC_CACHE = build()
    return _NC_CACHE


def _host_prep(q, k, Wr, Wi):
    """Build the 8 per-core input maps (numpy relayout/cast only)."""
    l = np.arange(L, dtype=np.float64)[:, None]
    m = np.arange(M, dtype=np.float64)[None, :]
    ang = 2.0 * np.pi * l * m / L
    F = np.concatenate([np.cos(ang), -np.sin(ang)], axis=1).astype(np.float32)  # [L, 2M]
    fh = F.astype(np.float16)
    fl = (F - fh.astype(np.float32)).astype(np.float16)
    # fh/fl as [p][(c, 2m)]
    fh = fh.reshape(NCHUNK, 128, 2 * M).transpose(1, 0, 2).reshape(128, 1024)
    fl = fl.reshape(NCHUNK, 128, 2 * M).transpose(1, 0, 2).reshape(128, 1024)

    cm = np.full(M, 2.0); cm[0] = 1.0
    ang2 = 2.0 * np.pi * m.T * np.arange(L, dtype=np.float64)[None, :] / L
    SC = 2.0 ** GSHIFT / (L * 512.0 * 512.0)
    g = np.concatenate([
        cm[:, None] * np.cos(ang2) * SC,
        -cm[:, None] * np.sin(ang2) * SC,
    ], axis=0).astype(np.float32).astype(np.float16)  # [2M, L]

    cpack = np.concatenate([fh, fl, g.astype(np.float16)], axis=1)  # [128, 3072]

    idq = np.eye(128, dtype=np.float32)

    maps = []
    for h in range(H):
        def split(x):
            xs = np.ascontiguousarray(x[:, :, h, :].transpose(1, 0, 2)).reshape(L, B * E)
            hi = xs.astype(np.float16)
            lo = (xs - hi.astype(np.float32)).astype(np.float16)
            return hi, lo
        qh, ql = split(q)
        kh, kl = split(k)
        # pack [c][hf][p][t][col]
        xp = np.empty((NCHUNK, NHALF, 128, 4, HB * E), np.float16)
        for t, src in enumerate((qh, ql, kh, kl)):
            s = src.reshape(NCHUNK, 128, NHALF, HB * E)
            xp[:, :, :, t, :] = s.transpose(0, 2, 1, 3)
        wpk = np.empty((E, 2, M, O), np.float32)
        wpk[:, 0] = (Wr[h] * 2.0 ** WSHIFT).transpose(0, 2, 1)  # [e,o,x]->[e,x,o]
        wpk[:, 1] = (Wi[h] * 2.0 ** WSHIFT).transpose(0, 2, 1)
        maps.append({
            "xp": xp,
            "cp": cpack,
            "wp": wpk.astype(np.float16),
            "idq": idq,
        })
    return maps


def kernel(q, k, v, Wr, Wi, _trace=False):
    q = np.asarray(q, np.float32)
    k = np.asarray(k, np.float32)
    Wr = np.asarray(Wr, np.float32)
    Wi = np.asarray(Wi, np.float32)
    nc = _get_nc()
    maps = _host_prep(q, k, Wr, Wi)
    try:
        res = run_bass_kernel_spmd(nc, maps, core_ids=list(range(H)), trace=_trace)
    except ModuleNotFoundError:
        res = run_bass_kernel_spmd(nc, maps, core_ids=list(range(H)), trace=False)
    # out_d[bp][p][l]: b = 2*bp + (p//64), o = p%64 -> plain b order
    out = np.empty((B, H, O, L), np.float32)
    for h in range(H):
        o = np.asarray(res.results[h]["out"], np.float32).reshape(B, O, L)
        o *= np.float32(OUT_SCALE)
        out[:, h] = o
    if _trace:
        kernel.last_results = res
    return out.astype(np.float32)


# revision 28
# speedup vs baseline: 1.3263x; 1.0318x over previous
"""FEDformer FourierCrossAttention kernel for 8 TRN2 NeuronCores.

Sharding: one head per core (H=8 == n_cores). Each core computes, for its head:
  Q = rfft(q)[:64 modes], K = rfft(k)[:64]      (DFT-as-matmul, hi/lo fp16 3-pass)
  X^T = K^T Q (complex, contract E)             (2-batch 256-col matmuls)
  T = tanh(X) (complex, tau/sin/cos form)       (ACT tanh+sin, DVE cody-waite RR)
  Y = sum_y T[x,y] K[e,y]                       (fp16 matmuls)
  Z = sum_e W[e,o,x] Y[e,x]   (W scaled 2^16)   (dual-accumulate Wr/Wi fp16 matmuls)
  out = irfft(Z / (512*512))  (G scaled 2^24)   (fp16 matmuls; 2^-40 applied on host)

The whole pipeline is split per batch-half (hf): half 0's attn/tanh/output
stages overlap half 1's DMA-paced DFT, and half 0's weight/irfft stages fill
the PE-idle window of half 1's tanh chain.

Batch indexing: global b = 16*hf + 4*g4 + 2*j + par, stored in the attn/tanh
stages at partition half j (pair LSB) and column group cg = 8*hf + 2*g4 + par.
Y/Z/out stages use plain global b ordering.
"""
import numpy as np

import concourse.bass as bass
import concourse.tile as tile
from concourse import bacc, mybir
from concourse.bass_utils import run_bass_kernel_spmd

F32 = mybir.dt.float32
F16 = mybir.dt.float16
F32R = mybir.dt.float32r
F8 = mybir.dt.float8e4
AF = mybir.ActivationFunctionType
OP = mybir.AluOpType

B, L, H, E, O, M = 32, 1024, 8, 64, 64, 64
NCHUNK = 8          # contraction chunks of 128 over L
NHALF = 2           # batch halves of 16 for DFT PSUM
WSHIFT = 16         # W scaled by 2^WSHIFT on host
GSHIFT = 24         # G scaled by 2^GSHIFT on host
OUT_SCALE = 2.0 ** (-WSHIFT - GSHIFT)
HB = B // NHALF     # 16 batches per half

PI = np.float64(np.pi)
PI_HI = np.float32(3.140625)
PI_MID = np.float32(PI - np.float64(np.float32(3.140625)))
PI_LO = np.float32(PI - np.float64(np.float32(3.140625)) - np.float64(PI_MID))
MAGIC = np.float32(1.5 * 2 ** 23)   # round-to-nearest via add/sub
RH_LIM = np.nextafter(np.float32(np.pi) - np.float32(np.pi / 2), np.float32(0))


def build(debug=False):
    nc = bacc.Bacc("TRN2", target_bir_lowering=False, debug=False, num_devices=8)

    # ---- I/O (per-core, host pre-sharded/relaid) ----
    # q/k hi fp16: [c][hf][p][t][col]; t in {qh, kh}, col = b_local*64 + e
    xh_d = nc.dram_tensor("xh", (NCHUNK, NHALF, 128, 2, HB * E), F16,
                          kind="ExternalInput")
    # q/k lo fp8 e4m3, scaled 2^12: t in {ql8, kl8}; the matching DFT matrix
    # is pre-scaled 2^-12 in fp16 (subnormal range is exact enough), so the
    # mixed fp16 x fp8 matmul accumulates into the same PSUM at scale 1.
    xl_d = nc.dram_tensor("xl", (NCHUNK, NHALF, 128, 2, HB * E), F8,
                          kind="ExternalInput")
    # packed fp16 consts: [p][fh(8*128) | fl(8*128) | g(1024) | fh12(8*128)]
    c_d = nc.dram_tensor("cp", (128, 4 * 1024), F16, kind="ExternalInput")
    # W packed fp16 (x2^16): [e][ri][x][o] = W{ri}[e, o, x]
    w_d = nc.dram_tensor("wp", (E, 2, M, O), F16, kind="ExternalInput")
    # transpose helper fp32
    idq_d = nc.dram_tensor("idq", (128, 128), F32, kind="ExternalInput")

    # out[bp][p][l]: p = (pair half)*64 + o; global b = 2*bp + (p>=64)
    out_d = nc.dram_tensor("out", (B // 2, 128, L), F16, kind="ExternalOutput")

    with tile.TileContext(nc) as tc:
        from contextlib import ExitStack
        stack = ExitStack()
        with stack:
            consts = stack.enter_context(tc.tile_pool(name="consts", bufs=1))
            chunks = stack.enter_context(tc.tile_pool(name="chunks", bufs=4))
            coeff = stack.enter_context(tc.tile_pool(name="coeff", bufs=1))
            work = stack.enter_context(tc.tile_pool(name="work", bufs=1))
            tmp = stack.enter_context(tc.tile_pool(name="tmp", bufs=1))
            outs = stack.enter_context(tc.tile_pool(name="outs", bufs=4))
            dft_ps = stack.enter_context(tc.tile_pool(name="dft_ps", bufs=1, space="PSUM"))
            tp_ps = stack.enter_context(tc.tile_pool(name="tp_ps", bufs=2, space="PSUM"))
            at_ps = stack.enter_context(tc.tile_pool(name="at_ps", bufs=2, space="PSUM"))

            # ---------- constants ----------
            c_t = consts.tile([128, 4 * 1024], F16, tag="cp")
            w_t = consts.tile([E, 2, M, O], F16, tag="w")
            idq_t = consts.tile([128, 128], F32, tag="idq")
            # pre-load ACT function set 18 (covers tanh+sin+square+copy):
            # the auto-placer is first-fit (tanh->set0, sin->set9) and would
            # otherwise thrash 1.3us table loads on every tanh<->sin switch.
            _ld = mybir.InstLoadActFuncSet(name=nc.get_next_instruction_name(), ins=[], outs=[])
            _ld.act_func_set_id = 18
            nc.scalar.add_instruction(_ld)
            nc.scalar.dma_start(out=c_t, in_=c_d[:])
            nc.scalar.dma_start(out=w_t, in_=w_d[:])
            nc.scalar.dma_start(out=idq_t, in_=idq_d[:])
            fh_t = c_t[:, 0:1024].rearrange("p (c m) -> p c m", m=2 * M)
            fl_t = c_t[:, 1024:2048].rearrange("p (c m) -> p c m", m=2 * M)
            g_t = c_t[:, 2048:3072]
            f12_t = c_t[:, 3072:4096].rearrange("p (c m) -> p c m", m=2 * M)

            # ---------- persistent state ----------
            qm_h = [coeff.tile([128, 1024], F32, tag=f"qmh{hf}", name=f"qm_h{hf}")
                    for hf in range(NHALF)]
            km_h = [coeff.tile([128, 1024], F32, tag=f"kmh{hf}", name=f"km_h{hf}")
                    for hf in range(NHALF)]
            km16_t = coeff.tile([128, B, E], F16, tag="km16")
            # layout [p=(par,e), ri, bp, y] so stationary attn1 slices merge
            # into a single contiguous free dim (BIR requirement)
            qe_h = [work.tile([128, 2, 8, 64], F32, tag=f"qeh{hf}", name=f"qe_h{hf}") for hf in range(NHALF)]
            ke_h = [work.tile([128, 2, 8, 64], F32, tag=f"keh{hf}", name=f"ke_h{hf}") for hf in range(NHALF)]
            qf_h = [work.tile([128, 2, 8, 64], F32, tag=f"qfh{hf}", name=f"qf_h{hf}") for hf in range(NHALF)]
            # A/B packed: ab[p = 64*j + y, cg, 0:64 = Re X^T, 64:128 = Im X^T]
            ab_t = work.tile([128, 16, 128], F32, tag="ab")
            halfpi = consts.tile([128, 1], F32, tag="halfpi", name="halfpi")
            nc.vector.memset(halfpi[:], float(np.pi / 2))
            t_t = work.tile([128, 16, 128], F16, tag="t")
            tf_t = work.tile([128, 16, 128], F16, tag="tf")
            tt_t = work.tile([128, B, 128], F16, tag="tt")
            tt_v = tt_t[:].rearrange("p (hg j par) c -> p hg j par c", j=2, par=2)
            y_t = work.tile([E, B, 2, M], F16, tag="y")
            yf_t = work.tile([E, B, 2, M], F16, tag="yf")
            z_t = work.tile([O, B, 2, M], F16, tag="z")
            zp_g = [work.tile([128, 8, O], F16, tag=f"zp{g}", name=f"zp_g{g}")
                    for g in range(B // 8)]
            idk16 = consts.tile([64, 64], F16, tag="id16")

            # ---------- stage 5+6, per half: complex tanh + attn2 ----------
            def tanh_half(hf):
                cgs = slice(8 * hf, 8 * hf + 8)
                # A=Re X^T, B=Im X^T, strided views of ab_t [128, 8, 64]
                av = ab_t[:, cgs, 0:64]
                bv = ab_t[:, cgs, 64:128]
                def ctt(n):
                    return tmp.tile([128, 512], F32, tag="ct", name=f"ct_{n}{hf}", bufs=6)
                def v2(t):
                    return t[:].rearrange("p (g m) -> p g m", m=64)
                ct_n = ctt("n")
                nc.vector.tensor_scalar(v2(ct_n), bv, float(1.0 / PI), float(MAGIC), OP.mult, OP.add)
                nc.vector.tensor_scalar_sub(ct_n[:], ct_n[:], float(MAGIC))
                ct_rh = ctt("rh")
                nc.vector.cody_waite_cascade(v2(ct_rh), bv, ct_n[:], float(PI_HI), float(PI_MID), float(PI_LO))
                # clamp |rh| so rh+pi/2 (cos) and 2*rh (sin) stay in [-pi, pi]
                nc.vector.tensor_scalar(ct_rh[:], ct_rh[:], -float(RH_LIM), float(RH_LIM), OP.max, OP.min)
                ct_tau = ctt("tau")
                nc.scalar.activation(v2(ct_tau), av, AF.Tanh)
                ct_s = ctt("s")
                nc.scalar.activation(ct_s[:], ct_rh[:], AF.Sin)
                ct_c = ctt("c")
                nc.scalar.activation(ct_c[:], ct_rh[:], AF.Sin, bias=halfpi[:])
                # squares on DVE (keeps ACT on the Tanh/Sin table set: a table
                # reload costs 1.3us and lands on the critical chain)
                ct_s2 = ctt("s2")
                nc.vector.tensor_mul(ct_s2[:], ct_s[:], ct_s[:])
                ct_c2 = ctt("c2")
                nc.vector.tensor_mul(ct_c2[:], ct_c[:], ct_c[:])
                ct_sc = ctt("sc")
                nc.vector.tensor_mul(ct_sc[:], ct_s[:], ct_c[:])
                ct_t2 = ctt("t2")
                nc.vector.tensor_mul(ct_t2[:], ct_tau[:], ct_tau[:])
                ct_d = ctt("d")
                nc.vector.tensor_mul(ct_d[:], ct_t2[:], ct_s2[:])
                nc.vector.tensor_add(ct_d[:], ct_d[:], ct_c2[:])
                ct_r = ctt("r")
                nc.vector.reciprocal(ct_r[:], ct_d[:])
                nc.vector.tensor_scalar(ct_t2[:], ct_t2[:], -1.0, 1.0, OP.mult, OP.add)
                ct_u = ctt("u")
                nc.vector.tensor_mul(ct_u[:], ct_sc[:], ct_t2[:])
                # T = [Tr | Ti] fp16 ; Tf = [-Ti | Tr]   (same (j, cg) layout)
                nc.vector.tensor_mul(t_t[:, cgs, 0:64], v2(ct_tau), v2(ct_r))
                nc.vector.tensor_mul(t_t[:, cgs, 64:128], v2(ct_u), v2(ct_r))
                nc.vector.tensor_scalar_mul(tf_t[:, cgs, 0:64], t_t[:, cgs, 64:128], -1.0)
                nc.vector.tensor_copy(tf_t[:, cgs, 64:128], t_t[:, cgs, 0:64])

                # TT assembly for this half (feeds attn2, emitted later):
                # global b = 16hf + 4g4 + 2j + par lives at t[64j:64j+64, cg],
                # cg = 8hf + 2g4 + par; b factors as (hg=(hf,g4), j, par).
                # Parity-matched halves via DVE, others via SWDGE SBUF DMAs.
                hgs = slice(4 * hf, 4 * hf + 4)
                def cg_view(t):
                    return t.rearrange("p (hg par) c -> p hg par c", par=2)
                nc.vector.tensor_copy(tt_v[0:64, hgs, 0, :, :], cg_view(t_t[0:64, cgs, :]))
                nc.vector.tensor_copy(tt_v[64:128, hgs, 1, :, :], cg_view(tf_t[64:128, cgs, :]))
                nc.gpsimd.dma_start(out=tt_v[0:64, hgs, 1, :, :], in_=cg_view(t_t[64:128, cgs, :]))
                nc.gpsimd.dma_start(out=tt_v[64:128, hgs, 0, :, :], in_=cg_view(tf_t[0:64, cgs, :]))

            # ---------- attn2 (PE side), per half ----------
            # emitted in dependency-ready order: PE queues are in-order, so a
            # matmul waiting on the tanh chain must not be emitted before PE
            # work whose inputs are already available.
            def attn2_half(hf, copy_eng):
                # Y in plain global-b order
                for b4 in range(4 * hf, 4 * hf + 4):
                    yp = at_ps.tile([E, 4, 128], F32, tag="pt", bufs=2, name=f"yp{b4}")
                    for j in range(4):
                        b = b4 * 4 + j
                        nc.tensor.matmul(yp[:, j, :], km16_t[:, b, :], tt_t[:, b, :],
                                         start=True, stop=True)
                    dst = y_t[:, b4 * 4:(b4 + 1) * 4, :, :]
                    srcv = yp[:].rearrange("p b (ri m) -> p b ri m", m=M)
                    if copy_eng == "act" or b4 % 2 == 0:
                        nc.scalar.copy(dst, srcv)
                    else:
                        nc.vector.tensor_copy(dst, srcv)
                # Yf = [-Yi | Yr] for the dual-accumulate weight stage
                hb = slice(16 * hf, 16 * hf + 16)
                if copy_eng == "act":
                    nc.scalar.mul(yf_t[:, hb, 0, :], y_t[:, hb, 1, :], -1.0)
                    nc.scalar.copy(yf_t[:, hb, 1, :], y_t[:, hb, 0, :])
                else:
                    nc.vector.tensor_scalar_mul(yf_t[:, hb, 0, :], y_t[:, hb, 1, :], -1.0)
                    nc.vector.tensor_copy(yf_t[:, hb, 1, :], y_t[:, hb, 0, :])

            # ---------- stages 7-9, per half ----------
            # Zr = Wr^T Yr - Wi^T Yi ; Zi = Wr^T Yi + Wi^T Yr, via two
            # accumulating matmuls: Wr^T @ [Yr|Yi] + Wi^T @ [-Yi|Yr].
            # Then Z transposes -> Z' [(ri,x), (b, o)] and irfft out = Z'^T G.
            # PSUM comes from the transpose tag (free once transposes done).
            def stage789_half(hf, z_eng, out_eng):
                b0 = 16 * hf
                for x8 in range(M // 8):
                    wp = tp_ps.tile([O, 8, HB * 2], F32, tag="tp", bufs=2,
                                    name=f"wp{hf}_{x8}")
                    for j in range(8):
                        x = x8 * 8 + j
                        yv = y_t[:, b0:b0 + HB, :, x].rearrange("p b ri -> p (b ri)")
                        yfv = yf_t[:, b0:b0 + HB, :, x].rearrange("p b ri -> p (b ri)")
                        nc.tensor.matmul(wp[:, j, :], w_t[:, 0, x, :], yv,
                                         start=True, stop=False)
                        nc.tensor.matmul(wp[:, j, :], w_t[:, 1, x, :], yfv,
                                         start=False, stop=True)
                    dst = z_t[:, b0:b0 + HB, :, x8 * 8:(x8 + 1) * 8].rearrange("p b ri x -> p x b ri")
                    srcv = wp[:].rearrange("p x (b ri) -> p x b ri", ri=2)
                    if z_eng == "act" or x8 % 2 == 0:
                        nc.scalar.copy(dst, srcv)
                    else:
                        nc.vector.tensor_copy(dst, srcv)

                for b8 in range(2 * hf, 2 * hf + 2):
                    zt = tp_ps.tile([128, 8, O], F16, tag="tp", bufs=2,
                                    name=f"zt{b8}")
                    for j in range(8):
                        b = b8 * 8 + j
                        nc.tensor.transpose(
                            zt[:, j, :],
                            z_t[:, b, :, :].rearrange("p ri m -> p (ri m)"),
                            idk16[:],
                        )
                    if z_eng == "act" or b8 % 2 == 1:
                        nc.scalar.copy(zp_g[b8][:], zt[:])
                    else:
                        nc.vector.tensor_copy(zp_g[b8][:], zt[:])

                # irfft + staged fp16 output (host applies OUT_SCALE;
                # fp16 can't hold out*2^-40 without underflow)
                for bp in range(8 * hf, 8 * hf + 8):
                    otg = outs.tile([128, 1024], F16, tag="ot", name=f"ot{bp}")
                    for g in range(2):
                        opg = tp_ps.tile([128, 512], F32, tag="tp", bufs=2,
                                         name=f"op{bp}_{g}")
                        nc.tensor.matmul(
                            opg[:, :],
                            zp_g[bp // 4][:, (bp % 4) * 2:(bp % 4) * 2 + 2, :]
                            .rearrange("p b o -> p (b o)"),
                            g_t[:, g * 512:(g + 1) * 512],
                            start=True, stop=True,
                        )
                        if out_eng == "act" or (bp + g) % 2 == 0:
                            nc.scalar.copy(otg[:, g * 512:(g + 1) * 512], opg[:])
                        else:
                            nc.vector.tensor_copy(otg[:, g * 512:(g + 1) * 512], opg[:])
                    nc.sync.dma_start(out=out_d[bp], in_=otg[:])

            # ---------- main per-half pipeline ----------
            def dft_half(hf):
                # ----- stage 1+2: DFT (hi/lo 3-pass) -----
                qm_ps = dft_ps.tile([128, 1024], F32, tag="qmps", name=f"qm_ps{hf}", bufs=1)
                km_ps = dft_ps.tile([128, 1024], F32, tag="kmps", name=f"km_ps{hf}", bufs=1)
                for c in range(NCHUNK):
                    xh_c = chunks.tile([128, 2, HB * E], F16, tag="xh", name=f"xh{hf}_{c}")
                    xl_c = chunks.tile([128, 2, HB * E], F8, tag="xl", name=f"xl{hf}_{c}")
                    nc.sync.dma_start(out=xh_c, in_=xh_d[c, hf])
                    nc.sync.dma_start(out=xl_c, in_=xl_d[c, hf])
                    first = c == 0
                    last = c == NCHUNK - 1
                    passes = (
                        (fh_t[:, c, :], xh_c, 0, qm_ps, first, False),
                        (fh_t[:, c, :], xh_c, 1, km_ps, first, False),
                        (fl_t[:, c, :], xh_c, 0, qm_ps, False, False),
                        (fl_t[:, c, :], xh_c, 1, km_ps, False, False),
                        (f12_t[:, c, :], xl_c, 0, qm_ps, False, last),
                        (f12_t[:, c, :], xl_c, 1, km_ps, False, last),
                    )
                    for lhs, src, ti, ps, is_start, is_stop in passes:
                        for g in range(2):
                            nc.tensor.matmul(
                                ps[:, g * 512:(g + 1) * 512],
                                lhs,
                                src[:, ti, g * 512:(g + 1) * 512],
                                start=is_start,
                                stop=is_stop,
                            )
                nc.vector.tensor_copy(qm_h[hf][:], qm_ps[:])
                nc.scalar.copy(km_h[hf][:], km_ps[:])
                nc.vector.tensor_copy(
                    km16_t[:, hf * HB:(hf + 1) * HB, :],
                    km_ps[:].rearrange("p (b e) -> p b e", e=E),
                )
                if hf == 0:
                    nc.vector.tensor_copy(idk16[:], idq_t[0:64, 0:64])

            def tr_half(hf):
                # ----- stage 3: pair transposes -> Q_e, K_e -----
                # in [2m, (b0-e|b1-e)] -> out [(b0-e|b1-e), 2m]; even b on
                # partitions 0:64, odd on 64:128.
                qm_p = qm_h[hf][:].rearrange("p (bp c) -> p bp c", c=128)
                km_p = km_h[hf][:].rearrange("p (bp c) -> p bp c", c=128)
                for g2 in range(4):
                    tp = tp_ps.tile([128, 2, 128], F32, tag="tp", name=f"tp{hf}_{g2}")
                    tk = tp_ps.tile([128, 2, 128], F32, tag="tp", name=f"tk{hf}_{g2}")
                    for j in range(2):
                        bpl = g2 * 2 + j
                        nc.tensor.transpose(tp[:, j, :], qm_p[:, bpl, :], idq_t[:])
                        nc.tensor.transpose(tk[:, j, :], km_p[:, bpl, :], idq_t[:])
                    tpv = tp[:].rearrange("p j (ri y) -> p ri j y", ri=2)
                    tkv = tk[:].rearrange("p j (ri y) -> p ri j y", ri=2)
                    if g2 % 2 == 0:
                        nc.scalar.copy(qe_h[hf][:, :, g2 * 2:(g2 + 1) * 2, :], tpv)
                        nc.scalar.copy(ke_h[hf][:, :, g2 * 2:(g2 + 1) * 2, :], tkv)
                    else:
                        nc.vector.tensor_copy(qe_h[hf][:, :, g2 * 2:(g2 + 1) * 2, :], tpv)
                        nc.vector.tensor_copy(ke_h[hf][:, :, g2 * 2:(g2 + 1) * 2, :], tkv)
                nc.vector.tensor_scalar_mul(qf_h[hf][:, 0, :, :], qe_h[hf][:, 1, :, :], -1.0)
                nc.vector.tensor_copy(qf_h[hf][:, 1, :, :], qe_h[hf][:, 0, :, :])

            def attn1_half(hf):
                # ----- stage 4: attn1 -> X^T psum, A/B fp32 sbuf -----
                # 2 same-parity b per matmul pair (256 cols each); useful
                # quadrants j == j'; partition-aligned extraction.
                for par in range(2):
                    base = 64 * par
                    sl = slice(base, base + 64)
                    for g4 in range(4):
                        pt = at_ps.tile([128, 2, 2, 64], F32, tag="pt", bufs=2,
                                        name=f"pt{hf}_{par}_{g4}")
                        psl = slice(2 * g4, 2 * g4 + 2)
                        nc.tensor.matmul(pt[:], ke_h[hf][sl, 0, psl, :],
                                         qe_h[hf][sl, :, psl, :],
                                         start=True, stop=False)
                        nc.tensor.matmul(pt[:], ke_h[hf][sl, 1, psl, :],
                                         qf_h[hf][sl, :, psl, :],
                                         start=False, stop=True)
                        cg = 8 * hf + 2 * g4 + par
                        if (par + g4) % 2 == 0:
                            nc.scalar.copy(ab_t[0:64, cg, :].rearrange("p (ri y) -> p ri y", ri=2), pt[0:64, :, 0, :])
                            nc.vector.tensor_copy(ab_t[64:128, cg, :].rearrange("p (ri y) -> p ri y", ri=2), pt[64:128, :, 1, :])
                        else:
                            nc.vector.tensor_copy(ab_t[0:64, cg, :].rearrange("p (ri y) -> p ri y", ri=2), pt[0:64, :, 0, :])
                            nc.scalar.copy(ab_t[64:128, cg, :].rearrange("p (ri y) -> p ri y", ri=2), pt[64:128, :, 1, :])

            # Emission order = PE dependency-ready order (PE queues are
            # in-order; a stalled head blocks everything behind it).
            dft_half(0)
            tr_half(0)
            attn1_half(0)
            tanh_half(0)
            dft_half(1)
            attn2_half(0, "act")     # fills the DFT(1)->tr(1) psum-copy gap
            tr_half(1)
            attn1_half(1)
            tanh_half(1)
            stage789_half(0, "act", "mix")   # fills tanh(1)'s PE-idle window
            attn2_half(1, "mix")
            stage789_half(1, "mix", "mix")

    nc.compile()
    return nc


_NC_CACHE = None


def _get_nc():
    global _NC_CACHE
    if _NC_CACHE is None:
        _NC_CACHE = build()
    return _NC_CACHE


def _host_prep(q, k, Wr, Wi):
    """Build the 8 per-core input maps (numpy relayout/cast only)."""
    l = np.arange(L, dtype=np.float64)[:, None]
    m = np.arange(M, dtype=np.float64)[None, :]
    ang = 2.0 * np.pi * l * m / L
    F = np.concatenate([np.cos(ang), -np.sin(ang)], axis=1).astype(np.float32)  # [L, 2M]
    fh = F.astype(np.float16)
    fl = (F - fh.astype(np.float32)).astype(np.float16)
    # fh/fl as [p][(c, 2m)]
    fh = fh.reshape(NCHUNK, 128, 2 * M).transpose(1, 0, 2).reshape(128, 1024)
    fl = fl.reshape(NCHUNK, 128, 2 * M).transpose(1, 0, 2).reshape(128, 1024)

    cm = np.full(M, 2.0); cm[0] = 1.0
    ang2 = 2.0 * np.pi * m.T * np.arange(L, dtype=np.float64)[None, :] / L
    SC = 2.0 ** GSHIFT / (L * 512.0 * 512.0)
    g = np.concatenate([
        cm[:, None] * np.cos(ang2) * SC,
        -cm[:, None] * np.sin(ang2) * SC,
    ], axis=0).astype(np.float32).astype(np.float16)  # [2M, L]

    f12 = (F * 2.0 ** -12).astype(np.float16)
    f12 = f12.reshape(NCHUNK, 128, 2 * M).transpose(1, 0, 2).reshape(128, 1024)
    cpack = np.concatenate([fh, fl, g.astype(np.float16), f12], axis=1)  # [128, 4096]

    idq = np.eye(128, dtype=np.float32)

    from ml_dtypes import float8_e4m3fn as E4M3
    maps = []
    for h in range(H):
        def split(x):
            xs = np.ascontiguousarray(x[:, :, h, :].transpose(1, 0, 2)).reshape(L, B * E)
            hi = xs.astype(np.float16)
            lo = ((xs - hi.astype(np.float32)) * 2.0 ** 12).astype(E4M3)
            return hi, lo
        qh, ql8 = split(q)
        kh, kl8 = split(k)
        # pack [c][hf][p][t][col]
        xph = np.empty((NCHUNK, NHALF, 128, 2, HB * E), np.float16)
        xpl = np.empty((NCHUNK, NHALF, 128, 2, HB * E), E4M3)
        for t, (dst, src) in enumerate((( xph, qh), (xph, kh), (xpl, ql8), (xpl, kl8))):
            sv = src.reshape(NCHUNK, 128, NHALF, HB * E)
            dst[:, :, :, t % 2, :] = sv.transpose(0, 2, 1, 3)
        wpk = np.empty((E, 2, M, O), np.float32)
        wpk[:, 0] = (Wr[h] * 2.0 ** WSHIFT).transpose(0, 2, 1)  # [e,o,x]->[e,x,o]
        wpk[:, 1] = (Wi[h] * 2.0 ** WSHIFT).transpose(0, 2, 1)
        maps.append({
            "xh": xph,
            "xl": xpl,
            "cp": cpack,
            "wp": wpk.astype(np.float16),
            "idq": idq,
        })
    return maps


def kernel(q, k, v, Wr, Wi, _trace=False):
    q = np.asarray(q, np.float32)
    k = np.asarray(k, np.float32)
    Wr = np.asarray(Wr, np.float32)
    Wi = np.asarray(Wi, np.float32)
    nc = _get_nc()
    maps = _host_prep(q, k, Wr, Wi)
    try:
        res = run_bass_kernel_spmd(nc, maps, core_ids=list(range(H)), trace=_trace)
    except ModuleNotFoundError:
        res = run_bass_kernel_spmd(nc, maps, core_ids=list(range(H)), trace=False)
    # out_d[bp][p][l]: b = 2*bp + (p//64), o = p%64 -> plain b order
    out = np.empty((B, H, O, L), np.float32)
    for h in range(H):
        o = np.asarray(res.results[h]["out"], np.float32).reshape(B, O, L)
        o *= np.float32(OUT_SCALE)
        out[:, h] = o
    if _trace:
        kernel.last_results = res
    return out.astype(np.float32)


# revision 29
# speedup vs baseline: 1.3530x; 1.0202x over previous
"""FEDformer FourierCrossAttention kernel for 8 TRN2 NeuronCores.

Sharding: one head per core (H=8 == n_cores). Each core computes, for its head:
  Q = rfft(q)[:64 modes], K = rfft(k)[:64]      (DFT-as-matmul, hi/lo fp16 3-pass)
  X^T = K^T Q (complex, contract E)             (2-batch 256-col matmuls)
  T = tanh(X) (complex, tau/sin/cos form)       (ACT tanh+sin, DVE cody-waite RR)
  Y = sum_y T[x,y] K[e,y]                       (fp16 matmuls)
  Z = sum_e W[e,o,x] Y[e,x]   (W scaled 2^16)   (dual-accumulate Wr/Wi fp16 matmuls)
  out = irfft(Z / (512*512))  (G scaled 2^24)   (fp16 matmuls; 2^-40 applied on host)

The whole pipeline is split per batch-half (hf): half 0's attn/tanh/output
stages overlap half 1's DMA-paced DFT, and half 0's weight/irfft stages fill
the PE-idle window of half 1's tanh chain.

Batch indexing: global b = 16*hf + 4*g4 + 2*j + par, stored in the attn/tanh
stages at partition half j (pair LSB) and column group cg = 8*hf + 2*g4 + par.
Y/Z/out stages use plain global b ordering.
"""
import numpy as np

import concourse.bass as bass
import concourse.tile as tile
from concourse import bacc, mybir
from concourse.bass_utils import run_bass_kernel_spmd

F32 = mybir.dt.float32
F16 = mybir.dt.float16
F32R = mybir.dt.float32r
F8 = mybir.dt.float8e4
AF = mybir.ActivationFunctionType
OP = mybir.AluOpType

B, L, H, E, O, M = 32, 1024, 8, 64, 64, 64
NCHUNK = 8          # contraction chunks of 128 over L
NHALF = 2           # batch halves of 16 for DFT PSUM
WSHIFT = 16         # W scaled by 2^WSHIFT on host
GSHIFT = 24         # G scaled by 2^GSHIFT on host
OUT_SCALE = 2.0 ** (-WSHIFT - GSHIFT)
HB = B // NHALF     # 16 batches per half

PI = np.float64(np.pi)
PI_HI = np.float32(3.140625)
PI_MID = np.float32(PI - np.float64(np.float32(3.140625)))
PI_LO = np.float32(PI - np.float64(np.float32(3.140625)) - np.float64(PI_MID))
MAGIC = np.float32(1.5 * 2 ** 23)   # round-to-nearest via add/sub
RH_LIM = np.nextafter(np.float32(np.pi) - np.float32(np.pi / 2), np.float32(0))


def build(debug=False):
    nc = bacc.Bacc("TRN2", target_bir_lowering=False, debug=False, num_devices=8)

    # ---- I/O (per-core, host pre-sharded/relaid) ----
    # q/k hi fp16: [c][hf][p][t][col]; t in {qh, kh}, col = b_local*64 + e
    xh_d = nc.dram_tensor("xh", (NCHUNK, NHALF, 128, 2, HB * E), F16,
                          kind="ExternalInput")
    # q/k lo fp8 e4m3, scaled 2^12: t in {ql8, kl8}; the matching DFT matrix
    # is pre-scaled 2^-12 in fp16 (subnormal range is exact enough), so the
    # mixed fp16 x fp8 matmul accumulates into the same PSUM at scale 1.
    xl_d = nc.dram_tensor("xl", (NCHUNK, NHALF, 128, 2, HB * E), F8,
                          kind="ExternalInput")
    # packed fp16 consts: [p][fh(8*128) | fl(8*128) | g(1024) | fh12(8*128)]
    c_d = nc.dram_tensor("cp", (128, 4 * 1024), F16, kind="ExternalInput")
    # W packed fp16 (x2^16): [e][ri][x][o] = W{ri}[e, o, x]
    w_d = nc.dram_tensor("wp", (E, 2, M, O), F16, kind="ExternalInput")
    # transpose helper fp32
    idq_d = nc.dram_tensor("idq", (128, 128), F32, kind="ExternalInput")

    # out[bp][p][l]: p = (pair half)*64 + o; global b = 2*bp + (p>=64)
    out_d = nc.dram_tensor("out", (B // 2, 128, L), F16, kind="ExternalOutput")

    with tile.TileContext(nc) as tc:
        from contextlib import ExitStack
        stack = ExitStack()
        with stack:
            consts = stack.enter_context(tc.tile_pool(name="consts", bufs=1))
            chunks = stack.enter_context(tc.tile_pool(name="chunks", bufs=4))
            coeff = stack.enter_context(tc.tile_pool(name="coeff", bufs=1))
            work = stack.enter_context(tc.tile_pool(name="work", bufs=1))
            tmp = stack.enter_context(tc.tile_pool(name="tmp", bufs=1))
            outs = stack.enter_context(tc.tile_pool(name="outs", bufs=4))
            dft_ps = stack.enter_context(tc.tile_pool(name="dft_ps", bufs=1, space="PSUM"))
            tp_ps = stack.enter_context(tc.tile_pool(name="tp_ps", bufs=2, space="PSUM"))
            at_ps = stack.enter_context(tc.tile_pool(name="at_ps", bufs=2, space="PSUM"))

            # ---------- constants ----------
            c_t = consts.tile([128, 4 * 1024], F16, tag="cp")
            w_t = consts.tile([E, 2, M, O], F16, tag="w")
            idq_t = consts.tile([128, 128], F32, tag="idq")
            # pre-load ACT function set 18 (covers tanh+sin+square+copy):
            # the auto-placer is first-fit (tanh->set0, sin->set9) and would
            # otherwise thrash 1.3us table loads on every tanh<->sin switch.
            _ld = mybir.InstLoadActFuncSet(name=nc.get_next_instruction_name(), ins=[], outs=[])
            _ld.act_func_set_id = 18
            nc.scalar.add_instruction(_ld)
            nc.scalar.dma_start(out=c_t, in_=c_d[:])
            nc.scalar.dma_start(out=w_t, in_=w_d[:])
            nc.scalar.dma_start(out=idq_t, in_=idq_d[:])
            fh_t = c_t[:, 0:1024].rearrange("p (c m) -> p c m", m=2 * M)
            fl_t = c_t[:, 1024:2048].rearrange("p (c m) -> p c m", m=2 * M)
            g_t = c_t[:, 2048:3072]
            f12_t = c_t[:, 3072:4096].rearrange("p (c m) -> p c m", m=2 * M)

            # ---------- persistent state ----------
            qm_h = [coeff.tile([128, 1024], F32, tag=f"qmh{hf}", name=f"qm_h{hf}")
                    for hf in range(NHALF)]
            km_h = [coeff.tile([128, 1024], F32, tag=f"kmh{hf}", name=f"km_h{hf}")
                    for hf in range(NHALF)]
            km16_t = coeff.tile([128, B, E], F16, tag="km16")
            # layout [p=(par,e), ri, bp, y] so stationary attn1 slices merge
            # into a single contiguous free dim (BIR requirement)
            qe_h = [work.tile([128, 2, 8, 64], F32, tag=f"qeh{hf}", name=f"qe_h{hf}") for hf in range(NHALF)]
            ke_h = [work.tile([128, 2, 8, 64], F32, tag=f"keh{hf}", name=f"ke_h{hf}") for hf in range(NHALF)]
            qf_h = [work.tile([128, 2, 8, 64], F32, tag=f"qfh{hf}", name=f"qf_h{hf}") for hf in range(NHALF)]
            # A/B packed: ab[p = 64*j + y, cg, 0:64 = Re X^T, 64:128 = Im X^T]
            ab_t = work.tile([128, 16, 128], F32, tag="ab")
            halfpi = consts.tile([128, 1], F32, tag="halfpi", name="halfpi")
            nc.vector.memset(halfpi[:], float(np.pi / 2))
            t_t = work.tile([128, 16, 128], F16, tag="t")
            tf_t = work.tile([128, 16, 128], F16, tag="tf")
            tt_t = work.tile([128, B, 128], F16, tag="tt")
            tt_v = tt_t[:].rearrange("p (hg j par) c -> p hg j par c", j=2, par=2)
            y_t = work.tile([E, B, 2, M], F16, tag="y")
            yf_t = work.tile([E, B, 2, M], F16, tag="yf")
            z_t = work.tile([O, B, 2, M], F16, tag="z")
            zp_g = [work.tile([128, 8, O], F16, tag=f"zp{g}", name=f"zp_g{g}")
                    for g in range(B // 8)]
            idk16 = consts.tile([64, 64], F16, tag="id16")

            # ---------- stage 5+6, per half: complex tanh + attn2 ----------
            def tanh_half(hf):
                cgs = slice(8 * hf, 8 * hf + 8)
                # A=Re X^T, B=Im X^T, strided views of ab_t [128, 8, 64]
                av = ab_t[:, cgs, 0:64]
                bv = ab_t[:, cgs, 64:128]
                def ctt(n):
                    return tmp.tile([128, 512], F32, tag="ct", name=f"ct_{n}{hf}", bufs=6)
                def v2(t):
                    return t[:].rearrange("p (g m) -> p g m", m=64)
                ct_n = ctt("n")
                nc.vector.tensor_scalar(v2(ct_n), bv, float(1.0 / PI), float(MAGIC), OP.mult, OP.add)
                nc.vector.tensor_scalar_sub(ct_n[:], ct_n[:], float(MAGIC))
                ct_rh = ctt("rh")
                nc.vector.cody_waite_cascade(v2(ct_rh), bv, ct_n[:], float(PI_HI), float(PI_MID), float(PI_LO))
                # clamp |rh| so rh+pi/2 (cos) and 2*rh (sin) stay in [-pi, pi]
                nc.vector.tensor_scalar(ct_rh[:], ct_rh[:], -float(RH_LIM), float(RH_LIM), OP.max, OP.min)
                # T = tanh(a + ib) = (tau + i*sc*w)/ (tau^2 + cos^2(b)*w)
                # with w = 1 - tau^2; using sc = sin(2rh)/2 and
                # d = tau^2 + 2*cos^2(rh)*(1-tau^2)/2 to skip sin(rh)/s^2.
                ct_tau = ctt("tau")
                nc.scalar.activation(v2(ct_tau), av, AF.Tanh)
                ct_c = ctt("c")
                nc.scalar.activation(ct_c[:], ct_rh[:], AF.Sin, bias=halfpi[:])
                ct_sc2 = ctt("sc2")
                nc.scalar.activation(ct_sc2[:], ct_rh[:], AF.Sin, scale=2.0)
                ct_c2 = ctt("c2")
                nc.scalar.activation(ct_c2[:], ct_c[:], AF.Square)
                ct_t2 = ctt("t2")
                nc.scalar.activation(ct_t2[:], ct_tau[:], AF.Square)
                ct_w2 = ctt("w2")
                nc.vector.tensor_scalar(ct_w2[:], ct_t2[:], -0.5, 0.5, OP.mult, OP.add)
                ct_d = ctt("d")
                nc.vector.tensor_mul(ct_d[:], ct_c2[:], ct_w2[:])
                nc.vector.scalar_tensor_tensor(ct_d[:], ct_d[:], 2.0, ct_t2[:], OP.mult, OP.add)
                ct_r = ctt("r")
                nc.vector.reciprocal(ct_r[:], ct_d[:])
                ct_u = ctt("u")
                nc.vector.tensor_mul(ct_u[:], ct_sc2[:], ct_w2[:])
                # T = [Tr | Ti] fp16 ; Tf = [-Ti | Tr]   (same (j, cg) layout)
                nc.vector.tensor_mul(t_t[:, cgs, 0:64], v2(ct_tau), v2(ct_r))
                nc.vector.tensor_mul(t_t[:, cgs, 64:128], v2(ct_u), v2(ct_r))
                nc.vector.tensor_scalar_mul(tf_t[:, cgs, 0:64], t_t[:, cgs, 64:128], -1.0)
                nc.vector.tensor_copy(tf_t[:, cgs, 64:128], t_t[:, cgs, 0:64])

                # TT assembly for this half (feeds attn2, emitted later):
                # global b = 16hf + 4g4 + 2j + par lives at t[64j:64j+64, cg],
                # cg = 8hf + 2g4 + par; b factors as (hg=(hf,g4), j, par).
                # Parity-matched halves via DVE, others via SWDGE SBUF DMAs.
                hgs = slice(4 * hf, 4 * hf + 4)
                def cg_view(t):
                    return t.rearrange("p (hg par) c -> p hg par c", par=2)
                nc.vector.tensor_copy(tt_v[0:64, hgs, 0, :, :], cg_view(t_t[0:64, cgs, :]))
                nc.vector.tensor_copy(tt_v[64:128, hgs, 1, :, :], cg_view(tf_t[64:128, cgs, :]))
                nc.gpsimd.dma_start(out=tt_v[0:64, hgs, 1, :, :], in_=cg_view(t_t[64:128, cgs, :]))
                nc.gpsimd.dma_start(out=tt_v[64:128, hgs, 0, :, :], in_=cg_view(tf_t[0:64, cgs, :]))

            # ---------- attn2 (PE side), per half ----------
            # emitted in dependency-ready order: PE queues are in-order, so a
            # matmul waiting on the tanh chain must not be emitted before PE
            # work whose inputs are already available.
            def attn2_half(hf, copy_eng):
                # Y in plain global-b order
                for b4 in range(4 * hf, 4 * hf + 4):
                    yp = at_ps.tile([E, 4, 128], F32, tag="pt", bufs=2, name=f"yp{b4}")
                    for j in range(4):
                        b = b4 * 4 + j
                        nc.tensor.matmul(yp[:, j, :], km16_t[:, b, :], tt_t[:, b, :],
                                         start=True, stop=True)
                    dst = y_t[:, b4 * 4:(b4 + 1) * 4, :, :]
                    srcv = yp[:].rearrange("p b (ri m) -> p b ri m", m=M)
                    if copy_eng == "act" or b4 % 2 == 0:
                        nc.scalar.copy(dst, srcv)
                    else:
                        nc.vector.tensor_copy(dst, srcv)
                # Yf = [-Yi | Yr] for the dual-accumulate weight stage
                hb = slice(16 * hf, 16 * hf + 16)
                if copy_eng == "act":
                    nc.scalar.mul(yf_t[:, hb, 0, :], y_t[:, hb, 1, :], -1.0)
                    nc.scalar.copy(yf_t[:, hb, 1, :], y_t[:, hb, 0, :])
                else:
                    nc.vector.tensor_scalar_mul(yf_t[:, hb, 0, :], y_t[:, hb, 1, :], -1.0)
                    nc.vector.tensor_copy(yf_t[:, hb, 1, :], y_t[:, hb, 0, :])

            # ---------- stages 7-9, per half ----------
            # Zr = Wr^T Yr - Wi^T Yi ; Zi = Wr^T Yi + Wi^T Yr, via two
            # accumulating matmuls: Wr^T @ [Yr|Yi] + Wi^T @ [-Yi|Yr].
            # Then Z transposes -> Z' [(ri,x), (b, o)] and irfft out = Z'^T G.
            # PSUM comes from the transpose tag (free once transposes done).
            def stage789_half(hf, z_eng, out_eng):
                b0 = 16 * hf
                for x8 in range(M // 8):
                    wp = tp_ps.tile([O, 8, HB * 2], F32, tag="tp", bufs=2,
                                    name=f"wp{hf}_{x8}")
                    for j in range(8):
                        x = x8 * 8 + j
                        yv = y_t[:, b0:b0 + HB, :, x].rearrange("p b ri -> p (b ri)")
                        yfv = yf_t[:, b0:b0 + HB, :, x].rearrange("p b ri -> p (b ri)")
                        nc.tensor.matmul(wp[:, j, :], w_t[:, 0, x, :], yv,
                                         start=True, stop=False)
                        nc.tensor.matmul(wp[:, j, :], w_t[:, 1, x, :], yfv,
                                         start=False, stop=True)
                    dst = z_t[:, b0:b0 + HB, :, x8 * 8:(x8 + 1) * 8].rearrange("p b ri x -> p x b ri")
                    srcv = wp[:].rearrange("p x (b ri) -> p x b ri", ri=2)
                    if z_eng == "act" or x8 % 2 == 0:
                        nc.scalar.copy(dst, srcv)
                    else:
                        nc.vector.tensor_copy(dst, srcv)

                for b8 in range(2 * hf, 2 * hf + 2):
                    zt = tp_ps.tile([128, 8, O], F16, tag="tp", bufs=2,
                                    name=f"zt{b8}")
                    for j in range(8):
                        b = b8 * 8 + j
                        nc.tensor.transpose(
                            zt[:, j, :],
                            z_t[:, b, :, :].rearrange("p ri m -> p (ri m)"),
                            idk16[:],
                        )
                    if z_eng == "act" or b8 % 2 == 1:
                        nc.scalar.copy(zp_g[b8][:], zt[:])
                    else:
                        nc.vector.tensor_copy(zp_g[b8][:], zt[:])

                # irfft + staged fp16 output (host applies OUT_SCALE;
                # fp16 can't hold out*2^-40 without underflow)
                for bp in range(8 * hf, 8 * hf + 8):
                    otg = outs.tile([128, 1024], F16, tag="ot", name=f"ot{bp}")
                    for g in range(2):
                        opg = dft_ps.tile([128, 512], F32,
                                          tag=("qmps" if (bp + g) % 2 == 0 else "kmps"),
                                          bufs=1, name=f"op{bp}_{g}")
                        nc.tensor.matmul(
                            opg[:, :],
                            zp_g[bp // 4][:, (bp % 4) * 2:(bp % 4) * 2 + 2, :]
                            .rearrange("p b o -> p (b o)"),
                            g_t[:, g * 512:(g + 1) * 512],
                            start=True, stop=True,
                        )
                        if out_eng == "act" or (bp + g) % 2 == 0:
                            nc.scalar.copy(otg[:, g * 512:(g + 1) * 512], opg[:])
                        else:
                            nc.vector.tensor_copy(otg[:, g * 512:(g + 1) * 512], opg[:])
                    nc.sync.dma_start(out=out_d[bp], in_=otg[:])

            # ---------- main per-half pipeline ----------
            def dft_half(hf):
                # ----- stage 1+2: DFT (hi/lo 3-pass) -----
                qm_ps = dft_ps.tile([128, 1024], F32, tag="qmps", name=f"qm_ps{hf}", bufs=1)
                km_ps = dft_ps.tile([128, 1024], F32, tag="kmps", name=f"km_ps{hf}", bufs=1)
                for c in range(NCHUNK):
                    xh_c = chunks.tile([128, 2, HB * E], F16, tag="xh", name=f"xh{hf}_{c}")
                    xl_c = chunks.tile([128, 2, HB * E], F8, tag="xl", name=f"xl{hf}_{c}")
                    nc.sync.dma_start(out=xh_c, in_=xh_d[c, hf])
                    nc.sync.dma_start(out=xl_c, in_=xl_d[c, hf])
                    first = c == 0
                    last = c == NCHUNK - 1
                    passes = (
                        (fh_t[:, c, :], xh_c, 0, qm_ps, first, False),
                        (fh_t[:, c, :], xh_c, 1, km_ps, first, False),
                        (fl_t[:, c, :], xh_c, 0, qm_ps, False, False),
                        (fl_t[:, c, :], xh_c, 1, km_ps, False, False),
                        (f12_t[:, c, :], xl_c, 0, qm_ps, False, last),
                        (f12_t[:, c, :], xl_c, 1, km_ps, False, last),
                    )
                    for lhs, src, ti, ps, is_start, is_stop in passes:
                        for g in range(2):
                            nc.tensor.matmul(
                                ps[:, g * 512:(g + 1) * 512],
                                lhs,
                                src[:, ti, g * 512:(g + 1) * 512],
                                start=is_start,
                                stop=is_stop,
                            )
                nc.vector.tensor_copy(qm_h[hf][:], qm_ps[:])
                nc.scalar.copy(km_h[hf][:], km_ps[:])
                nc.vector.tensor_copy(
                    km16_t[:, hf * HB:(hf + 1) * HB, :],
                    km_ps[:].rearrange("p (b e) -> p b e", e=E),
                )
                if hf == 0:
                    nc.vector.tensor_copy(idk16[:], idq_t[0:64, 0:64])

            def tr_half(hf):
                # ----- stage 3: pair transposes -> Q_e, K_e -----
                # in [2m, (b0-e|b1-e)] -> out [(b0-e|b1-e), 2m]; even b on
                # partitions 0:64, odd on 64:128.
                qm_p = qm_h[hf][:].rearrange("p (bp c) -> p bp c", c=128)
                km_p = km_h[hf][:].rearrange("p (bp c) -> p bp c", c=128)
                for g2 in range(4):
                    tp = tp_ps.tile([128, 2, 128], F32, tag="tp", name=f"tp{hf}_{g2}")
                    tk = tp_ps.tile([128, 2, 128], F32, tag="tp", name=f"tk{hf}_{g2}")
                    for j in range(2):
                        bpl = g2 * 2 + j
                        nc.tensor.transpose(tp[:, j, :], qm_p[:, bpl, :], idq_t[:])
                        nc.tensor.transpose(tk[:, j, :], km_p[:, bpl, :], idq_t[:])
                    tpv = tp[:].rearrange("p j (ri y) -> p ri j y", ri=2)
                    tkv = tk[:].rearrange("p j (ri y) -> p ri j y", ri=2)
                    if g2 % 2 == 0:
                        nc.scalar.copy(qe_h[hf][:, :, g2 * 2:(g2 + 1) * 2, :], tpv)
                        nc.scalar.copy(ke_h[hf][:, :, g2 * 2:(g2 + 1) * 2, :], tkv)
                    else:
                        nc.vector.tensor_copy(qe_h[hf][:, :, g2 * 2:(g2 + 1) * 2, :], tpv)
                        nc.vector.tensor_copy(ke_h[hf][:, :, g2 * 2:(g2 + 1) * 2, :], tkv)
                nc.vector.tensor_scalar_mul(qf_h[hf][:, 0, :, :], qe_h[hf][:, 1, :, :], -1.0)
                nc.vector.tensor_copy(qf_h[hf][:, 1, :, :], qe_h[hf][:, 0, :, :])

            def attn1_half(hf):
                # ----- stage 4: attn1 -> X^T psum, A/B fp32 sbuf -----
                # 2 same-parity b per matmul pair (256 cols each); useful
                # quadrants j == j'; partition-aligned extraction.
                for par in range(2):
                    base = 64 * par
                    sl = slice(base, base + 64)
                    for g4 in range(4):
                        pt = at_ps.tile([128, 2, 2, 64], F32, tag="pt", bufs=2,
                                        name=f"pt{hf}_{par}_{g4}")
                        psl = slice(2 * g4, 2 * g4 + 2)
                        nc.tensor.matmul(pt[:], ke_h[hf][sl, 0, psl, :],
                                         qe_h[hf][sl, :, psl, :],
                                         start=True, stop=False)
                        nc.tensor.matmul(pt[:], ke_h[hf][sl, 1, psl, :],
                                         qf_h[hf][sl, :, psl, :],
                                         start=False, stop=True)
                        cg = 8 * hf + 2 * g4 + par
                        if (par + g4) % 2 == 0:
                            nc.scalar.copy(ab_t[0:64, cg, :].rearrange("p (ri y) -> p ri y", ri=2), pt[0:64, :, 0, :])
                            nc.vector.tensor_copy(ab_t[64:128, cg, :].rearrange("p (ri y) -> p ri y", ri=2), pt[64:128, :, 1, :])
                        else:
                            nc.vector.tensor_copy(ab_t[0:64, cg, :].rearrange("p (ri y) -> p ri y", ri=2), pt[0:64, :, 0, :])
                            nc.scalar.copy(ab_t[64:128, cg, :].rearrange("p (ri y) -> p ri y", ri=2), pt[64:128, :, 1, :])

            # Emission order = PE dependency-ready order (PE queues are
            # in-order; a stalled head blocks everything behind it).
            dft_half(0)
            tr_half(0)
            attn1_half(0)
            tanh_half(0)
            dft_half(1)
            attn2_half(0, "act")     # fills the DFT(1)->tr(1) psum-copy gap
            tr_half(1)
            attn1_half(1)
            tanh_half(1)
            stage789_half(0, "act", "mix")   # fills tanh(1)'s PE-idle window
            attn2_half(1, "mix")
            stage789_half(1, "mix", "mix")

    nc.compile()
    return nc


_NC_CACHE = None


def _get_nc():
    global _NC_CACHE
    if _NC_CACHE is None:
        _NC_CACHE = build()
    return _NC_CACHE


def _host_prep(q, k, Wr, Wi):
    """Build the 8 per-core input maps (numpy relayout/cast only)."""
    l = np.arange(L, dtype=np.float64)[:, None]
    m = np.arange(M, dtype=np.float64)[None, :]
    ang = 2.0 * np.pi * l * m / L
    F = np.concatenate([np.cos(ang), -np.sin(ang)], axis=1).astype(np.float32)  # [L, 2M]
    fh = F.astype(np.float16)
    fl = (F - fh.astype(np.float32)).astype(np.float16)
    # fh/fl as [p][(c, 2m)]
    fh = fh.reshape(NCHUNK, 128, 2 * M).transpose(1, 0, 2).reshape(128, 1024)
    fl = fl.reshape(NCHUNK, 128, 2 * M).transpose(1, 0, 2).reshape(128, 1024)

    cm = np.full(M, 2.0); cm[0] = 1.0
    ang2 = 2.0 * np.pi * m.T * np.arange(L, dtype=np.float64)[None, :] / L
    SC = 2.0 ** GSHIFT / (L * 512.0 * 512.0)
    g = np.concatenate([
        cm[:, None] * np.cos(ang2) * SC,
        -cm[:, None] * np.sin(ang2) * SC,
    ], axis=0).astype(np.float32).astype(np.float16)  # [2M, L]

    f12 = (F * 2.0 ** -12).astype(np.float16)
    f12 = f12.reshape(NCHUNK, 128, 2 * M).transpose(1, 0, 2).reshape(128, 1024)
    cpack = np.concatenate([fh, fl, g.astype(np.float16), f12], axis=1)  # [128, 4096]

    idq = np.eye(128, dtype=np.float32)

    from ml_dtypes import float8_e4m3fn as E4M3
    maps = []
    for h in range(H):
        def split(x):
            xs = np.ascontiguousarray(x[:, :, h, :].transpose(1, 0, 2)).reshape(L, B * E)
            hi = xs.astype(np.float16)
            lo = ((xs - hi.astype(np.float32)) * 2.0 ** 12).astype(E4M3)
            return hi, lo
        qh, ql8 = split(q)
        kh, kl8 = split(k)
        # pack [c][hf][p][t][col]
        xph = np.empty((NCHUNK, NHALF, 128, 2, HB * E), np.float16)
        xpl = np.empty((NCHUNK, NHALF, 128, 2, HB * E), E4M3)
        for t, (dst, src) in enumerate((( xph, qh), (xph, kh), (xpl, ql8), (xpl, kl8))):
            sv = src.reshape(NCHUNK, 128, NHALF, HB * E)
            dst[:, :, :, t % 2, :] = sv.transpose(0, 2, 1, 3)
        wpk = np.empty((E, 2, M, O), np.float32)
        wpk[:, 0] = (Wr[h] * 2.0 ** WSHIFT).transpose(0, 2, 1)  # [e,o,x]->[e,x,o]
        wpk[:, 1] = (Wi[h] * 2.0 ** WSHIFT).transpose(0, 2, 1)
        maps.append({
            "xh": xph,
            "xl": xpl,
            "cp": cpack,
            "wp": wpk.astype(np.float16),
            "idq": idq,
        })
    return maps


def kernel(q, k, v, Wr, Wi, _trace=False):
    q = np.asarray(q, np.float32)
    k = np.asarray(k, np.float32)
    Wr = np.asarray(Wr, np.float32)
    Wi = np.asarray(Wi, np.float32)
    nc = _get_nc()
    maps = _host_prep(q, k, Wr, Wi)
    try:
        res = run_bass_kernel_spmd(nc, maps, core_ids=list(range(H)), trace=_trace)
    except ModuleNotFoundError:
        res = run_bass_kernel_spmd(nc, maps, core_ids=list(range(H)), trace=False)
    # out_d[bp][p][l]: b = 2*bp + (p//64), o = p%64 -> plain b order
    out = np.empty((B, H, O, L), np.float32)
    for h in range(H):
        o = np.asarray(res.results[h]["out"], np.float32).reshape(B, O, L)
        o *= np.float32(OUT_SCALE)
        out[:, h] = o
    if _trace:
        kernel.last_results = res
    return out.astype(np.float32)
